# revision 1
# baseline (speedup 1.0000x reference)
"""Fused transformer encoder layer (attention w/ 2D-ALiBi bias + FFN) on 8 trn2 cores.

Sharding: core c handles batch b = c//2, token half h = c%2 (512 query rows).
K/V are computed per-core for the full 1024-token sequence of its batch
(duplicated across the 2 cores sharing a batch); outputs are disjoint row
slices of the final tensor, so no collectives are needed.

Bias trick: the alibi_2d bias slope_h*(|xi-xj|+|yi-yj|) is folded into the
QK^T contraction. |xi-xj| = xi + xj - 2*a_i.a_j with a_i in {0,1}^31 the
threshold indicators of xi, so dist(i,j) = s_i + s_j - 2*c_i.c_j (c = 62-dim
indicator, s = x+y). The per-query term slope*s_i is constant along the
softmax axis and is dropped. Q/K are augmented with 64 extra contraction dims
(s_j / pad / c_j on the K side; 1 / 0 / -2*c_i on the Q side), making the
score contraction K = 64+64 = 128 exactly — full PE array, bias for free.

bf16 precision care: the aug rows are small integers / {0,-2} — exact in
bf16. The attention scale AND the per-head slope are folded out of the bf16
data: Q-projection weights carry scale/slope_h per head (so scores come out
as S/slope_h) and the exact fp32 slope_h is re-applied as the exp()
activation's scale immediate. exp needs no max-subtraction (|S| <= ~50 by
construction).

Scores are computed keys-on-partitions (S^T layout) so the exp() output is
already P^T for the AV matmul (no transpose). Softmax denominators come from
an appended ones-column in V; normalization is deferred past the (linear)
output projection boundary: each head's O^T rows are scaled by a broadcasted
1/den (built with a small fp32 selector matmul) before the head-summing
projection.
"""

import math
import sys
import time

for _p in ("/opt/trn_rl_repo",):
    if _p not in sys.path:
        sys.path.insert(0, _p)

import numpy as np
import ml_dtypes

import concourse.bass as bass
import concourse.tile as tile
from concourse import bacc, mybir
from concourse.masks import make_identity

F32 = mybir.dt.float32
F32R = mybir.dt.float32r
BF16 = mybir.dt.bfloat16
F8 = mybir.dt.float8e4
BF = ml_dtypes.bfloat16
F8NP = ml_dtypes.float8_e4m3
DR = mybir.MatmulPerfMode.DoubleRow

# fp8 FFN config: activations split into fp8 hi+lo planes (quantization error
# feedback), weights plain fp8. Splitting halves the DoubleRow speedup for
# that operand but removes its quantization error from the output.
FFN_X_SPLIT = True   # x1T (FFN1 input) hi/lo
FFN_H_SPLIT = True   # h1T (FFN2 input) hi/lo

# fp8e4m3 normals span [2^-6, 448]; W1/W2 entries (sigma ~ 1/32) and the lo
# planes would otherwise land in the subnormal range and lose most precision,
# so everything is pre-scaled up into the normal range and the product scale
# is divided back out at the PSUM->SBUF boundary (gelu scale / y descale).
W1_PRESCALE = 256.0
W2_PRESCALE = 256.0
X_PRESCALE = 16.0
H_PRESCALE = 32.0
S_PRESCALE = 8.0     # src (sigma 1) for the QKV projections
WP_PRESCALE = 256.0  # Wq/Wk/Wv/Wo (sigma 1/32)
OT_PRESCALE = 16.0   # normalized attention outputs (sigma ~1)

D = 1024          # d_model
H = 16            # heads
HD = 64           # head dim
DFF = 4096
B = 4
N = 1024          # sequence length
NT = 512          # tokens (query rows) per core
GRID = 32
EPS = 1e-5
NCORES = 8
SCALE = HD ** -0.5


def _alibi_slopes(n):
    def pow2(n_):
        start = 2.0 ** (-(2.0 ** -(math.log2(n_) - 3)))
        return [start * start ** i for i in range(n_)]
    if math.log2(n).is_integer():
        return np.array(pow2(n), dtype=np.float64)
    m = 2 ** math.floor(math.log2(n))
    s = pow2(m)
    s += [s[-1] * 0.5 ** (i + 1) for i in range(n - m)]
    return np.array(s, dtype=np.float64)


SLOPES = _alibi_slopes(H)


def build_nc(trivial_affine=False):
    """trivial_affine: g1/g2 all-ones and be1/be2/b2 all-zeros -> skip those ops."""
    nc = bacc.Bacc()

    srcT8 = nc.declare_dram_parameter("srcT8", [2, D, N], F8, isOutput=False)
    srcQT8 = nc.declare_dram_parameter("srcQT8", [2, D, NT], F8, isOutput=False)
    src_rows = nc.declare_dram_parameter("src_rows", [NT, D], F32, isOutput=False)
    WqT8 = nc.declare_dram_parameter("WqT8", [2, D, D], F8, isOutput=False)
    WkT8 = nc.declare_dram_parameter("WkT8", [2, D, D], F8, isOutput=False)
    WvT8 = nc.declare_dram_parameter("WvT8", [2, D, D], F8, isOutput=False)
    WoT8 = nc.declare_dram_parameter("WoT8", [2, D, D], F8, isOutput=False)
    # W1 pre-swizzled on host: W1S[w, ft, p, dc*128+j] = W1.T[dc*128+p, ft*128+j]
    # (w = fp8 hi/lo plane) so each FFN1 weight block is contiguous per plane.
    W1S = nc.declare_dram_parameter("W1S", [2, 32, 128, D], F8, isOutput=False)
    W2Tb = nc.declare_dram_parameter("W2Tb", [2, DFF, D], F8, isOutput=False)
    kaug_x = nc.declare_dram_parameter("kaug_x", [64, N], BF16, isOutput=False)
    qaug_x = nc.declare_dram_parameter("qaug_x", [H, 64, NT], BF16, isOutput=False)
    b1r = nc.declare_dram_parameter("b1r", [128, 32], F32, isOutput=False)
    b2 = nc.declare_dram_parameter("b2", [1, D], F32, isOutput=False)
    g1 = nc.declare_dram_parameter("g1", [1, D], F32, isOutput=False)
    be1 = nc.declare_dram_parameter("be1", [1, D], F32, isOutput=False)
    g2 = nc.declare_dram_parameter("g2", [1, D], F32, isOutput=False)
    be2 = nc.declare_dram_parameter("be2", [1, D], F32, isOutput=False)
    out = nc.declare_dram_parameter("out", [NT, D], F32, isOutput=True)

    AF = mybir.ActivationFunctionType
    OP = mybir.AluOpType

    with tile.TileContext(nc) as tc:
        with (
            tc.tile_pool(name="misc", bufs=1) as misc,
            tc.tile_pool(name="lnp", bufs=4) as lnp,
            tc.tile_pool(name="pre", bufs=1) as pre,
        ):
            eps_sb = misc.tile([128, 1], F32, tag="eps")
            nc.vector.memset(eps_sb, EPS)
            ident = misc.tile([128, 128], F32, tag="ident")
            make_identity(nc, ident)
            # OT8[p, c, q]: head 2c in partitions 0:64, head 2c+1 in 64:128;
            # fp8 hi/lo planes (scaled by OT_PRESCALE) for the 3-term out-proj
            OT8_hi = misc.tile([128, 8, NT], F8, tag="ot8h")
            OT8_lo = misc.tile([128, 8, NT], F8, tag="ot8l")
            # single ones row at partition 64 for the 1/den broadcast matmul
            # (memset can't write f32r; round through a f32 staging row)
            ones64 = misc.tile([65, 128], F32R, tag="ones64")
            ones_f = misc.tile([65, 128], F32, tag="ones_f")
            nc.vector.memset(ones_f[64:65, :], 1.0)
            with nc.allow_low_precision(reason="exact 1.0 fits f32r"):
                nc.vector.tensor_copy(out=ones64[64:65, :], in_=ones_f[64:65, :])

            def ln_apply(x_ap, gbc, bbc):
                stats = lnp.tile([128, 2, 6], F32, tag="lnstats", name="lnstats")
                for sg in range(2):
                    nc.vector.bn_stats(
                        out=stats[:, sg, :], in_=x_ap[:, sg * 512 : sg * 512 + 512]
                    )
                mv = lnp.tile([128, 2], F32, tag="lnmv", name="lnmv")
                nc.vector.bn_aggr(out=mv, in_=stats)
                nc.scalar.activation(
                    out=mv[:, 1:2], in_=mv[:, 1:2], func=AF.Sqrt,
                    bias=eps_sb, scale=1.0,
                )
                nc.vector.reciprocal(out=mv[:, 1:2], in_=mv[:, 1:2])
                nc.vector.tensor_scalar(
                    out=x_ap, in0=x_ap,
                    scalar1=mv[:, 0:1], scalar2=mv[:, 1:2],
                    op0=OP.subtract, op1=OP.mult,
                )
                if gbc is not None:
                    nc.vector.tensor_mul(out=x_ap, in0=x_ap, in1=gbc)
                if bbc is not None:
                    nc.vector.tensor_add(out=x_ap, in0=x_ap, in1=bbc)

            # ============ attention scope ============
            with tc.tile_pool(name="att", bufs=1) as att:
                kaug = att.tile([128, H, N], BF16, tag="kaug")
                qaug = att.tile([128, H, NT], BF16, tag="qaug")
                v_sb = att.tile([128, 8, H * 65], BF16, tag="vsb")
                v4 = v_sb.rearrange("p m (h w) -> p m h w", w=65)
                nc.vector.memset(v4[:, :, :, 64], 1.0)

                # --- phase 1: projections (fp8 3-term DoubleRow) ---
                # DMA emission order tracks first-use order so the PE can
                # start as soon as the Q operands land.
                P_DS = 1.0 / (S_PRESCALE * WP_PRESCALE)
                # (activation plane, weight plane) product terms; lo*lo skipped
                TERMS = ((0, 0), (1, 0), (0, 1))

                def mm3t(ps, w8, x8, wslice, xslice):
                    i = 0
                    for a, b in TERMS:
                        for dr in range(4):
                            pl = slice(2 * dr, 2 * dr + 2)
                            nc.tensor.matmul(
                                ps,
                                w8[:, b, pl, wslice],
                                x8[:, a, pl, xslice],
                                start=(i == 0), stop=(i == 11),
                                perf_mode=DR,
                            )
                            i += 1

                # HWDGE issues ~1 descriptor / 625ns, so DMA COUNT (not bytes)
                # gates the start: batch everything into few large transfers.
                with tc.tile_pool(name="ph1", bufs=1) as ph1:
                    sqt = ph1.tile([128, 2, 8, NT], F8, tag="sqt")
                    sq_vw = srcQT8[:, :, :].rearrange("w (c p) n -> p w c n", p=128)
                    wqf = ph1.tile([128, 2, 8, D], F8, tag="wqf")
                    wq_vw = WqT8[:, :, :].rearrange("w (c p) n -> p w c n", p=128)
                    for w in range(2):
                        nc.sync.dma_start(out=sqt[:, w, :, :], in_=sq_vw[:, w, :, :])
                        nc.sync.dma_start(out=wqf[:, w, :, :], in_=wq_vw[:, w, :, :])
                    stf = ph1.tile([128, 2, 8, N], F8, tag="stf")
                    st_vw = srcT8[:, :, :].rearrange("w (c p) n -> p w c n", p=128)
                    wkf = ph1.tile([128, 2, 8, D], F8, tag="wkf")
                    wk_vw = WkT8[:, :, :].rearrange("w (c p) n -> p w c n", p=128)
                    for w in range(2):
                        nc.sync.dma_start(out=stf[:, w, :, :], in_=st_vw[:, w, :, :])
                        nc.sync.dma_start(out=wkf[:, w, :, :], in_=wk_vw[:, w, :, :])
                    # aug rows: one DMA into head slot 0 + log2 doubling
                    # on-chip; qaug: a single strided DMA covers all heads
                    nc.sync.dma_start(out=kaug[64:128, 0, :], in_=kaug_x[:, :])
                    nc.sync.dma_start(
                        out=qaug[64:128, :, :],
                        in_=qaug_x[:, :, :].rearrange("h p n -> p h n"),
                    )
                    wvf = ph1.tile([128, 2, 8, D], F8, tag="wvf")
                    wv_vw = WvT8[:, :, :].rearrange("w (c p) n -> p w c n", p=128)
                    for w in range(2):
                        nc.sync.dma_start(out=wvf[:, w, :, :], in_=wv_vw[:, w, :, :])
                    for g in range(4):
                        n_ = 1 << g
                        nc.sync.dma_start(
                            out=kaug[64:128, n_ : 2 * n_, :],
                            in_=kaug[64:128, 0:n_, :],
                        )

                    # Q + K projections -> qaug/kaug top halves (shared pool)
                    with tc.tile_pool(name="psQK", bufs=3, space="PSUM") as psQK:
                        for dt in range(8):
                            qps = psQK.tile([128, NT], F32, tag="proj", name="qps")
                            mm3t(qps, wqf, sqt,
                                 slice(dt * 128, dt * 128 + 128), slice(None))
                            nc.scalar.activation(
                                out=qaug[0:64, 2 * dt, :], in_=qps[0:64, :],
                                func=AF.Copy,
                                scale=float(SCALE / SLOPES[2 * dt] * P_DS),
                            )
                            nc.vector.tensor_scalar_mul(
                                out=qaug[0:64, 2 * dt + 1, :], in0=qps[64:128, :],
                                scalar1=float(SCALE / SLOPES[2 * dt + 1] * P_DS),
                            )
                        for dt in range(8):
                            for mh in range(2):
                                kps = psQK.tile([128, 512], F32, tag="proj", name="kps")
                                mm3t(kps, wkf, stf,
                                     slice(dt * 128, dt * 128 + 128),
                                     slice(mh * 512, mh * 512 + 512))
                                nc.scalar.activation(
                                    out=kaug[0:64, 2 * dt, mh * 512 : mh * 512 + 512],
                                    in_=kps[0:64, :], func=AF.Copy, scale=P_DS,
                                )
                                nc.vector.tensor_scalar_mul(
                                    out=kaug[0:64, 2 * dt + 1, mh * 512 : mh * 512 + 512],
                                    in0=kps[64:128, :], scalar1=P_DS,
                                )

                    # V projection (natural layout, + ones col); 4-bank groups
                    # so the last PSUM banks free quickly before attention
                    with tc.tile_pool(name="psV", bufs=1, space="PSUM") as psV:
                        for dh in range(2):
                            for mg in range(2):
                                vps = [
                                    psV.tile([128, 512], F32, tag=f"vps{m}", name=f"vps{m}")
                                    for m in range(4)
                                ]
                                for a, b in TERMS:
                                    for dr in range(4):
                                        pl = slice(2 * dr, 2 * dr + 2)
                                        for lm in range(4):
                                            mt = mg * 4 + lm
                                            nc.tensor.matmul(
                                                vps[lm],
                                                stf[:, a, pl, mt * 128 : mt * 128 + 128],
                                                wvf[:, b, pl, dh * 512 : dh * 512 + 512],
                                                start=(a == 0 and b == 0 and dr == 0),
                                                stop=(b == 1 and dr == 3),
                                                perf_mode=DR,
                                            )
                                for lm in range(4):
                                    vdst = v4[:, mg * 4 + lm, dh * 8 : dh * 8 + 8, 0:64]
                                    vsrc = vps[lm].rearrange("p (h w) -> p h w", w=64)
                                    if lm % 2 == 0:
                                        nc.scalar.activation(
                                            out=vdst, in_=vsrc, func=AF.Copy,
                                            scale=P_DS,
                                        )
                                    else:
                                        nc.vector.tensor_scalar_mul(
                                            out=vdst, in0=vsrc, scalar1=P_DS
                                        )

                # --- phase 2: attention, 3-stage pipeline:
                #     QKA(h) | QKB(h-1) | AV(h-2) (+ per-head normalize) ---
                # Prefetch everything phases 3-5 need (except streamed W1/W2)
                # while the DMA queues are otherwise idle during attention.
                if trivial_affine:
                    g1bc = be1bc = b2bc = g2bc = be2bc = None
                else:
                    g1bc = pre.tile([128, D], F32, tag="g1bc")
                    be1bc = pre.tile([128, D], F32, tag="be1bc")
                    b2bc = pre.tile([128, D], F32, tag="b2bc")
                    g2bc = pre.tile([128, D], F32, tag="g2bc")
                    be2bc = pre.tile([128, D], F32, tag="be2bc")
                    for t_, src_ in (
                        (g1bc, g1), (be1bc, be1),
                        (b2bc, b2), (g2bc, g2), (be2bc, be2),
                    ):
                        nc.sync.dma_start(
                            out=t_, in_=src_[:, :].to_broadcast([128, D])
                        )
                srar = pre.tile([128, 4, D], F32, tag="srcrows")
                nc.sync.dma_start(
                    out=srar,
                    in_=src_rows[:, :].rearrange("(nt p) d -> p nt d", p=128),
                )
                wof = pre.tile([128, 2, 8, D], F8, tag="wof")
                wo_vw = WoT8[:, :, :].rearrange("w (c p) n -> p w c n", p=128)
                for w in range(2):
                    nc.sync.dma_start(out=wof[:, w, :, :], in_=wo_vw[:, w, :, :])
                b1_sb = pre.tile([128, 32], F32, tag="b1")
                nc.sync.dma_start(out=b1_sb, in_=b1r[:, :])
                with (
                    tc.tile_pool(name="ptp", bufs=3) as ptp,
                    tc.tile_pool(name="stgp", bufs=2) as stgp,
                    tc.tile_pool(name="otnp", bufs=2) as otnp,
                    tc.tile_pool(name="psS", bufs=3, space="PSUM") as psS,
                    tc.tile_pool(name="psOT", bufs=1, space="PSUM") as psOT,
                    tc.tile_pool(name="psD", bufs=1, space="PSUM") as psD,
                ):
                    # Two-stage pipeline at 2-key-tile granularity: per step,
                    # 4x [2 score MMs -> exp chunk] for head h interleaved
                    # with the 8 AV MMs for head h-1, then normalize h-1.
                    # Score PSUM rotates through 3 chunk buffers (6 banks) so
                    # the PE never waits for the exp of the previous chunk.
                    pts = {}
                    otps = {}
                    for step in range(H + 1):
                        h = step
                        hp = step - 1
                        if h < H:
                            pt = ptp.tile([128, 8, NT], BF16, tag="pt", name="pt")
                            pts[h] = pt
                        if hp >= 0:
                            otps[hp] = psOT.tile([65, NT], F32, tag="ot", name="otp")
                        for c in range(4):
                            if h < H:
                                stc = psS.tile(
                                    [128, 2, NT], F32, tag="st", name="stc"
                                )
                                for j in range(2):
                                    mt = 2 * c + j
                                    nc.tensor.matmul(
                                        stc[:, j, :],
                                        kaug[:, h, mt * 128 : mt * 128 + 128],
                                        qaug[:, h, :],
                                        start=True, stop=True,
                                    )
                                nc.scalar.activation(
                                    out=pt[:, 2 * c : 2 * c + 2, :], in_=stc,
                                    func=AF.Exp, scale=float(SLOPES[h]),
                                )
                            if hp >= 0:
                                for j in range(2):
                                    mt = 2 * c + j
                                    nc.tensor.matmul(
                                        otps[hp],
                                        v_sb[:, mt, hp * 65 : hp * 65 + 65],
                                        pts[hp][:, mt, :],
                                        start=(mt == 0), stop=(mt == 7),
                                    )
                        if hp >= 0:
                            pts.pop(hp)
                            otp = otps.pop(hp)
                            base = (hp % 2) * 64
                            cc = hp // 2
                            bsl = slice(base, base + 64)
                            # per-head normalize: 1/den at partition 64, f32r
                            # selector matmul broadcasts it to all partitions
                            stg = stgp.tile([65, NT], F32R, tag="stg", name="stg")
                            with nc.allow_low_precision(
                                reason="f32r 1/den broadcast keeps ~19 mantissa bits"
                            ):
                                nc.vector.reciprocal(
                                    out=stg[64:65, :], in_=otp[64:65, :]
                                )
                            dbc = psD.tile([128, NT], F32, tag="dbc", name="dbc")
                            nc.tensor.matmul(
                                dbc, ones64[64:65, :], stg[64:65, :],
                                start=True, stop=True,
                            )
                            otn = otnp.tile([128, NT], F32, tag="otn", name="otn")
                            nc.vector.tensor_scalar_mul(
                                out=otn[bsl, :], in0=otp[0:64, :],
                                scalar1=OT_PRESCALE,
                            )
                            otf = otnp.tile([128, NT], F32, tag="otf", name="otf")
                            nc.vector.tensor_mul(
                                out=otf[bsl, :], in0=otn[bsl, :], in1=dbc[bsl, :]
                            )
                            nc.gpsimd.tensor_copy(
                                out=OT8_hi[bsl, cc, :], in_=otf[bsl, :]
                            )
                            nc.vector.tensor_sub(
                                out=OT8_lo[bsl, cc, :],
                                in0=otf[bsl, :], in1=OT8_hi[bsl, cc, :],
                            )

            # ============ post-attention scope ============
            with tc.tile_pool(name="ffn", bufs=1) as ffn:
                W2_sb = ffn.tile([128, 2, 32, D], F8, tag="w2")
                w2_v = W2Tb[:, :, :].rearrange("w (c p) n -> p w c n", p=128)
                x1_sb = ffn.tile([128, 4, D], F32, tag="x1")
                x1T_hi = ffn.tile([128, 8, NT], F8, tag="x1Th")
                x1T_lo = (
                    ffn.tile([128, 8, NT], F8, tag="x1Tl", name="x1T_lo") if FFN_X_SPLIT else None
                )

                # --- phase 3: out-proj, LN1, transpose ---
                with (
                    tc.tile_pool(name="psS2", bufs=2, space="PSUM") as psS2,
                    tc.tile_pool(name="psT", bufs=2, space="PSUM") as psT,
                ):
                    def transposes(nt):
                        for c in range(8):
                            tp = psT.tile([128, 128], F32, tag="tp", name="tp")
                            nc.tensor.transpose(
                                tp, x1_sb[:, nt, c * 128 : c * 128 + 128], ident
                            )
                            hi = x1T_hi[:, c, nt * 128 : nt * 128 + 128]
                            nc.scalar.activation(
                                out=hi, in_=tp, func=AF.Copy, scale=X_PRESCALE
                            )
                            if FFN_X_SPLIT:
                                nc.vector.scalar_tensor_tensor(
                                    out=x1T_lo[:, c, nt * 128 : nt * 128 + 128],
                                    in0=tp, scalar=X_PRESCALE, in1=hi,
                                    op0=OP.mult, op1=OP.subtract,
                                )

                    OT8 = (OT8_hi, OT8_lo)
                    O_DS = 1.0 / (OT_PRESCALE * WP_PRESCALE)
                    for nt in range(4):
                        for dh in range(2):
                            s2 = psS2.tile([128, 512], F32, tag="s2", name="s2")
                            i = 0
                            for a, b in TERMS:
                                for dr in range(4):
                                    pl = slice(2 * dr, 2 * dr + 2)
                                    nc.tensor.matmul(
                                        s2,
                                        OT8[a][:, pl, nt * 128 : nt * 128 + 128],
                                        wof[:, b, pl, dh * 512 : dh * 512 + 512],
                                        start=(i == 0), stop=(i == 11),
                                        perf_mode=DR,
                                    )
                                    i += 1
                            nc.vector.scalar_tensor_tensor(
                                out=x1_sb[:, nt, dh * 512 : dh * 512 + 512],
                                in0=s2, scalar=O_DS,
                                in1=srar[:, nt, dh * 512 : dh * 512 + 512],
                                op0=OP.mult, op1=OP.add,
                            )
                        if nt >= 1:
                            transposes(nt - 1)
                        ln_apply(x1_sb[:, nt, :], g1bc, be1bc)
                    transposes(3)

                # --- phase 4: FFN1 (fp8 DoubleRow matmuls, gelu into fp8 h1T) ---
                h1T_hi = ffn.tile([128, 32, NT], F8, tag="h1Th")
                h1T_lo = (
                    ffn.tile([128, 32, NT], F8, tag="h1Tl", name="h1T_lo") if FFN_H_SPLIT else None
                )
                with (
                    tc.tile_pool(name="w1p", bufs=3) as w1p,
                    tc.tile_pool(name="h1gp", bufs=3) as h1gp,
                    tc.tile_pool(name="psH", bufs=3, space="PSUM") as psH,
                ):
                    # term list: (x plane, w plane); lo*lo is negligible
                    x_terms = [(x1T_hi, 0), (x1T_lo, 0), (x1T_hi, 1)] \
                        if FFN_X_SPLIT else [(x1T_hi, 0), (x1T_hi, 1)]
                    for ft in range(32):
                        w1 = w1p.tile([128, 2, 8, 128], F8, tag="w1col", name="w1")
                        nc.sync.dma_start(
                            out=w1.rearrange("p w c n -> p w (c n)"),
                            in_=W1S[:, ft, :, :].rearrange("w p n -> p w n"),
                        )
                        if ft % 4 == 0:
                            q = ft // 4
                            for w in range(2):
                                nc.sync.dma_start(
                                    out=W2_sb[:, w, q * 4 : q * 4 + 4, :],
                                    in_=w2_v[:, w, q * 4 : q * 4 + 4, :],
                                )
                        hps = psH.tile([128, NT], F32, tag="h1", name="hps")
                        nmm = 4 * len(x_terms)
                        i = 0
                        for xh, wp in x_terms:
                            for dr in range(4):
                                nc.tensor.matmul(
                                    hps,
                                    w1[:, wp, 2 * dr : 2 * dr + 2, :],
                                    xh[:, 2 * dr : 2 * dr + 2, :],
                                    start=(i == 0), stop=(i == nmm - 1),
                                    perf_mode=DR,
                                )
                                i += 1
                        # PSUM holds (X*W1 prescales)*h; descale via gelu's
                        # input scale, rescale the fp8 planes by H_PRESCALE.
                        in_ds = 1.0 / (X_PRESCALE * W1_PRESCALE)
                        if FFN_H_SPLIT:
                            h1g = h1gp.tile([128, NT], BF16, tag="h1g", name="h1g")
                            nc.scalar.activation(
                                out=h1g, in_=hps, func=AF.Gelu,
                                bias=b1_sb[:, ft : ft + 1], scale=in_ds,
                            )
                            nc.gpsimd.tensor_scalar_mul(
                                out=h1T_hi[:, ft, :], in0=h1g, scalar1=H_PRESCALE
                            )
                            nc.vector.scalar_tensor_tensor(
                                out=h1T_lo[:, ft, :], in0=h1g, scalar=H_PRESCALE,
                                in1=h1T_hi[:, ft, :], op0=OP.mult, op1=OP.subtract,
                            )
                        else:
                            nc.scalar.activation(
                                out=h1T_hi[:, ft, :], in_=hps, func=AF.Gelu,
                                bias=b1_sb[:, ft : ft + 1], scale=in_ds,
                            )

                # --- phase 5: FFN2 + residual + LN2 + store ---
                out_v = out[:, :].rearrange("(nt p) d -> p nt d", p=128)
                with tc.tile_pool(name="psY", bufs=3, space="PSUM") as psY:
                    h_terms = [(h1T_hi, 0), (h1T_lo, 0), (h1T_hi, 1)] \
                        if FFN_H_SPLIT else [(h1T_hi, 0), (h1T_hi, 1)]
                    for nt in range(4):
                        for dh in range(2):
                            yps = psY.tile([128, 512], F32, tag="y", name="yps")
                            nmm = 16 * len(h_terms)
                            i = 0
                            for hh, wp in h_terms:
                                for dr in range(16):
                                    nc.tensor.matmul(
                                        yps,
                                        hh[:, 2 * dr : 2 * dr + 2, nt * 128 : nt * 128 + 128],
                                        W2_sb[:, wp, 2 * dr : 2 * dr + 2, dh * 512 : dh * 512 + 512],
                                        start=(i == 0), stop=(i == nmm - 1),
                                        perf_mode=DR,
                                    )
                                    i += 1
                            y_ds = 1.0 / (
                                (H_PRESCALE if FFN_H_SPLIT else 1.0) * W2_PRESCALE
                            )
                            nc.vector.scalar_tensor_tensor(
                                out=x1_sb[:, nt, dh * 512 : dh * 512 + 512],
                                in0=yps, scalar=y_ds,
                                in1=x1_sb[:, nt, dh * 512 : dh * 512 + 512],
                                op0=OP.mult, op1=OP.add,
                            )
                        if b2bc is not None:
                            nc.vector.tensor_add(
                                out=x1_sb[:, nt, :], in0=x1_sb[:, nt, :], in1=b2bc
                            )
                        ln_apply(x1_sb[:, nt, :], g2bc, be2bc)
                        nc.sync.dma_start(out=out_v[:, nt, :], in_=x1_sb[:, nt, :])

    nc.finalize()
    return nc


def _hilo8(a):
    """Stack round-to-nearest fp8 hi and residual lo planes: [2, *a.shape]."""
    hi = np.asarray(a, np.float32).astype(F8NP)
    lo = (np.asarray(a, np.float32) - hi.astype(np.float32)).astype(F8NP)
    return np.ascontiguousarray(np.stack([hi, lo], axis=0))


def host_prep(inputs):
    """Build the 8 per-core input maps from the full problem inputs."""
    src = np.asarray(inputs["src"], np.float32)
    coords = np.asarray(inputs["coords"])
    Wq = np.asarray(inputs["Wq"], np.float32)
    Wk = np.asarray(inputs["Wk"], np.float32)
    Wv = np.asarray(inputs["Wv"], np.float32)
    Wo = np.asarray(inputs["Wo"], np.float32)
    W1 = np.asarray(inputs["W1"], np.float32)
    b1 = np.asarray(inputs["b1"], np.float32)
    W2 = np.asarray(inputs["W2"], np.float32)
    b2 = np.asarray(inputs["b2"], np.float32)
    g1 = np.asarray(inputs["g1"], np.float32)
    be1 = np.asarray(inputs["be1"], np.float32)
    g2 = np.asarray(inputs["g2"], np.float32)
    be2 = np.asarray(inputs["be2"], np.float32)

    # Projection weights as fp8 hi/lo planes; the per-head SCALE/slope_h for
    # q goes in as the PSUM->qaug copy descale on device.
    shared = {
        "WqT8": _hilo8(WP_PRESCALE * Wq.T),
        "WkT8": _hilo8(WP_PRESCALE * Wk.T),
        "WvT8": _hilo8(WP_PRESCALE * Wv.T),
        "WoT8": _hilo8(WP_PRESCALE * Wo.T),
        # W1S[w, ft, p, dc*128+j] = hi/lo fp8 planes of W1.T[dc*128+p, ft*128+j]
        "W1S": _hilo8(
            (W1_PRESCALE * W1.T)
            .reshape(8, 128, 32, 128).transpose(2, 1, 0, 3).reshape(32, 128, D)
        ),
        "W2Tb": _hilo8(W2_PRESCALE * W2.T),
        "b1r": np.ascontiguousarray(b1.reshape(32, 128).T),
        "b2": b2.reshape(1, D),
        "g1": g1.reshape(1, D),
        "be1": be1.reshape(1, D),
        "g2": g2.reshape(1, D),
        "be2": be2.reshape(1, D),
    }

    in_maps = []
    for c in range(NCORES):
        b = c // 2
        half = c % 2
        rows = slice(half * NT, (half + 1) * NT)
        x = coords[b, :, 0].astype(np.float64)
        y = coords[b, :, 1].astype(np.float64)
        s = (x + y).astype(np.float32)
        thr = np.arange(1, GRID, dtype=np.float64)
        cx = (x[None, :] >= thr[:, None]).astype(np.float32)
        cy = (y[None, :] >= thr[:, None]).astype(np.float32)
        kaug = np.concatenate(
            [s.reshape(1, N), np.zeros((1, N), np.float32), cx, cy], axis=0
        ).astype(BF)
        qaug = np.empty((H, 64, NT), np.float32)
        qaug[:, 0, :] = 1.0
        qaug[:, 1, :] = 0.0
        qaug[:, 2:33, :] = -2.0 * cx[None, :, rows]
        qaug[:, 33:64, :] = -2.0 * cy[None, :, rows]
        srcTb = np.ascontiguousarray(src[b].T)
        m = dict(shared)
        m.update(
            {
                "srcT8": _hilo8(S_PRESCALE * srcTb),
                "srcQT8": _hilo8(S_PRESCALE * srcTb[:, rows]),
                "src_rows": np.ascontiguousarray(src[b, rows, :]),
                "kaug_x": kaug,
                "qaug_x": qaug.astype(BF),
            }
        )
        in_maps.append(m)
    return in_maps


_NCS = {}
LAST_RUN_S = None


def get_nc(trivial_affine=True):
    if trivial_affine not in _NCS:
        _NCS[trivial_affine] = build_nc(trivial_affine)
    return _NCS[trivial_affine]


def _affine_trivial(inputs):
    return (
        np.all(np.asarray(inputs["g1"]) == 1.0)
        and np.all(np.asarray(inputs["g2"]) == 1.0)
        and not np.any(np.asarray(inputs["be1"]))
        and not np.any(np.asarray(inputs["be2"]))
        and not np.any(np.asarray(inputs["b2"]))
    )


def kernel(**inputs):
    global LAST_RUN_S
    from concourse.bass_utils import run_bass_kernel_spmd

    nc = get_nc(bool(_affine_trivial(inputs)))
    in_maps = host_prep(inputs)
    t0 = time.monotonic()
    res = run_bass_kernel_spmd(nc, in_maps, list(range(NCORES)))
    LAST_RUN_S = time.monotonic() - t0
    full = np.empty((B, N, D), np.float32)
    for c in range(NCORES):
        b = c // 2
        half = c % 2
        full[b, half * NT : (half + 1) * NT, :] = res.results[c]["out"]
    return full



# revision 39
# speedup vs baseline: 1.1679x; 1.1679x over previous
"""Fused transformer encoder layer (attention w/ 2D-ALiBi bias + FFN) on 8 trn2 cores.

Sharding: core c handles batch b = c//2, token half h = c%2 (512 query rows).
K/V are computed per-core for the full 1024-token sequence of its batch
(duplicated across the 2 cores sharing a batch); outputs are disjoint row
slices of the final tensor, so no collectives are needed.

Bias trick: the alibi_2d bias slope_h*(|xi-xj|+|yi-yj|) is folded into the
QK^T contraction. |xi-xj| = xi + xj - 2*a_i.a_j with a_i in {0,1}^31 the
threshold indicators of xi, so dist(i,j) = s_i + s_j - 2*c_i.c_j (c = 62-dim
indicator, s = x+y). The per-query term slope*s_i is constant along the
softmax axis and is dropped. Q/K are augmented with 64 extra contraction dims
(s_j / pad / c_j on the K side; 1 / 0 / -2*c_i on the Q side), making the
score contraction K = 64+64 = 128 exactly — full PE array, bias for free.

bf16 precision care: the aug rows are small integers / {0,-2} — exact in
bf16. The attention scale AND the per-head slope are folded out of the bf16
data: Q-projection weights carry scale/slope_h per head (so scores come out
as S/slope_h) and the exact fp32 slope_h is re-applied as the exp()
activation's scale immediate. exp needs no max-subtraction (|S| <= ~50 by
construction).

Scores are computed keys-on-partitions (S^T layout) so the exp() output is
already P^T for the AV matmul (no transpose). Softmax denominators come from
an appended ones-column in V; normalization is deferred past the (linear)
output projection boundary: each head's O^T rows are scaled by a broadcasted
1/den (built with a small fp32 selector matmul) before the head-summing
projection.
"""

import math
import sys
import time

for _p in ("/opt/trn_rl_repo",):
    if _p not in sys.path:
        sys.path.insert(0, _p)

import numpy as np
import ml_dtypes

import concourse.bass as bass
import concourse.tile as tile
from concourse import bacc, mybir
from concourse.masks import make_identity

F32 = mybir.dt.float32
F32R = mybir.dt.float32r
BF16 = mybir.dt.bfloat16
F8 = mybir.dt.float8e4
BF = ml_dtypes.bfloat16
F8NP = ml_dtypes.float8_e4m3
DR = mybir.MatmulPerfMode.DoubleRow

# fp8 FFN config: activations split into fp8 hi+lo planes (quantization error
# feedback), weights plain fp8. Splitting halves the DoubleRow speedup for
# that operand but removes its quantization error from the output.
FFN_X_SPLIT = True   # x1T (FFN1 input) hi/lo
FFN_H_SPLIT = True   # h1T (FFN2 input) hi/lo

# fp8e4m3 normals span [2^-6, 448]; W1/W2 entries (sigma ~ 1/32) and the lo
# planes would otherwise land in the subnormal range and lose most precision,
# so everything is pre-scaled up into the normal range and the product scale
# is divided back out at the PSUM->SBUF boundary (gelu scale / y descale).
W1_PRESCALE = 256.0
W2_PRESCALE = 256.0
X_PRESCALE = 16.0
H_PRESCALE = 32.0
S_PRESCALE = 8.0     # src (sigma 1) for the QKV projections
WP_PRESCALE = 256.0  # Wq/Wk/Wv/Wo (sigma 1/32)
OT_PRESCALE = 16.0   # normalized attention outputs (sigma ~1)

D = 1024          # d_model
H = 16            # heads
HD = 64           # head dim
DFF = 4096
B = 4
N = 1024          # sequence length
NT = 512          # tokens (query rows) per core
GRID = 32
EPS = 1e-5
NCORES = 8
SCALE = HD ** -0.5


def _alibi_slopes(n):
    def pow2(n_):
        start = 2.0 ** (-(2.0 ** -(math.log2(n_) - 3)))
        return [start * start ** i for i in range(n_)]
    if math.log2(n).is_integer():
        return np.array(pow2(n), dtype=np.float64)
    m = 2 ** math.floor(math.log2(n))
    s = pow2(m)
    s += [s[-1] * 0.5 ** (i + 1) for i in range(n - m)]
    return np.array(s, dtype=np.float64)


SLOPES = _alibi_slopes(H)


def build_nc(trivial_affine=False):
    """trivial_affine: g1/g2 all-ones and be1/be2/b2 all-zeros -> skip those ops."""
    nc = bacc.Bacc()

    # srcT8 columns are permuted per-core so the core's own 512 query rows
    # come FIRST (Q proj reads stf[:, :, :, 0:NT]); key order is a contraction
    # axis everywhere else, so the permutation is invisible provided kaug_x
    # and the V layout use the same order (host_prep keeps them consistent).
    srcT8 = nc.declare_dram_parameter("srcT8", [2, D, N], F8, isOutput=False)
    src_rows = nc.declare_dram_parameter("src_rows", [NT, D], F32, isOutput=False)
    # Wq/Wk swizzled per output-block dt (head pair): WqS8[dt, w, p, c*128+j]
    # = Wq.T[c*128+p, dt*128+j], so each dt block (both planes) is one
    # contiguous DMA and head pair dt can project as soon as it lands.
    WqS8 = nc.declare_dram_parameter("WqS8", [8, 2, 128, D], F8, isOutput=False)
    WkS8 = nc.declare_dram_parameter("WkS8", [8, 2, 128, D], F8, isOutput=False)
    # Wv swizzled by dh half: WvS8[dh, w, p, c*512+j] = Wv.T[c*128+p, dh*512+j]
    WvS8 = nc.declare_dram_parameter("WvS8", [2, 2, 128, 8 * 512], F8, isOutput=False)
    WoT8 = nc.declare_dram_parameter("WoT8", [2, D, D], F8, isOutput=False)
    # W1 pre-swizzled on host: W1S[w, ft, p, dc*128+j] = W1.T[dc*128+p, ft*128+j]
    # (w = fp8 hi/lo plane) so each FFN1 weight block is contiguous per plane.
    W1S = nc.declare_dram_parameter("W1S", [2, 32, 128, D], F8, isOutput=False)
    # W2 split by output half dh so dh0 can stream during FFN1 and dh1
    # during the FFN2 dh0 pass: W2S[dh, w, dff, j] = W2.T[dff, dh*512+j]
    W2S = nc.declare_dram_parameter("W2S", [2, 2, DFF, 512], F8, isOutput=False)
    # aug blocks are head-independent; the singleton dim enables stride-0
    # broadcast DMAs into all head slots.
    kaug_x = nc.declare_dram_parameter("kaug_x", [64, 1, N], BF16, isOutput=False)
    qaug_x = nc.declare_dram_parameter("qaug_x", [64, 1, NT], BF16, isOutput=False)
    b1r = nc.declare_dram_parameter("b1r", [128, 32], F32, isOutput=False)
    b2 = nc.declare_dram_parameter("b2", [1, D], F32, isOutput=False)
    g1 = nc.declare_dram_parameter("g1", [1, D], F32, isOutput=False)
    be1 = nc.declare_dram_parameter("be1", [1, D], F32, isOutput=False)
    g2 = nc.declare_dram_parameter("g2", [1, D], F32, isOutput=False)
    be2 = nc.declare_dram_parameter("be2", [1, D], F32, isOutput=False)
    out = nc.declare_dram_parameter("out", [NT, D], F32, isOutput=True)

    AF = mybir.ActivationFunctionType
    OP = mybir.AluOpType

    with tile.TileContext(nc) as tc:
        with (
            tc.tile_pool(name="misc", bufs=1) as misc,
            tc.tile_pool(name="lnp", bufs=4) as lnp,
            tc.tile_pool(name="pre", bufs=1) as pre,
        ):
            eps_sb = misc.tile([128, 1], F32, tag="eps")
            nc.vector.memset(eps_sb, EPS)
            ident = misc.tile([128, 128], F32, tag="ident")
            make_identity(nc, ident)
            ident_bf = misc.tile([128, 128], BF16, tag="identbf")
            make_identity(nc, ident_bf)
            # OT8[p, c, q]: head 2c in partitions 0:64, head 2c+1 in 64:128;
            # fp8 hi/lo planes (scaled by OT_PRESCALE) for the 3-term out-proj
            OT8_hi = misc.tile([128, 8, NT], F8, tag="ot8h")
            OT8_lo = misc.tile([128, 8, NT], F8, tag="ot8l")

            def ln_apply(x_ap, gbc, bbc):
                stats = lnp.tile([128, 2, 6], F32, tag="lnstats", name="lnstats")
                for sg in range(2):
                    nc.vector.bn_stats(
                        out=stats[:, sg, :], in_=x_ap[:, sg * 512 : sg * 512 + 512]
                    )
                mv = lnp.tile([128, 2], F32, tag="lnmv", name="lnmv")
                nc.vector.bn_aggr(out=mv, in_=stats)
                nc.scalar.activation(
                    out=mv[:, 1:2], in_=mv[:, 1:2], func=AF.Sqrt,
                    bias=eps_sb, scale=1.0,
                )
                nc.vector.reciprocal(out=mv[:, 1:2], in_=mv[:, 1:2])
                # apply on Pool: DVE is the phase-3/5 critical path
                nc.gpsimd.tensor_scalar(
                    out=x_ap, in0=x_ap,
                    scalar1=mv[:, 0:1], scalar2=mv[:, 1:2],
                    op0=OP.subtract, op1=OP.mult,
                )
                if gbc is not None:
                    nc.vector.tensor_mul(out=x_ap, in0=x_ap, in1=gbc)
                if bbc is not None:
                    nc.vector.tensor_add(out=x_ap, in0=x_ap, in1=bbc)

            # ============ merged projections + attention ============
            # Single software pipeline: per step s, project K/Q for head pair
            # s, run scores+exp for pair s-1, AV+normalize for pair s-2, and
            # transpose/quantize pair s-3. V projections burst at steps 1/4.
            # AV runs in natural orientation (A = P^T chunk, B = V columns
            # incl. a ones column -> out [128 queries, 64 vdims + den]), so
            # softmax denominators are per-PARTITION and normalization is a
            # plain tensor_scalar; O^T for the out-proj comes from cheap bf16
            # PE transposes of head pairs.
            with tc.tile_pool(name="att", bufs=1) as att:
                kaug = att.tile([128, H, N], BF16, tag="kaug")
                qaug = att.tile([128, H, NT], BF16, tag="qaug")
                v_sb = att.tile([128, 8, H * 65], BF16, tag="vsb")
                v4 = v_sb.rearrange("p m (h w) -> p m h w", w=65)
                nc.vector.memset(v4[:, :, :, 64], 1.0)

                P_DS = 1.0 / (S_PRESCALE * WP_PRESCALE)
                # (activation plane, weight plane) product terms; lo*lo skipped
                TERMS = ((0, 0), (1, 0), (0, 1))

                def mm3t(ps, w8, x8, wslice, xslice):
                    i = 0
                    for a, b in TERMS:
                        for dr in range(4):
                            pl = slice(2 * dr, 2 * dr + 2)
                            nc.tensor.matmul(
                                ps,
                                w8[:, b, pl, wslice],
                                x8[:, a, pl, xslice],
                                start=(i == 0), stop=(i == 11),
                                perf_mode=DR,
                            )
                            i += 1

                with tc.tile_pool(name="ph1", bufs=1) as ph1:
                    # DMA emission order tracks first-use order; weights are
                    # host-swizzled so each head pair's block is one
                    # contiguous transfer.
                    wqf = ph1.tile([128, 8, 2, 8, 128], F8, tag="wqf")
                    stf = ph1.tile([128, 2, 8, N], F8, tag="stf")
                    st_vw = srcT8[:, :, :].rearrange("w (c p) n -> p w c n", p=128)
                    wkf = ph1.tile([128, 8, 2, 8, 128], F8, tag="wkf")
                    wvf = ph1.tile([128, 2, 2, 8, 512], F8, tag="wvf")

                    def dma_wdt(dst, src, dt):
                        nc.sync.dma_start(
                            out=dst[:, dt, :, :, :],
                            in_=src[dt, :, :, :].rearrange(
                                "w p (c n) -> p w c n", c=8
                            ),
                        )

                    def dma_aug(h0, h1):
                        # stride-0 broadcast of the shared aug block into
                        # head slots [h0, h1)
                        nc.sync.dma_start(
                            out=kaug[64:128, h0:h1, :],
                            in_=kaug_x[:, :, :].to_broadcast([64, h1 - h0, N]),
                        )
                        nc.sync.dma_start(
                            out=qaug[64:128, h0:h1, :],
                            in_=qaug_x[:, :, :].to_broadcast([64, h1 - h0, NT]),
                        )

                    # need-ordered: Q(0) first (own src columns + wq block 0),
                    # then K(0), aug for early heads, V(dh0), and the rest
                    # staggered against consumption.
                    nc.sync.dma_start(out=stf[:, 0, :, 0:NT], in_=st_vw[:, 0, :, 0:NT])
                    dma_wdt(wqf, WqS8, 0)
                    nc.sync.dma_start(out=stf[:, 1, :, 0:NT], in_=st_vw[:, 1, :, 0:NT])
                    nc.sync.dma_start(out=stf[:, 0, :, NT:N], in_=st_vw[:, 0, :, NT:N])
                    dma_wdt(wkf, WkS8, 0)
                    nc.sync.dma_start(out=stf[:, 1, :, NT:N], in_=st_vw[:, 1, :, NT:N])
                    dma_aug(0, 2)
                    dma_wdt(wqf, WqS8, 1)
                    dma_wdt(wkf, WkS8, 1)
                    for w in range(2):
                        nc.sync.dma_start(
                            out=wvf[:, 0, w, :, :],
                            in_=WvS8[0, w, :, :].rearrange("p (c n) -> p c n", c=8),
                        )
                    dma_aug(2, 4)
                    dma_wdt(wqf, WqS8, 2)
                    dma_wdt(wkf, WkS8, 2)
                    dma_aug(4, 6)
                    dma_wdt(wqf, WqS8, 3)
                    dma_wdt(wkf, WkS8, 3)
                    dma_aug(6, 8)
                    dma_wdt(wqf, WqS8, 4)
                    dma_wdt(wkf, WkS8, 4)
                    dma_aug(8, 12)
                    for w in range(2):
                        nc.sync.dma_start(
                            out=wvf[:, 1, w, :, :],
                            in_=WvS8[1, w, :, :].rearrange("p (c n) -> p c n", c=8),
                        )
                    dma_aug(12, 16)
                    for dt in range(5, 8):
                        dma_wdt(wqf, WqS8, dt)
                        dma_wdt(wkf, WkS8, dt)

                    # phase 3-5 prefetches ride the queue tail; they land
                    # long before the out-proj needs them.
                    if trivial_affine:
                        g1bc = be1bc = b2bc = g2bc = be2bc = None
                    else:
                        g1bc = pre.tile([128, D], F32, tag="g1bc")
                        be1bc = pre.tile([128, D], F32, tag="be1bc")
                        b2bc = pre.tile([128, D], F32, tag="b2bc")
                        g2bc = pre.tile([128, D], F32, tag="g2bc")
                        be2bc = pre.tile([128, D], F32, tag="be2bc")
                        for t_, src_ in (
                            (g1bc, g1), (be1bc, be1),
                            (b2bc, b2), (g2bc, g2), (be2bc, be2),
                        ):
                            nc.sync.dma_start(
                                out=t_, in_=src_[:, :].to_broadcast([128, D])
                            )
                    srar = pre.tile([128, 4, D], F32, tag="srcrows")
                    nc.sync.dma_start(
                        out=srar,
                        in_=src_rows[:, :].rearrange("(nt p) d -> p nt d", p=128),
                    )
                    wof = pre.tile([128, 2, 8, D], F8, tag="wof")
                    wo_vw = WoT8[:, :, :].rearrange("w (c p) n -> p w c n", p=128)
                    for w in range(2):
                        nc.sync.dma_start(out=wof[:, w, :, :], in_=wo_vw[:, w, :, :])
                    b1_sb = pre.tile([128, 32], F32, tag="b1")
                    nc.sync.dma_start(out=b1_sb, in_=b1r[:, :])

                    with (
                        tc.tile_pool(name="ptp", bufs=2) as ptp,
                        tc.tile_pool(name="obp", bufs=3) as obp,
                        tc.tile_pool(name="rcp", bufs=2) as rcp,
                        tc.tile_pool(name="psS", bufs=2, space="PSUM") as psS,
                        tc.tile_pool(name="psP", bufs=2, space="PSUM") as psP,
                        tc.tile_pool(name="psO", bufs=1, space="PSUM") as psO,
                        tc.tile_pool(name="psT", bufs=1, space="PSUM") as psT,
                    ):
                        pts = {}
                        obfs = {}

                        # p-state warmup: the PE needs ~3us of continuous
                        # execution to reach 2.4 GHz; burn the DMA cold-start
                        # on dependency-free dummy matmuls so the first real
                        # projections run at full clock.
                        wu = psP.tile([128, 512], F32, tag="pj", name="warm")
                        for _ in range(8):
                            nc.tensor.matmul(
                                wu[:, 0:128], ident, ident, start=True, stop=True
                            )

                        def mm3p(ps, w8, dt, xslice):
                            i = 0
                            for a, b in TERMS:
                                for dr in range(4):
                                    pl = slice(2 * dr, 2 * dr + 2)
                                    nc.tensor.matmul(
                                        ps,
                                        w8[:, dt, b, pl, :],
                                        stf[:, a, pl, xslice],
                                        start=(i == 0), stop=(i == 11),
                                        perf_mode=DR,
                                    )
                                    i += 1

                        def emit_K(dt, mh):
                            kps = psP.tile([128, 512], F32, tag="pj", name="kps")
                            mm3p(kps, wkf, dt, slice(mh * 512, mh * 512 + 512))
                            ksl = slice(mh * 512, mh * 512 + 512)
                            nc.vector.tensor_scalar_mul(
                                out=kaug[0:64, 2 * dt, ksl],
                                in0=kps[0:64, :], scalar1=P_DS,
                            )
                            nc.vector.tensor_scalar_mul(
                                out=kaug[0:64, 2 * dt + 1, ksl],
                                in0=kps[64:128, :], scalar1=P_DS,
                            )

                        def emit_Q(dt):
                            # own query rows are the FIRST NT columns of stf
                            qps = psP.tile([128, NT], F32, tag="pj", name="qps")
                            mm3p(qps, wqf, dt, slice(0, NT))
                            nc.vector.tensor_scalar_mul(
                                out=qaug[0:64, 2 * dt, :], in0=qps[0:64, :],
                                scalar1=float(SCALE / SLOPES[2 * dt] * P_DS),
                            )
                            nc.vector.tensor_scalar_mul(
                                out=qaug[0:64, 2 * dt + 1, :], in0=qps[64:128, :],
                                scalar1=float(SCALE / SLOPES[2 * dt + 1] * P_DS),
                            )

                        def emit_V_pair(p):
                            # V projection for head pair p only (just-in-time
                            # for AV(2p) one step later; spreads V across the
                            # pipeline and fills the exp-drain tail)
                            dh, hc = p // 4, (p % 4) * 128
                            for mg in range(2):
                                vt = psP.tile(
                                    [128, 4, 128], F32, tag="pj", name="vps"
                                )
                                for lm in range(4):
                                    mt = mg * 4 + lm
                                    i = 0
                                    for a, b in TERMS:
                                        for dr in range(4):
                                            pl = slice(2 * dr, 2 * dr + 2)
                                            nc.tensor.matmul(
                                                vt[:, lm, :],
                                                stf[:, a, pl, mt * 128 : mt * 128 + 128],
                                                wvf[:, dh, b, pl, hc : hc + 128],
                                                start=(i == 0), stop=(i == 11),
                                                perf_mode=DR,
                                            )
                                            i += 1
                                nc.vector.tensor_scalar_mul(
                                    out=v4[:, mg * 4 : mg * 4 + 4, 2 * p : 2 * p + 2, 0:64],
                                    in0=vt.rearrange("p m (h w) -> p m h w", w=64),
                                    scalar1=P_DS,
                                )

                        def emit_S(h, cs):
                            pt = pts[h]
                            for c in cs:
                                stc = psS.tile([128, 2, NT], F32, tag="st", name="stc")
                                for j in range(2):
                                    mt = 2 * c + j
                                    nc.tensor.matmul(
                                        stc[:, j, :],
                                        kaug[:, h, mt * 128 : mt * 128 + 128],
                                        qaug[:, h, :],
                                        start=True, stop=True,
                                    )
                                nc.scalar.activation(
                                    out=pt[:, 2 * c : 2 * c + 2, :], in_=stc,
                                    func=AF.Exp, scale=float(SLOPES[h]),
                                )

                        def emit_AV(h):
                            # natural orientation: out [128 queries, 64 vdims
                            # + den]; col 64 accumulates the softmax denom via
                            # the ones column in V.
                            cc = h // 2
                            if h % 2 == 0:
                                obfs[cc] = obp.tile(
                                    [128, 4, 128], BF16, tag="ob", name="obf"
                                )
                            po = psO.tile([128, 4, 65], F32, tag="po", name="po")
                            for qc in range(4):
                                for mt in range(8):
                                    nc.tensor.matmul(
                                        po[:, qc, :],
                                        pts[h][:, mt, qc * 128 : qc * 128 + 128],
                                        v4[:, mt, h, 0:65],
                                        start=(mt == 0), stop=(mt == 7),
                                    )
                            rc = rcp.tile([128, 4], F32, tag="rc", name="rc")
                            nc.vector.reciprocal(out=rc, in_=po[:, :, 64])
                            nc.vector.tensor_scalar_mul(
                                out=rc, in0=rc, scalar1=OT_PRESCALE
                            )
                            # normalize promptly on DVE (psO has one buffer;
                            # the next AV waits on these reads)
                            hb = (h % 2) * 64
                            for qc in range(4):
                                nc.vector.tensor_scalar_mul(
                                    out=obfs[cc][:, qc, hb : hb + 64],
                                    in0=po[:, qc, 0:64],
                                    scalar1=rc[:, qc : qc + 1],
                                )
                            pts.pop(h)

                        def emit_pair(cc):
                            ob = obfs.pop(cc)
                            tp = psT.tile([128, 4, 128], BF16, tag="tp", name="tp")
                            for qc in range(4):
                                nc.tensor.transpose(
                                    tp[:, qc, :], ob[:, qc, :], ident_bf
                                )
                            tpf = tp.rearrange("p a b -> p (a b)")
                            # gpsimd cannot touch PSUM on hw; Act does the copy
                            nc.scalar.activation(
                                out=OT8_hi[:, cc, :], in_=tpf, func=AF.Copy
                            )
                            nc.vector.tensor_sub(
                                out=OT8_lo[:, cc, :], in0=tpf, in1=OT8_hi[:, cc, :]
                            )

                        def step(s):
                            h0, h1 = 2 * (s - 1), 2 * (s - 1) + 1
                            av0, av1 = 2 * (s - 2), 2 * (s - 2) + 1
                            if 2 <= s <= 9:
                                emit_AV(av0)
                            if s < 8:
                                emit_Q(s)
                            if 1 <= s <= 8:
                                pts[h0] = ptp.tile(
                                    [128, 8, NT], BF16, tag="pt", name="pt"
                                )
                                emit_S(h0, (0, 1))
                            if 2 <= s <= 9:
                                emit_AV(av1)
                            if s < 8:
                                emit_K(s, 0)
                            if 1 <= s <= 8:
                                emit_S(h0, (2, 3))
                            if s < 8:
                                emit_K(s, 1)
                            if 1 <= s <= 8:
                                pts[h1] = ptp.tile(
                                    [128, 8, NT], BF16, tag="pt", name="pt"
                                )
                                emit_S(h1, (0, 1))
                            if 1 <= s <= 8:
                                emit_V_pair(s - 1)
                            if 1 <= s <= 8:
                                emit_S(h1, (2, 3))
                            if s >= 3:
                                emit_pair(s - 3)

                        for s in range(11):
                            step(s)

            # ============ post-attention scope ============
            with tc.tile_pool(name="ffn", bufs=1) as ffn:
                # W2 halves [p, dh, w, c, n]: dh0 streams during FFN1, dh1
                # during the FFN2 dh0 pass (keeps FFN1's W1 stream PE-bound)
                W2h = ffn.tile([128, 2, 2, 32, 512], F8, tag="w2")
                w2_v = [
                    W2S[dh, :, :, :].rearrange("w (c p) n -> p w c n", p=128)
                    for dh in range(2)
                ]

                def dma_w2(dh, cg):
                    # one [128, w, 4, 512] chunk (0.25 MB) per call
                    for w in range(2):
                        nc.sync.dma_start(
                            out=W2h[:, dh, w, 4 * cg : 4 * cg + 4, :],
                            in_=w2_v[dh][:, w, 4 * cg : 4 * cg + 4, :],
                        )
                x1_sb = ffn.tile([128, 4, D], F32, tag="x1")
                x1T_hi = ffn.tile([128, 8, NT], F8, tag="x1Th")
                x1T_lo = (
                    ffn.tile([128, 8, NT], F8, tag="x1Tl", name="x1T_lo") if FFN_X_SPLIT else None
                )

                # --- phase 3: out-proj, LN1, transpose ---
                with (
                    tc.tile_pool(name="psS2", bufs=2, space="PSUM") as psS2,
                    tc.tile_pool(name="psT", bufs=2, space="PSUM") as psT,
                ):
                    def transposes(nt):
                        for c in range(8):
                            tp = psT.tile([128, 128], F32, tag="tp", name="tp")
                            nc.tensor.transpose(
                                tp, x1_sb[:, nt, c * 128 : c * 128 + 128], ident
                            )
                            hi = x1T_hi[:, c, nt * 128 : nt * 128 + 128]
                            nc.scalar.activation(
                                out=hi, in_=tp, func=AF.Copy, scale=X_PRESCALE
                            )
                            if FFN_X_SPLIT:
                                nc.vector.scalar_tensor_tensor(
                                    out=x1T_lo[:, c, nt * 128 : nt * 128 + 128],
                                    in0=tp, scalar=X_PRESCALE, in1=hi,
                                    op0=OP.mult, op1=OP.subtract,
                                )

                    OT8 = (OT8_hi, OT8_lo)
                    O_DS = 1.0 / (OT_PRESCALE * WP_PRESCALE)
                    for nt in range(4):
                        for dh in range(2):
                            s2 = psS2.tile([128, 512], F32, tag="s2", name="s2")
                            i = 0
                            for a, b in TERMS:
                                for dr in range(4):
                                    pl = slice(2 * dr, 2 * dr + 2)
                                    nc.tensor.matmul(
                                        s2,
                                        OT8[a][:, pl, nt * 128 : nt * 128 + 128],
                                        wof[:, b, pl, dh * 512 : dh * 512 + 512],
                                        start=(i == 0), stop=(i == 11),
                                        perf_mode=DR,
                                    )
                                    i += 1
                            nc.vector.scalar_tensor_tensor(
                                out=x1_sb[:, nt, dh * 512 : dh * 512 + 512],
                                in0=s2, scalar=O_DS,
                                in1=srar[:, nt, dh * 512 : dh * 512 + 512],
                                op0=OP.mult, op1=OP.add,
                            )
                        if nt >= 1:
                            transposes(nt - 1)
                        ln_apply(x1_sb[:, nt, :], g1bc, be1bc)
                    transposes(3)

                # --- phase 4: FFN1 (fp8 DoubleRow matmuls, gelu into fp8 h1T) ---
                h1T_hi = ffn.tile([128, 32, NT], F8, tag="h1Th")
                h1T_lo = (
                    ffn.tile([128, 32, NT], F8, tag="h1Tl", name="h1T_lo") if FFN_H_SPLIT else None
                )
                with (
                    tc.tile_pool(name="w1p", bufs=10) as w1p,
                    tc.tile_pool(name="h1gp", bufs=3) as h1gp,
                    tc.tile_pool(name="psH", bufs=3, space="PSUM") as psH,
                ):
                    # term list: (x plane, w plane); lo*lo is negligible
                    x_terms = [(x1T_hi, 0), (x1T_lo, 0), (x1T_hi, 1)] \
                        if FFN_X_SPLIT else [(x1T_hi, 0), (x1T_hi, 1)]
                    for ft in range(32):
                        w1 = w1p.tile([128, 2, 8, 128], F8, tag="w1col", name="w1")
                        nc.sync.dma_start(
                            out=w1.rearrange("p w c n -> p w (c n)"),
                            in_=W1S[:, ft, :, :].rearrange("w p n -> p w n"),
                        )
                        # W2 dh0 chunks ride behind the first 8 W1 chunks so
                        # the FFN1 stream never starves on its own weights
                        if ft >= 8 and (ft - 8) % 3 == 0 and (ft - 8) // 3 < 8:
                            dma_w2(0, (ft - 8) // 3)
                        hps = psH.tile([128, NT], F32, tag="h1", name="hps")
                        nmm = 4 * len(x_terms)
                        i = 0
                        for xh, wp in x_terms:
                            for dr in range(4):
                                nc.tensor.matmul(
                                    hps,
                                    w1[:, wp, 2 * dr : 2 * dr + 2, :],
                                    xh[:, 2 * dr : 2 * dr + 2, :],
                                    start=(i == 0), stop=(i == nmm - 1),
                                    perf_mode=DR,
                                )
                                i += 1
                        # PSUM holds (X*W1 prescales)*h; descale via gelu's
                        # input scale, rescale the fp8 planes by H_PRESCALE.
                        in_ds = 1.0 / (X_PRESCALE * W1_PRESCALE)
                        if FFN_H_SPLIT:
                            h1g = h1gp.tile([128, NT], BF16, tag="h1g", name="h1g")
                            nc.scalar.activation(
                                out=h1g, in_=hps, func=AF.Gelu,
                                bias=b1_sb[:, ft : ft + 1], scale=in_ds,
                            )
                            nc.vector.tensor_scalar_mul(
                                out=h1T_hi[:, ft, :], in0=h1g, scalar1=H_PRESCALE
                            )
                            nc.vector.scalar_tensor_tensor(
                                out=h1T_lo[:, ft, :], in0=h1g, scalar=H_PRESCALE,
                                in1=h1T_hi[:, ft, :], op0=OP.mult, op1=OP.subtract,
                            )
                        else:
                            nc.scalar.activation(
                                out=h1T_hi[:, ft, :], in_=hps, func=AF.Gelu,
                                bias=b1_sb[:, ft : ft + 1], scale=in_ds,
                            )

                # --- phase 5: FFN2 (dh-major) + residual + LN2 + store ---
                out_v = out[:, :].rearrange("(nt p) d -> p nt d", p=128)
                with tc.tile_pool(name="psY", bufs=3, space="PSUM") as psY:
                    h_terms = [(h1T_hi, 0), (h1T_lo, 0), (h1T_hi, 1)] \
                        if FFN_H_SPLIT else [(h1T_hi, 0), (h1T_hi, 1)]
                    y_ds = 1.0 / (
                        (H_PRESCALE if FFN_H_SPLIT else 1.0) * W2_PRESCALE
                    )
                    for dh in range(2):
                        for nt in range(4):
                            if dh == 0:
                                dma_w2(1, 2 * nt)
                                dma_w2(1, 2 * nt + 1)
                            yps = psY.tile([128, 512], F32, tag="y", name="yps")
                            nmm = 16 * len(h_terms)
                            i = 0
                            for hh, wp in h_terms:
                                for dr in range(16):
                                    nc.tensor.matmul(
                                        yps,
                                        hh[:, 2 * dr : 2 * dr + 2, nt * 128 : nt * 128 + 128],
                                        W2h[:, dh, wp, 2 * dr : 2 * dr + 2, :],
                                        start=(i == 0), stop=(i == nmm - 1),
                                        perf_mode=DR,
                                    )
                                    i += 1
                            nc.vector.scalar_tensor_tensor(
                                out=x1_sb[:, nt, dh * 512 : dh * 512 + 512],
                                in0=yps, scalar=y_ds,
                                in1=x1_sb[:, nt, dh * 512 : dh * 512 + 512],
                                op0=OP.mult, op1=OP.add,
                            )
                            if dh == 1:
                                if b2bc is not None:
                                    nc.vector.tensor_add(
                                        out=x1_sb[:, nt, :],
                                        in0=x1_sb[:, nt, :], in1=b2bc,
                                    )
                                ln_apply(x1_sb[:, nt, :], g2bc, be2bc)
                                nc.sync.dma_start(
                                    out=out_v[:, nt, :], in_=x1_sb[:, nt, :]
                                )

    nc.finalize()
    return nc


def _hilo8(a):
    """Stack round-to-nearest fp8 hi and residual lo planes: [2, *a.shape]."""
    hi = np.asarray(a, np.float32).astype(F8NP)
    lo = (np.asarray(a, np.float32) - hi.astype(np.float32)).astype(F8NP)
    return np.ascontiguousarray(np.stack([hi, lo], axis=0))


def host_prep(inputs):
    """Build the 8 per-core input maps from the full problem inputs."""
    src = np.asarray(inputs["src"], np.float32)
    coords = np.asarray(inputs["coords"])
    Wq = np.asarray(inputs["Wq"], np.float32)
    Wk = np.asarray(inputs["Wk"], np.float32)
    Wv = np.asarray(inputs["Wv"], np.float32)
    Wo = np.asarray(inputs["Wo"], np.float32)
    W1 = np.asarray(inputs["W1"], np.float32)
    b1 = np.asarray(inputs["b1"], np.float32)
    W2 = np.asarray(inputs["W2"], np.float32)
    b2 = np.asarray(inputs["b2"], np.float32)
    g1 = np.asarray(inputs["g1"], np.float32)
    be1 = np.asarray(inputs["be1"], np.float32)
    g2 = np.asarray(inputs["g2"], np.float32)
    be2 = np.asarray(inputs["be2"], np.float32)

    def _blk8(wt, nblk, blk):
        # [dt, w, p, c*blk + j] from wt.T-like [c*128+p, dt*blk+j]
        x = (WP_PRESCALE * wt.T).reshape(8, 128, nblk, blk)
        x = x.transpose(2, 1, 0, 3).reshape(nblk, 128, 8 * blk)
        return np.ascontiguousarray(_hilo8(x).transpose(1, 0, 2, 3))

    # Projection weights as fp8 hi/lo planes; the per-head SCALE/slope_h for
    # q goes in as the PSUM->qaug copy descale on device.
    shared = {
        "WqS8": _blk8(Wq, 8, 128),
        "WkS8": _blk8(Wk, 8, 128),
        "WvS8": _blk8(Wv, 2, 512),
        "WoT8": _hilo8(WP_PRESCALE * Wo.T),
        # W1S[w, ft, p, dc*128+j] = hi/lo fp8 planes of W1.T[dc*128+p, ft*128+j]
        "W1S": _hilo8(
            (W1_PRESCALE * W1.T)
            .reshape(8, 128, 32, 128).transpose(2, 1, 0, 3).reshape(32, 128, D)
        ),
        # W2S[dh, w, dff, j] = hi/lo planes of W2.T[dff, dh*512+j]
        "W2S": np.ascontiguousarray(
            _hilo8(
                (W2_PRESCALE * W2.T).reshape(DFF, 2, 512).transpose(1, 0, 2)
            ).transpose(1, 0, 2, 3)
        ),
        "b1r": np.ascontiguousarray(b1.reshape(32, 128).T),
        "b2": b2.reshape(1, D),
        "g1": g1.reshape(1, D),
        "be1": be1.reshape(1, D),
        "g2": g2.reshape(1, D),
        "be2": be2.reshape(1, D),
    }

    in_maps = []
    for c in range(NCORES):
        b = c // 2
        half = c % 2
        rows = slice(half * NT, (half + 1) * NT)
        # key-axis permutation: own query rows first (Q proj reads the first
        # NT columns of srcT8); keys are a contraction axis everywhere, so
        # only kaug_x must be permuted consistently.
        perm = np.r_[half * NT : (half + 1) * NT, (1 - half) * NT : (2 - half) * NT]
        x = coords[b, :, 0].astype(np.float64)
        y = coords[b, :, 1].astype(np.float64)
        s = (x + y).astype(np.float32)
        thr = np.arange(1, GRID, dtype=np.float64)
        cx = (x[None, :] >= thr[:, None]).astype(np.float32)
        cy = (y[None, :] >= thr[:, None]).astype(np.float32)
        kaug = np.concatenate(
            [s.reshape(1, N), np.zeros((1, N), np.float32), cx, cy], axis=0
        ).astype(BF)
        qaug = np.empty((64, NT), np.float32)
        qaug[0, :] = 1.0
        qaug[1, :] = 0.0
        qaug[2:33, :] = -2.0 * cx[:, rows]
        qaug[33:64, :] = -2.0 * cy[:, rows]
        srcTb = np.ascontiguousarray(src[b].T[:, perm])
        m = dict(shared)
        m.update(
            {
                "srcT8": _hilo8(S_PRESCALE * srcTb),
                "src_rows": np.ascontiguousarray(src[b, rows, :]),
                "kaug_x": np.ascontiguousarray(kaug[:, perm]).reshape(64, 1, N),
                "qaug_x": qaug.astype(BF).reshape(64, 1, NT),
            }
        )
        in_maps.append(m)
    return in_maps


_NCS = {}
LAST_RUN_S = None


def get_nc(trivial_affine=True):
    if trivial_affine not in _NCS:
        _NCS[trivial_affine] = build_nc(trivial_affine)
    return _NCS[trivial_affine]


def _affine_trivial(inputs):
    return (
        np.all(np.asarray(inputs["g1"]) == 1.0)
        and np.all(np.asarray(inputs["g2"]) == 1.0)
        and not np.any(np.asarray(inputs["be1"]))
        and not np.any(np.asarray(inputs["be2"]))
        and not np.any(np.asarray(inputs["b2"]))
    )


def kernel(**inputs):
    global LAST_RUN_S
    from concourse.bass_utils import run_bass_kernel_spmd

    nc = get_nc(bool(_affine_trivial(inputs)))
    in_maps = host_prep(inputs)
    t0 = time.monotonic()
    res = run_bass_kernel_spmd(nc, in_maps, list(range(NCORES)))
    LAST_RUN_S = time.monotonic() - t0
    full = np.empty((B, N, D), np.float32)
    for c in range(NCORES):
        b = c // 2
        half = c % 2
        full[b, half * NT : (half + 1) * NT, :] = res.results[c]["out"]
    return full



# revision 65
# speedup vs baseline: 1.1995x; 1.0271x over previous
"""Fused transformer encoder layer (attention w/ 2D-ALiBi bias + FFN) on 8 trn2 cores.

Sharding: core c handles batch b = c//2, token half h = c%2 (512 query rows).
K/V are computed per-core for the full 1024-token sequence of its batch
(duplicated across the 2 cores sharing a batch); outputs are disjoint row
slices of the final tensor, so no collectives are needed.

Bias trick: the alibi_2d bias slope_h*(|xi-xj|+|yi-yj|) is folded into the
QK^T contraction. |xi-xj| = xi + xj - 2*a_i.a_j with a_i in {0,1}^31 the
threshold indicators of xi, so dist(i,j) = s_i + s_j - 2*c_i.c_j (c = 62-dim
indicator, s = x+y). The per-query term slope*s_i is constant along the
softmax axis and is dropped. Q/K are augmented with 64 extra contraction dims
(s_j / pad / c_j on the K side; 1 / 0 / -2*c_i on the Q side), making the
score contraction K = 64+64 = 128 exactly — full PE array, bias for free.

bf16 precision care: the aug rows are small integers / {0,-2} — exact in
bf16. The attention scale AND the per-head slope are folded out of the bf16
data: Q-projection weights carry scale/slope_h per head (so scores come out
as S/slope_h) and the exact fp32 slope_h is re-applied as the exp()
activation's scale immediate. exp needs no max-subtraction (|S| <= ~50 by
construction).

Scores are computed keys-on-partitions (S^T layout) so the exp() output is
already P^T for the AV matmul (no transpose). Softmax denominators come from
an appended ones-column in V; normalization is deferred past the (linear)
output projection boundary: each head's O^T rows are scaled by a broadcasted
1/den (built with a small fp32 selector matmul) before the head-summing
projection.
"""

import math
import sys
import time

for _p in ("/opt/trn_rl_repo",):
    if _p not in sys.path:
        sys.path.insert(0, _p)

import numpy as np
import ml_dtypes

import concourse.bass as bass
import concourse.tile as tile
from concourse import bacc, mybir
from concourse.masks import make_identity

F32 = mybir.dt.float32
F32R = mybir.dt.float32r
BF16 = mybir.dt.bfloat16
F8 = mybir.dt.float8e4
BF = ml_dtypes.bfloat16
F8NP = ml_dtypes.float8_e4m3
DR = mybir.MatmulPerfMode.DoubleRow

# fp8 FFN config: activations split into fp8 hi+lo planes (quantization error
# feedback), weights plain fp8. Splitting halves the DoubleRow speedup for
# that operand but removes its quantization error from the output.
FFN_X_SPLIT = True   # x1T (FFN1 input) hi/lo
FFN_H_SPLIT = True   # h1T (FFN2 input) hi/lo

# fp8e4m3 normals span [2^-6, 448]; W1/W2 entries (sigma ~ 1/32) and the lo
# planes would otherwise land in the subnormal range and lose most precision,
# so everything is pre-scaled up into the normal range and the product scale
# is divided back out at the PSUM->SBUF boundary (gelu scale / y descale).
W1_PRESCALE = 256.0
W2_PRESCALE = 256.0
X_PRESCALE = 16.0
H_PRESCALE = 32.0
S_PRESCALE = 8.0     # src (sigma 1) for the QKV projections
WP_PRESCALE = 256.0  # Wq/Wk/Wv/Wo (sigma 1/32)
OT_PRESCALE = 16.0   # normalized attention outputs (sigma ~1)

D = 1024          # d_model
H = 16            # heads
HD = 64           # head dim
DFF = 4096
B = 4
N = 1024          # sequence length
NT = 512          # tokens (query rows) per core
GRID = 32
EPS = 1e-5
NCORES = 8
SCALE = HD ** -0.5


def _alibi_slopes(n):
    def pow2(n_):
        start = 2.0 ** (-(2.0 ** -(math.log2(n_) - 3)))
        return [start * start ** i for i in range(n_)]
    if math.log2(n).is_integer():
        return np.array(pow2(n), dtype=np.float64)
    m = 2 ** math.floor(math.log2(n))
    s = pow2(m)
    s += [s[-1] * 0.5 ** (i + 1) for i in range(n - m)]
    return np.array(s, dtype=np.float64)


SLOPES = _alibi_slopes(H)


def build_nc(trivial_affine=False):
    """trivial_affine: g1/g2 all-ones and be1/be2/b2 all-zeros -> skip those ops."""
    nc = bacc.Bacc()

    # srcT8 columns are permuted per-core so the core's own 512 query rows
    # come FIRST (Q proj reads stf[:, :, :, 0:NT]); key order is a contraction
    # axis everywhere else, so the permutation is invisible provided kaug_x
    # and the V layout use the same order (host_prep keeps them consistent).
    srcT8 = nc.declare_dram_parameter("srcT8", [2, D, N], F8, isOutput=False)
    src_rows = nc.declare_dram_parameter("src_rows", [NT, D], F32, isOutput=False)
    # Wq/Wk swizzled per output-block dt (head pair): WqS8[dt, w, p, c*128+j]
    # = Wq.T[c*128+p, dt*128+j], so each dt block (both planes) is one
    # contiguous DMA and head pair dt can project as soon as it lands.
    WqS8 = nc.declare_dram_parameter("WqS8", [8, 2, 128, D], F8, isOutput=False)
    WkS8 = nc.declare_dram_parameter("WkS8", [8, 2, 128, D], F8, isOutput=False)
    # Wv swizzled by dh half: WvS8[dh, w, p, c*512+j] = Wv.T[c*128+p, dh*512+j]
    WvS8 = nc.declare_dram_parameter("WvS8", [2, 2, 128, 8 * 512], F8, isOutput=False)
    WoT8 = nc.declare_dram_parameter("WoT8", [2, D, D], F8, isOutput=False)
    # W1 pre-swizzled on host: W1S[w, ft, p, dc*128+j] = W1.T[dc*128+p, ft*128+j]
    # (w = fp8 hi/lo plane) so each FFN1 weight block is contiguous per plane.
    W1S = nc.declare_dram_parameter("W1S", [2, 32, 128, D], F8, isOutput=False)
    # W2 split by output half dh so dh0 can stream during FFN1 and dh1
    # during the FFN2 dh0 pass: W2S[dh, w, dff, j] = W2.T[dff, dh*512+j]
    W2S = nc.declare_dram_parameter("W2S", [2, 2, DFF, 512], F8, isOutput=False)
    # aug blocks are head-independent; the singleton dim enables stride-0
    # broadcast DMAs into all head slots.
    kaug_x = nc.declare_dram_parameter("kaug_x", [64, 1, N], BF16, isOutput=False)
    qaug_x = nc.declare_dram_parameter("qaug_x", [64, 1, NT], BF16, isOutput=False)
    b1r = nc.declare_dram_parameter("b1r", [128, 32], F32, isOutput=False)
    b2 = nc.declare_dram_parameter("b2", [1, D], F32, isOutput=False)
    g1 = nc.declare_dram_parameter("g1", [1, D], F32, isOutput=False)
    be1 = nc.declare_dram_parameter("be1", [1, D], F32, isOutput=False)
    g2 = nc.declare_dram_parameter("g2", [1, D], F32, isOutput=False)
    be2 = nc.declare_dram_parameter("be2", [1, D], F32, isOutput=False)
    out = nc.declare_dram_parameter("out", [NT, D], F32, isOutput=True)

    AF = mybir.ActivationFunctionType
    OP = mybir.AluOpType

    with tile.TileContext(nc) as tc:
        with (
            tc.tile_pool(name="misc", bufs=1) as misc,
            tc.tile_pool(name="lnp", bufs=4) as lnp,
            tc.tile_pool(name="pre", bufs=1) as pre,
        ):
            eps_sb = misc.tile([128, 1], F32, tag="eps")
            nc.vector.memset(eps_sb, EPS)
            ident = misc.tile([128, 128], F32, tag="ident")
            make_identity(nc, ident)
            ident_bf = misc.tile([128, 128], BF16, tag="identbf")
            make_identity(nc, ident_bf)
            ident_f8 = misc.tile([128, 128], F8, tag="identf8")
            make_identity(nc, ident_f8)
            # OT8[p, c, q]: head 2c in partitions 0:64, head 2c+1 in 64:128;
            # fp8 hi/lo planes (scaled by OT_PRESCALE) for the 3-term out-proj
            OT8_hi = misc.tile([128, 8, NT], F8, tag="ot8h")
            OT8_lo = misc.tile([128, 8, NT], F8, tag="ot8l")

            def ln_apply(x_ap, gbc, bbc):
                stats = lnp.tile([128, 2, 6], F32, tag="lnstats", name="lnstats")
                for sg in range(2):
                    nc.vector.bn_stats(
                        out=stats[:, sg, :], in_=x_ap[:, sg * 512 : sg * 512 + 512]
                    )
                mv = lnp.tile([128, 2], F32, tag="lnmv", name="lnmv")
                nc.vector.bn_aggr(out=mv, in_=stats)
                nc.scalar.activation(
                    out=mv[:, 1:2], in_=mv[:, 1:2], func=AF.Sqrt,
                    bias=eps_sb, scale=1.0,
                )
                nc.vector.reciprocal(out=mv[:, 1:2], in_=mv[:, 1:2])
                nc.vector.tensor_scalar(
                    out=x_ap, in0=x_ap,
                    scalar1=mv[:, 0:1], scalar2=mv[:, 1:2],
                    op0=OP.subtract, op1=OP.mult,
                )
                if gbc is not None:
                    nc.vector.tensor_mul(out=x_ap, in0=x_ap, in1=gbc)
                if bbc is not None:
                    nc.vector.tensor_add(out=x_ap, in0=x_ap, in1=bbc)

            # ============ merged projections + attention ============
            # Single software pipeline: per step s, project K/Q for head pair
            # s, run scores+exp for pair s-1, AV+normalize for pair s-2, and
            # transpose/quantize pair s-3. V projections burst at steps 1/4.
            # AV runs in natural orientation (A = P^T chunk, B = V columns
            # incl. a ones column -> out [128 queries, 64 vdims + den]), so
            # softmax denominators are per-PARTITION and normalization is a
            # plain tensor_scalar; O^T for the out-proj comes from cheap bf16
            # PE transposes of head pairs.
            # wqk opens before att so its released zone sits at the stack
            # bottom: the FFN W1 ring reuses it, anchored on the early
            # K(7)/Q(7) last-uses instead of the late attention tail.
            with (
                tc.tile_pool(name="wqk", bufs=1) as wqk,
                tc.tile_pool(name="att", bufs=1) as att,
            ):
                kaug = att.tile([128, H, N], BF16, tag="kaug")
                qaug = att.tile([128, H, NT], BF16, tag="qaug")
                v_sb = att.tile([128, 8, H * 65], BF16, tag="vsb")
                v4 = v_sb.rearrange("p m (h w) -> p m h w", w=65)
                nc.vector.memset(v4[:, :, :, 64], 1.0)

                P_DS = 1.0 / (S_PRESCALE * WP_PRESCALE)
                # (activation plane, weight plane) product terms; lo*lo skipped
                TERMS = ((0, 0), (1, 0), (0, 1))

                def mm3t(ps, w8, x8, wslice, xslice):
                    i = 0
                    for a, b in TERMS:
                        for dr in range(4):
                            pl = slice(2 * dr, 2 * dr + 2)
                            nc.tensor.matmul(
                                ps,
                                w8[:, b, pl, wslice],
                                x8[:, a, pl, xslice],
                                start=(i == 0), stop=(i == 11),
                                perf_mode=DR,
                            )
                            i += 1

                with tc.tile_pool(name="ph1", bufs=1) as ph1:
                    # DMA emission order tracks first-use order; weights are
                    # host-swizzled so each head pair's block is one
                    # contiguous transfer.
                    wqf = wqk.tile([128, 8, 2, 8, 128], F8, tag="wqf")
                    stf = ph1.tile([128, 2, 8, N], F8, tag="stf")
                    st_vw = srcT8[:, :, :].rearrange("w (c p) n -> p w c n", p=128)
                    wkf = wqk.tile([128, 8, 2, 8, 128], F8, tag="wkf")
                    wvf = ph1.tile([128, 2, 2, 8, 512], F8, tag="wvf")

                    def dma_wdt(dst, src, dt):
                        nc.sync.dma_start(
                            out=dst[:, dt, :, :, :],
                            in_=src[dt, :, :, :].rearrange(
                                "w p (c n) -> p w c n", c=8
                            ),
                        )

                    def dma_aug(h0, h1):
                        # stride-0 broadcast of the shared aug block into
                        # head slots [h0, h1)
                        nc.sync.dma_start(
                            out=kaug[64:128, h0:h1, :],
                            in_=kaug_x[:, :, :].to_broadcast([64, h1 - h0, N]),
                        )
                        nc.sync.dma_start(
                            out=qaug[64:128, h0:h1, :],
                            in_=qaug_x[:, :, :].to_broadcast([64, h1 - h0, NT]),
                        )

                    # need-ordered: Q(0) first (own src columns + wq block 0),
                    # then K(0), aug for early heads, V(dh0), and the rest
                    # staggered against consumption.
                    nc.sync.dma_start(out=stf[:, 0, :, 0:NT], in_=st_vw[:, 0, :, 0:NT])
                    dma_wdt(wqf, WqS8, 0)
                    nc.sync.dma_start(out=stf[:, 1, :, 0:NT], in_=st_vw[:, 1, :, 0:NT])
                    nc.sync.dma_start(out=stf[:, 0, :, NT:N], in_=st_vw[:, 0, :, NT:N])
                    dma_wdt(wkf, WkS8, 0)
                    nc.sync.dma_start(out=stf[:, 1, :, NT:N], in_=st_vw[:, 1, :, NT:N])
                    dma_aug(0, 2)
                    dma_wdt(wqf, WqS8, 1)
                    dma_wdt(wkf, WkS8, 1)
                    for w in range(2):
                        nc.sync.dma_start(
                            out=wvf[:, 0, w, :, :],
                            in_=WvS8[0, w, :, :].rearrange("p (c n) -> p c n", c=8),
                        )
                    dma_aug(2, 4)
                    dma_wdt(wqf, WqS8, 2)
                    dma_wdt(wkf, WkS8, 2)
                    dma_aug(4, 6)
                    dma_wdt(wqf, WqS8, 3)
                    dma_wdt(wkf, WkS8, 3)
                    dma_aug(6, 8)
                    dma_wdt(wqf, WqS8, 4)
                    dma_wdt(wkf, WkS8, 4)
                    dma_aug(8, 12)
                    for w in range(2):
                        nc.sync.dma_start(
                            out=wvf[:, 1, w, :, :],
                            in_=WvS8[1, w, :, :].rearrange("p (c n) -> p c n", c=8),
                        )
                    dma_aug(12, 16)
                    for dt in range(5, 8):
                        dma_wdt(wqf, WqS8, dt)
                        dma_wdt(wkf, WkS8, dt)

                    # phase 3-5 prefetches ride the queue tail; they land
                    # long before the out-proj needs them.
                    if trivial_affine:
                        g1bc = be1bc = b2bc = g2bc = be2bc = None
                    else:
                        g1bc = pre.tile([128, D], F32, tag="g1bc")
                        be1bc = pre.tile([128, D], F32, tag="be1bc")
                        b2bc = pre.tile([128, D], F32, tag="b2bc")
                        g2bc = pre.tile([128, D], F32, tag="g2bc")
                        be2bc = pre.tile([128, D], F32, tag="be2bc")
                        for t_, src_ in (
                            (g1bc, g1), (be1bc, be1),
                            (b2bc, b2), (g2bc, g2), (be2bc, be2),
                        ):
                            nc.sync.dma_start(
                                out=t_, in_=src_[:, :].to_broadcast([128, D])
                            )
                    srar = pre.tile([128, 4, D], F32, tag="srcrows")
                    nc.sync.dma_start(
                        out=srar,
                        in_=src_rows[:, :].rearrange("(nt p) d -> p nt d", p=128),
                    )
                    wof = pre.tile([128, 2, 8, D], F8, tag="wof")
                    wo_vw = WoT8[:, :, :].rearrange("w (c p) n -> p w c n", p=128)
                    for w in range(2):
                        nc.sync.dma_start(out=wof[:, w, :, :], in_=wo_vw[:, w, :, :])
                    b1_sb = pre.tile([128, 32], F32, tag="b1")
                    nc.sync.dma_start(out=b1_sb, in_=b1r[:, :])

                    with (
                        tc.tile_pool(name="ptp", bufs=2) as ptp,
                        tc.tile_pool(name="obp", bufs=3) as obp,
                        tc.tile_pool(name="rcp", bufs=2) as rcp,
                        tc.tile_pool(name="psS", bufs=2, space="PSUM") as psS,
                        tc.tile_pool(name="psP", bufs=2, space="PSUM") as psP,
                        tc.tile_pool(name="psO", bufs=1, space="PSUM") as psO,
                        tc.tile_pool(name="psT", bufs=1, space="PSUM") as psT,
                    ):
                        pts = {}
                        obfs = {}

                        # p-state warmup: the PE needs ~3us of continuous
                        # execution to reach 2.4 GHz; burn the DMA cold-start
                        # on dependency-free dummy matmuls so the first real
                        # projections run at full clock.
                        wu = psP.tile([128, 512], F32, tag="pj", name="warm")
                        for _ in range(8):
                            nc.tensor.matmul(
                                wu[:, 0:128], ident, ident, start=True, stop=True
                            )

                        def mm3p(ps, w8, dt, xslice):
                            i = 0
                            for a, b in TERMS:
                                for dr in range(4):
                                    pl = slice(2 * dr, 2 * dr + 2)
                                    nc.tensor.matmul(
                                        ps,
                                        w8[:, dt, b, pl, :],
                                        stf[:, a, pl, xslice],
                                        start=(i == 0), stop=(i == 11),
                                        perf_mode=DR,
                                    )
                                    i += 1

                        def emit_K(dt, mh):
                            kps = psP.tile([128, 512], F32, tag="pj", name="kps")
                            mm3p(kps, wkf, dt, slice(mh * 512, mh * 512 + 512))
                            ksl = slice(mh * 512, mh * 512 + 512)
                            nc.vector.tensor_scalar_mul(
                                out=kaug[0:64, 2 * dt, ksl],
                                in0=kps[0:64, :], scalar1=P_DS,
                            )
                            nc.vector.tensor_scalar_mul(
                                out=kaug[0:64, 2 * dt + 1, ksl],
                                in0=kps[64:128, :], scalar1=P_DS,
                            )

                        def emit_Q(dt):
                            # own query rows are the FIRST NT columns of stf
                            qps = psP.tile([128, NT], F32, tag="pj", name="qps")
                            mm3p(qps, wqf, dt, slice(0, NT))
                            nc.vector.tensor_scalar_mul(
                                out=qaug[0:64, 2 * dt, :], in0=qps[0:64, :],
                                scalar1=float(SCALE / SLOPES[2 * dt] * P_DS),
                            )
                            nc.vector.tensor_scalar_mul(
                                out=qaug[0:64, 2 * dt + 1, :], in0=qps[64:128, :],
                                scalar1=float(SCALE / SLOPES[2 * dt + 1] * P_DS),
                            )

                        def emit_V_pair(p):
                            # V projection for head pair p only (just-in-time
                            # for AV(2p) one step later; spreads V across the
                            # pipeline and fills the exp-drain tail)
                            dh, hc = p // 4, (p % 4) * 128
                            for mg in range(2):
                                vt = psP.tile(
                                    [128, 4, 128], F32, tag="pj", name="vps"
                                )
                                for lm in range(4):
                                    mt = mg * 4 + lm
                                    i = 0
                                    for a, b in TERMS:
                                        for dr in range(4):
                                            pl = slice(2 * dr, 2 * dr + 2)
                                            nc.tensor.matmul(
                                                vt[:, lm, :],
                                                stf[:, a, pl, mt * 128 : mt * 128 + 128],
                                                wvf[:, dh, b, pl, hc : hc + 128],
                                                start=(i == 0), stop=(i == 11),
                                                perf_mode=DR,
                                            )
                                            i += 1
                                nc.vector.tensor_scalar_mul(
                                    out=v4[:, mg * 4 : mg * 4 + 4, 2 * p : 2 * p + 2, 0:64],
                                    in0=vt.rearrange("p m (h w) -> p m h w", w=64),
                                    scalar1=P_DS,
                                )

                        def emit_S(h, cs):
                            pt = pts[h]
                            for c in cs:
                                stc = psS.tile([128, 2, NT], F32, tag="st", name="stc")
                                for j in range(2):
                                    mt = 2 * c + j
                                    nc.tensor.matmul(
                                        stc[:, j, :],
                                        kaug[:, h, mt * 128 : mt * 128 + 128],
                                        qaug[:, h, :],
                                        start=True, stop=True,
                                    )
                                nc.scalar.activation(
                                    out=pt[:, 2 * c : 2 * c + 2, :], in_=stc,
                                    func=AF.Exp, scale=float(SLOPES[h]),
                                )

                        def emit_AV(h):
                            # natural orientation: out [128 queries, 64 vdims
                            # + den]; col 64 accumulates the softmax denom via
                            # the ones column in V.
                            cc = h // 2
                            if h % 2 == 0:
                                obfs[cc] = obp.tile(
                                    [128, 4, 128], BF16, tag="ob", name="obf"
                                )
                            po = psO.tile([128, 4, 65], F32, tag="po", name="po")
                            for qc in range(4):
                                for mt in range(8):
                                    nc.tensor.matmul(
                                        po[:, qc, :],
                                        pts[h][:, mt, qc * 128 : qc * 128 + 128],
                                        v4[:, mt, h, 0:65],
                                        start=(mt == 0), stop=(mt == 7),
                                    )
                            rc = rcp.tile([128, 4], F32, tag="rc", name="rc")
                            nc.vector.reciprocal(out=rc, in_=po[:, :, 64])
                            nc.vector.tensor_scalar_mul(
                                out=rc, in0=rc, scalar1=OT_PRESCALE
                            )
                            # normalize promptly on DVE (psO has one buffer;
                            # the next AV waits on these reads)
                            hb = (h % 2) * 64
                            for qc in range(4):
                                nc.vector.tensor_scalar_mul(
                                    out=obfs[cc][:, qc, hb : hb + 64],
                                    in0=po[:, qc, 0:64],
                                    scalar1=rc[:, qc : qc + 1],
                                )
                            pts.pop(h)

                        def emit_pair(cc):
                            ob = obfs.pop(cc)
                            tp = psT.tile([128, 4, 128], BF16, tag="tp", name="tp")
                            for qc in range(4):
                                nc.tensor.transpose(
                                    tp[:, qc, :], ob[:, qc, :], ident_bf
                                )
                            tpf = tp.rearrange("p a b -> p (a b)")
                            # gpsimd cannot touch PSUM on hw; Act does the copy
                            nc.scalar.activation(
                                out=OT8_hi[:, cc, :], in_=tpf, func=AF.Copy
                            )
                            nc.vector.tensor_sub(
                                out=OT8_lo[:, cc, :], in0=tpf, in1=OT8_hi[:, cc, :]
                            )

                        def step(s):
                            h0, h1 = 2 * (s - 1), 2 * (s - 1) + 1
                            av0, av1 = 2 * (s - 2), 2 * (s - 2) + 1
                            if 2 <= s <= 9:
                                emit_AV(av0)
                            if s < 8:
                                emit_Q(s)
                            if 1 <= s <= 8:
                                pts[h0] = ptp.tile(
                                    [128, 8, NT], BF16, tag="pt", name="pt"
                                )
                                emit_S(h0, (0, 1))
                            if 2 <= s <= 9:
                                emit_AV(av1)
                            if s < 8:
                                emit_K(s, 0)
                            if 1 <= s <= 8:
                                emit_S(h0, (2, 3))
                            if s < 8:
                                emit_K(s, 1)
                            if 1 <= s <= 8:
                                pts[h1] = ptp.tile(
                                    [128, 8, NT], BF16, tag="pt", name="pt"
                                )
                                emit_S(h1, (0, 1))
                            if 1 <= s <= 8:
                                emit_V_pair(s - 1)
                            if 1 <= s <= 8:
                                emit_S(h1, (2, 3))
                            if s >= 3:
                                emit_pair(s - 3)

                        for s in range(11):
                            step(s)

            # ============ post-attention scope ============
            with (
                tc.tile_pool(name="w1p", bufs=20) as w1p,
                tc.tile_pool(name="ffn", bufs=1) as ffn,
            ):
                # w1p opens FIRST so the W1 ring lands on the earliest-freed
                # attention SBUF and its stream starts during the tail.
                # W2 halves [p, dh, w, c, n]: dh0 streams during FFN1, dh1
                # during the FFN2 dh0 pass (keeps FFN1's W1 stream PE-bound)
                W2h = ffn.tile([128, 2, 2, 32, 512], F8, tag="w2")
                w2_v = [
                    W2S[dh, :, :, :].rearrange("w (c p) n -> p w c n", p=128)
                    for dh in range(2)
                ]

                def dma_w2(dh, cg):
                    # one [128, w, 4, 512] chunk (0.25 MB) per call
                    for w in range(2):
                        nc.sync.dma_start(
                            out=W2h[:, dh, w, 4 * cg : 4 * cg + 4, :],
                            in_=w2_v[dh][:, w, 4 * cg : 4 * cg + 4, :],
                        )
                x1_sb = ffn.tile([128, 4, D], F32, tag="x1")
                x1T_hi = ffn.tile([128, 8, NT], F8, tag="x1Th")
                x1T_lo = (
                    ffn.tile([128, 8, NT], F8, tag="x1Tl", name="x1T_lo") if FFN_X_SPLIT else None
                )

                # --- phase 3: out-proj, LN1, transpose ---
                with (
                    tc.tile_pool(name="psS2", bufs=2, space="PSUM") as psS2,
                    tc.tile_pool(name="psT", bufs=2, space="PSUM") as psT,
                    tc.tile_pool(name="xqp", bufs=2) as xqp,
                ):
                    def transposes(nt, xq):
                        # xq holds bf16 16*LN(x1)[nt]; hw forbids plain fp8
                        # transposes, so transpose bf16 and split hi/lo after.
                        nsl = slice(nt * 128, nt * 128 + 128)
                        tp = psT.tile([128, 8, 128], BF16, tag="tp", name="tp")
                        for c in range(8):
                            nc.tensor.transpose(
                                tp[:, c, :],
                                xq[:, c * 128 : c * 128 + 128],
                                ident_bf,
                            )
                        nc.scalar.activation(
                            out=x1T_hi[:, :, nsl], in_=tp, func=AF.Copy
                        )
                        if FFN_X_SPLIT:
                            nc.vector.tensor_sub(
                                out=x1T_lo[:, :, nsl],
                                in0=tp, in1=x1T_hi[:, :, nsl],
                            )

                    def quantize_half(nt, hsl, xq):
                        nc.scalar.activation(
                            out=xq[:, hsl], in_=x1_sb[:, nt, hsl],
                            func=AF.Copy, scale=X_PRESCALE,
                        )

                    OT8 = (OT8_hi, OT8_lo)
                    O_DS = 1.0 / (OT_PRESCALE * WP_PRESCALE)
                    st1 = [
                        lnp.tile([128, 2, 6], F32, tag=f"ln1s{nt}", name=f"ln1s{nt}")
                        for nt in range(4)
                    ]
                    xqs = {}
                    for nt in range(4):
                        for dh in range(2):
                            s2 = psS2.tile([128, 512], F32, tag="s2", name="s2")
                            # dr-major: only the last 3 matmuls (head pairs
                            # 6-7) wait on the final attention quantize
                            i = 0
                            for dr in range(4):
                                pl = slice(2 * dr, 2 * dr + 2)
                                for a, b in TERMS:
                                    nc.tensor.matmul(
                                        s2,
                                        OT8[a][:, pl, nt * 128 : nt * 128 + 128],
                                        wof[:, b, pl, dh * 512 : dh * 512 + 512],
                                        start=(i == 0), stop=(i == 11),
                                        perf_mode=DR,
                                    )
                                    i += 1
                            hsl = slice(dh * 512, dh * 512 + 512)
                            nc.vector.scalar_tensor_tensor(
                                out=x1_sb[:, nt, hsl],
                                in0=s2, scalar=O_DS,
                                in1=srar[:, nt, hsl],
                                op0=OP.mult, op1=OP.add,
                            )
                            # half-stats immediately: shortens the LN chain
                            nc.vector.bn_stats(
                                out=st1[nt][:, dh, :], in_=x1_sb[:, nt, hsl]
                            )
                        if nt >= 1:
                            transposes(nt - 1, xqs.pop(nt - 1))
                        mv = lnp.tile([128, 2], F32, tag="lnmv", name="lnmv")
                        nc.vector.bn_aggr(out=mv, in_=st1[nt])
                        nc.scalar.activation(
                            out=mv[:, 1:2], in_=mv[:, 1:2], func=AF.Sqrt,
                            bias=eps_sb, scale=1.0,
                        )
                        nc.vector.reciprocal(out=mv[:, 1:2], in_=mv[:, 1:2])
                        # apply + quantize per half so each half's fp8 planes
                        # chain independently (dh0 on DVE, dh1 on Pool)
                        xq = xqp.tile([128, D], BF16, tag="xq", name="xq")
                        xqs[nt] = xq
                        for hh2, eng in ((0, nc.vector), (1, nc.gpsimd)):
                            h2 = slice(hh2 * 512, hh2 * 512 + 512)
                            eng.tensor_scalar(
                                out=x1_sb[:, nt, h2], in0=x1_sb[:, nt, h2],
                                scalar1=mv[:, 0:1], scalar2=mv[:, 1:2],
                                op0=OP.subtract, op1=OP.mult,
                            )
                            if g1bc is not None:
                                eng.tensor_mul(
                                    out=x1_sb[:, nt, h2],
                                    in0=x1_sb[:, nt, h2], in1=g1bc[:, h2],
                                )
                            if be1bc is not None:
                                eng.tensor_add(
                                    out=x1_sb[:, nt, h2],
                                    in0=x1_sb[:, nt, h2], in1=be1bc[:, h2],
                                )
                            quantize_half(nt, h2, xq)
                    transposes(3, xqs.pop(3))

                # --- phase 4: FFN1 (fp8 DoubleRow matmuls, gelu into fp8 h1T) ---
                h1T_hi = ffn.tile([128, 32, NT], F8, tag="h1Th")
                h1T_lo = (
                    ffn.tile([128, 32, NT], F8, tag="h1Tl", name="h1T_lo") if FFN_H_SPLIT else None
                )
                NPRE = 0
                with (
                    tc.tile_pool(name="h1gp", bufs=3) as h1gp,
                    tc.tile_pool(name="psH", bufs=3, space="PSUM") as psH,
                    tc.tile_pool(name="psHp", bufs=max(NPRE, 1), space="PSUM") as psHp,
                ):
                    # term list: (x plane, w plane); lo*lo is negligible
                    x_terms = [(x1T_hi, 0), (x1T_lo, 0), (x1T_hi, 1)] \
                        if FFN_X_SPLIT else [(x1T_hi, 0), (x1T_hi, 1)]

                    def dma_w1(ft):
                        w1 = w1p.tile([128, 2, 8, 128], F8, tag="w1col", name="w1")
                        # ring-buffer DMAs block SP on their WAR semaphore;
                        # the 20-deep ring keeps the WAR anchor far behind
                        # consumption so SP's queue never stalls.
                        nc.sync.dma_start(
                            out=w1.rearrange("p w c n -> p w (c n)"),
                            in_=W1S[:, ft, :, :].rearrange("w p n -> p w n"),
                        )
                        return w1

                    def mm_ft_nt(hps, w1, ft, nt):
                        nsl = slice(nt * 128, nt * 128 + 128)
                        i = 0
                        for xh, wp in x_terms:
                            for dr in range(4):
                                nc.tensor.matmul(
                                    hps[:, nsl],
                                    w1[:, wp, 2 * dr : 2 * dr + 2, :],
                                    xh[:, 2 * dr : 2 * dr + 2, nsl],
                                    start=(i == 0), stop=(i == 11),
                                    perf_mode=DR,
                                )
                                i += 1

                    def h1_quant(hps, ft):
                        # PSUM holds (X*W1 prescales)*h; descale via gelu's
                        # input scale, rescale the fp8 planes by H_PRESCALE.
                        in_ds = 1.0 / (X_PRESCALE * W1_PRESCALE)
                        if FFN_H_SPLIT:
                            h1g = h1gp.tile([128, NT], BF16, tag="h1g", name="h1g")
                            nc.scalar.activation(
                                out=h1g, in_=hps, func=AF.Gelu,
                                bias=b1_sb[:, ft : ft + 1], scale=in_ds,
                            )
                            nc.vector.tensor_scalar_mul(
                                out=h1T_hi[:, ft, :], in0=h1g, scalar1=H_PRESCALE
                            )
                            nc.vector.scalar_tensor_tensor(
                                out=h1T_lo[:, ft, :], in0=h1g, scalar=H_PRESCALE,
                                in1=h1T_hi[:, ft, :], op0=OP.mult, op1=OP.subtract,
                            )
                        else:
                            nc.scalar.activation(
                                out=h1T_hi[:, ft, :], in_=hps, func=AF.Gelu,
                                bias=b1_sb[:, ft : ft + 1], scale=in_ds,
                            )

                    # The first NPRE fts run nt-sliced and nt-major: their
                    # (ft, nt) groups start as each x1T token tile lands,
                    # filling the PE during the phase-3 LN/quantize drain.
                    pre_w1 = [dma_w1(ft) for ft in range(NPRE)]
                    pre_h = [
                        psHp.tile([128, NT], F32, tag="h1p", name="hpsp")
                        for _ in range(NPRE)
                    ]
                    for nt in range(3):
                        for ft in range(NPRE):
                            mm_ft_nt(pre_h[ft], pre_w1[ft], ft, nt)
                    for ft in range(NPRE):
                        mm_ft_nt(pre_h[ft], pre_w1[ft], ft, 3)
                        h1_quant(pre_h[ft], ft)
                    for ft in range(NPRE, 32):
                        w1 = dma_w1(ft)
                        if NPRE <= ft < NPRE + 8:
                            dma_w2(0, ft - NPRE)
                            dma_w2(1, ft - NPRE)
                        hps = psH.tile([128, NT], F32, tag="h1", name="hps")
                        nmm = 4 * len(x_terms)
                        i = 0
                        for xh, wp in x_terms:
                            for dr in range(4):
                                nc.tensor.matmul(
                                    hps,
                                    w1[:, wp, 2 * dr : 2 * dr + 2, :],
                                    xh[:, 2 * dr : 2 * dr + 2, :],
                                    start=(i == 0), stop=(i == nmm - 1),
                                    perf_mode=DR,
                                )
                                i += 1
                        h1_quant(hps, ft)

                # --- phase 5: FFN2 (dh-major) + residual + LN2 + store ---
                # LN2 stats for the dh0 half are computed during the dh0
                # pass; after the dh1 STT only sg1 stats + apply + store
                # remain on the critical path.
                out_v = out[:, :].rearrange("(nt p) d -> p nt d", p=128)
                with tc.tile_pool(name="psY", bufs=3, space="PSUM") as psY:
                    h_terms = [(h1T_hi, 0), (h1T_lo, 0), (h1T_hi, 1)] \
                        if FFN_H_SPLIT else [(h1T_hi, 0), (h1T_hi, 1)]
                    y_ds = 1.0 / (
                        (H_PRESCALE if FFN_H_SPLIT else 1.0) * W2_PRESCALE
                    )
                    st2 = [
                        lnp.tile([128, 2, 6], F32, tag=f"ln2s{nt}", name=f"ln2s{nt}")
                        for nt in range(4)
                    ]
                    for dh in range(2):
                        for nt in range(4):
                            yps = psY.tile([128, 512], F32, tag="y", name="yps")
                            nmm = 16 * len(h_terms)
                            i = 0
                            for hh, wp in h_terms:
                                for dr in range(16):
                                    nc.tensor.matmul(
                                        yps,
                                        hh[:, 2 * dr : 2 * dr + 2, nt * 128 : nt * 128 + 128],
                                        W2h[:, dh, wp, 2 * dr : 2 * dr + 2, :],
                                        start=(i == 0), stop=(i == nmm - 1),
                                        perf_mode=DR,
                                    )
                                    i += 1
                            hsl = slice(dh * 512, dh * 512 + 512)
                            nc.vector.scalar_tensor_tensor(
                                out=x1_sb[:, nt, hsl],
                                in0=yps, scalar=y_ds,
                                in1=x1_sb[:, nt, hsl],
                                op0=OP.mult, op1=OP.add,
                            )
                            if b2bc is not None:
                                nc.vector.tensor_add(
                                    out=x1_sb[:, nt, hsl],
                                    in0=x1_sb[:, nt, hsl],
                                    in1=b2bc[:, hsl],
                                )
                            nc.vector.bn_stats(
                                out=st2[nt][:, dh, :], in_=x1_sb[:, nt, hsl]
                            )
                            if dh == 1:
                                mv = lnp.tile([128, 2], F32, tag="ln2mv", name="ln2mv")
                                nc.vector.bn_aggr(out=mv, in_=st2[nt])
                                nc.scalar.activation(
                                    out=mv[:, 1:2], in_=mv[:, 1:2], func=AF.Sqrt,
                                    bias=eps_sb, scale=1.0,
                                )
                                nc.vector.reciprocal(out=mv[:, 1:2], in_=mv[:, 1:2])
                                for hh2 in range(2):
                                    h2 = slice(hh2 * 512, hh2 * 512 + 512)
                                    nc.vector.tensor_scalar(
                                        out=x1_sb[:, nt, h2], in0=x1_sb[:, nt, h2],
                                        scalar1=mv[:, 0:1], scalar2=mv[:, 1:2],
                                        op0=OP.subtract, op1=OP.mult,
                                    )
                                    if g2bc is not None:
                                        nc.vector.tensor_mul(
                                            out=x1_sb[:, nt, h2],
                                            in0=x1_sb[:, nt, h2], in1=g2bc[:, h2],
                                        )
                                    if be2bc is not None:
                                        nc.vector.tensor_add(
                                            out=x1_sb[:, nt, h2],
                                            in0=x1_sb[:, nt, h2], in1=be2bc[:, h2],
                                        )
                                    nc.sync.dma_start(
                                        out=out_v[:, nt, h2], in_=x1_sb[:, nt, h2]
                                    )

    nc.finalize()
    return nc


def _hilo8(a):
    """Stack round-to-nearest fp8 hi and residual lo planes: [2, *a.shape]."""
    hi = np.asarray(a, np.float32).astype(F8NP)
    lo = (np.asarray(a, np.float32) - hi.astype(np.float32)).astype(F8NP)
    return np.ascontiguousarray(np.stack([hi, lo], axis=0))


def host_prep(inputs):
    """Build the 8 per-core input maps from the full problem inputs."""
    src = np.asarray(inputs["src"], np.float32)
    coords = np.asarray(inputs["coords"])
    Wq = np.asarray(inputs["Wq"], np.float32)
    Wk = np.asarray(inputs["Wk"], np.float32)
    Wv = np.asarray(inputs["Wv"], np.float32)
    Wo = np.asarray(inputs["Wo"], np.float32)
    W1 = np.asarray(inputs["W1"], np.float32)
    b1 = np.asarray(inputs["b1"], np.float32)
    W2 = np.asarray(inputs["W2"], np.float32)
    b2 = np.asarray(inputs["b2"], np.float32)
    g1 = np.asarray(inputs["g1"], np.float32)
    be1 = np.asarray(inputs["be1"], np.float32)
    g2 = np.asarray(inputs["g2"], np.float32)
    be2 = np.asarray(inputs["be2"], np.float32)

    def _blk8(wt, nblk, blk):
        # [dt, w, p, c*blk + j] from wt.T-like [c*128+p, dt*blk+j]
        x = (WP_PRESCALE * wt.T).reshape(8, 128, nblk, blk)
        x = x.transpose(2, 1, 0, 3).reshape(nblk, 128, 8 * blk)
        return np.ascontiguousarray(_hilo8(x).transpose(1, 0, 2, 3))

    # Projection weights as fp8 hi/lo planes; the per-head SCALE/slope_h for
    # q goes in as the PSUM->qaug copy descale on device.
    shared = {
        "WqS8": _blk8(Wq, 8, 128),
        "WkS8": _blk8(Wk, 8, 128),
        "WvS8": _blk8(Wv, 2, 512),
        "WoT8": _hilo8(WP_PRESCALE * Wo.T),
        # W1S[w, ft, p, dc*128+j] = hi/lo fp8 planes of W1.T[dc*128+p, ft*128+j]
        "W1S": _hilo8(
            (W1_PRESCALE * W1.T)
            .reshape(8, 128, 32, 128).transpose(2, 1, 0, 3).reshape(32, 128, D)
        ),
        # W2S[dh, w, dff, j] = hi/lo planes of W2.T[dff, dh*512+j]
        "W2S": np.ascontiguousarray(
            _hilo8(
                (W2_PRESCALE * W2.T).reshape(DFF, 2, 512).transpose(1, 0, 2)
            ).transpose(1, 0, 2, 3)
        ),
        "b1r": np.ascontiguousarray(b1.reshape(32, 128).T),
        "b2": b2.reshape(1, D),
        "g1": g1.reshape(1, D),
        "be1": be1.reshape(1, D),
        "g2": g2.reshape(1, D),
        "be2": be2.reshape(1, D),
    }

    in_maps = []
    for c in range(NCORES):
        b = c // 2
        half = c % 2
        rows = slice(half * NT, (half + 1) * NT)
        # key-axis permutation: own query rows first (Q proj reads the first
        # NT columns of srcT8); keys are a contraction axis everywhere, so
        # only kaug_x must be permuted consistently.
        perm = np.r_[half * NT : (half + 1) * NT, (1 - half) * NT : (2 - half) * NT]
        x = coords[b, :, 0].astype(np.float64)
        y = coords[b, :, 1].astype(np.float64)
        s = (x + y).astype(np.float32)
        thr = np.arange(1, GRID, dtype=np.float64)
        cx = (x[None, :] >= thr[:, None]).astype(np.float32)
        cy = (y[None, :] >= thr[:, None]).astype(np.float32)
        kaug = np.concatenate(
            [s.reshape(1, N), np.zeros((1, N), np.float32), cx, cy], axis=0
        ).astype(BF)
        qaug = np.empty((64, NT), np.float32)
        qaug[0, :] = 1.0
        qaug[1, :] = 0.0
        qaug[2:33, :] = -2.0 * cx[:, rows]
        qaug[33:64, :] = -2.0 * cy[:, rows]
        srcTb = np.ascontiguousarray(src[b].T[:, perm])
        m = dict(shared)
        m.update(
            {
                "srcT8": _hilo8(S_PRESCALE * srcTb),
                "src_rows": np.ascontiguousarray(src[b, rows, :]),
                "kaug_x": np.ascontiguousarray(kaug[:, perm]).reshape(64, 1, N),
                "qaug_x": qaug.astype(BF).reshape(64, 1, NT),
            }
        )
        in_maps.append(m)
    return in_maps


_NCS = {}
LAST_RUN_S = None


def get_nc(trivial_affine=True):
    if trivial_affine not in _NCS:
        _NCS[trivial_affine] = build_nc(trivial_affine)
    return _NCS[trivial_affine]


def _affine_trivial(inputs):
    return (
        np.all(np.asarray(inputs["g1"]) == 1.0)
        and np.all(np.asarray(inputs["g2"]) == 1.0)
        and not np.any(np.asarray(inputs["be1"]))
        and not np.any(np.asarray(inputs["be2"]))
        and not np.any(np.asarray(inputs["b2"]))
    )


def kernel(**inputs):
    global LAST_RUN_S
    from concourse.bass_utils import run_bass_kernel_spmd

    nc = get_nc(bool(_affine_trivial(inputs)))
    in_maps = host_prep(inputs)
    t0 = time.monotonic()
    res = run_bass_kernel_spmd(nc, in_maps, list(range(NCORES)))
    LAST_RUN_S = time.monotonic() - t0
    full = np.empty((B, N, D), np.float32)
    for c in range(NCORES):
        b = c // 2
        half = c % 2
        full[b, half * NT : (half + 1) * NT, :] = res.results[c]["out"]
    return full



# revision 72
# speedup vs baseline: 1.2133x; 1.0115x over previous
"""Fused transformer encoder layer (attention w/ 2D-ALiBi bias + FFN) on 8 trn2 cores.

Sharding: core c handles batch b = c//2, token half h = c%2 (512 query rows).
K/V are computed per-core for the full 1024-token sequence of its batch
(duplicated across the 2 cores sharing a batch); outputs are disjoint row
slices of the final tensor, so no collectives are needed.

Bias trick: the alibi_2d bias slope_h*(|xi-xj|+|yi-yj|) is folded into the
QK^T contraction. |xi-xj| = xi + xj - 2*a_i.a_j with a_i in {0,1}^31 the
threshold indicators of xi, so dist(i,j) = s_i + s_j - 2*c_i.c_j (c = 62-dim
indicator, s = x+y). The per-query term slope*s_i is constant along the
softmax axis and is dropped. Q/K are augmented with 64 extra contraction dims
(s_j / pad / c_j on the K side; 1 / 0 / -2*c_i on the Q side), making the
score contraction K = 64+64 = 128 exactly — full PE array, bias for free.

bf16 precision care: the aug rows are small integers / {0,-2} — exact in
bf16. The attention scale AND the per-head slope are folded out of the bf16
data: Q-projection weights carry scale/slope_h per head (so scores come out
as S/slope_h) and the exact fp32 slope_h is re-applied as the exp()
activation's scale immediate. exp needs no max-subtraction (|S| <= ~50 by
construction).

Scores are computed keys-on-partitions (S^T layout) so the exp() output is
already P^T for the AV matmul (no transpose). Softmax denominators come from
an appended ones-column in V; normalization is deferred past the (linear)
output projection boundary: each head's O^T rows are scaled by a broadcasted
1/den (built with a small fp32 selector matmul) before the head-summing
projection.
"""

import math
import sys
import time

for _p in ("/opt/trn_rl_repo",):
    if _p not in sys.path:
        sys.path.insert(0, _p)

import numpy as np
import ml_dtypes

import concourse.bass as bass
import concourse.tile as tile
from concourse import bacc, mybir
from concourse.masks import make_identity

F32 = mybir.dt.float32
F32R = mybir.dt.float32r
BF16 = mybir.dt.bfloat16
F8 = mybir.dt.float8e4
BF = ml_dtypes.bfloat16
F8NP = ml_dtypes.float8_e4m3
DR = mybir.MatmulPerfMode.DoubleRow

# fp8 FFN config: activations split into fp8 hi+lo planes (quantization error
# feedback), weights plain fp8. Splitting halves the DoubleRow speedup for
# that operand but removes its quantization error from the output.
FFN_X_SPLIT = True   # x1T (FFN1 input) hi/lo
FFN_H_SPLIT = True   # h1T (FFN2 input) hi/lo

# fp8e4m3 normals span [2^-6, 448]; W1/W2 entries (sigma ~ 1/32) and the lo
# planes would otherwise land in the subnormal range and lose most precision,
# so everything is pre-scaled up into the normal range and the product scale
# is divided back out at the PSUM->SBUF boundary (gelu scale / y descale).
W1_PRESCALE = 256.0
W2_PRESCALE = 256.0
X_PRESCALE = 16.0
H_PRESCALE = 32.0
S_PRESCALE = 8.0     # src (sigma 1) for the QKV projections
WP_PRESCALE = 256.0  # Wq/Wk/Wv/Wo (sigma 1/32)
OT_PRESCALE = 16.0   # normalized attention outputs (sigma ~1)

D = 1024          # d_model
H = 16            # heads
HD = 64           # head dim
DFF = 4096
B = 4
N = 1024          # sequence length
NT = 512          # tokens (query rows) per core
GRID = 32
EPS = 1e-5
NCORES = 8
SCALE = HD ** -0.5


def _alibi_slopes(n):
    def pow2(n_):
        start = 2.0 ** (-(2.0 ** -(math.log2(n_) - 3)))
        return [start * start ** i for i in range(n_)]
    if math.log2(n).is_integer():
        return np.array(pow2(n), dtype=np.float64)
    m = 2 ** math.floor(math.log2(n))
    s = pow2(m)
    s += [s[-1] * 0.5 ** (i + 1) for i in range(n - m)]
    return np.array(s, dtype=np.float64)


SLOPES = _alibi_slopes(H)


def build_nc(trivial_affine=False):
    """trivial_affine: g1/g2 all-ones and be1/be2/b2 all-zeros -> skip those ops."""
    nc = bacc.Bacc()

    # srcT8 columns are permuted per-core so the core's own 512 query rows
    # come FIRST (Q proj reads stf[:, :, :, 0:NT]); key order is a contraction
    # axis everywhere else, so the permutation is invisible provided kaug_x
    # and the V layout use the same order (host_prep keeps them consistent).
    srcT8 = nc.declare_dram_parameter("srcT8", [2, D, N], F8, isOutput=False)
    src_rows = nc.declare_dram_parameter("src_rows", [NT, D], F32, isOutput=False)
    # Wq/Wk swizzled per output-block dt (head pair): WqS8[dt, w, p, c*128+j]
    # = Wq.T[c*128+p, dt*128+j], so each dt block (both planes) is one
    # contiguous DMA and head pair dt can project as soon as it lands.
    WqS8 = nc.declare_dram_parameter("WqS8", [8, 2, 128, D], F8, isOutput=False)
    WkS8 = nc.declare_dram_parameter("WkS8", [8, 2, 128, D], F8, isOutput=False)
    # Wv swizzled by dh half: WvS8[dh, w, p, c*512+j] = Wv.T[c*128+p, dh*512+j]
    WvS8 = nc.declare_dram_parameter("WvS8", [2, 2, 128, 8 * 512], F8, isOutput=False)
    WoT8 = nc.declare_dram_parameter("WoT8", [2, D, D], F8, isOutput=False)
    # W1 pre-swizzled on host: W1S[w, ft, p, dc*128+j] = W1.T[dc*128+p, ft*128+j]
    # (w = fp8 hi/lo plane) so each FFN1 weight block is contiguous per plane.
    W1S = nc.declare_dram_parameter("W1S", [2, 32, 128, D], F8, isOutput=False)
    # W2 split by output half dh so dh0 can stream during FFN1 and dh1
    # during the FFN2 dh0 pass: W2S[dh, w, dff, j] = W2.T[dff, dh*512+j]
    W2S = nc.declare_dram_parameter("W2S", [2, 2, DFF, 512], F8, isOutput=False)
    # aug blocks are head-independent; the singleton dim enables stride-0
    # broadcast DMAs into all head slots.
    kaug_x = nc.declare_dram_parameter("kaug_x", [64, 1, N], BF16, isOutput=False)
    qaug_x = nc.declare_dram_parameter("qaug_x", [64, 1, NT], BF16, isOutput=False)
    b1r = nc.declare_dram_parameter("b1r", [128, 32], F32, isOutput=False)
    b2 = nc.declare_dram_parameter("b2", [1, D], F32, isOutput=False)
    g1 = nc.declare_dram_parameter("g1", [1, D], F32, isOutput=False)
    be1 = nc.declare_dram_parameter("be1", [1, D], F32, isOutput=False)
    g2 = nc.declare_dram_parameter("g2", [1, D], F32, isOutput=False)
    be2 = nc.declare_dram_parameter("be2", [1, D], F32, isOutput=False)
    out = nc.declare_dram_parameter("out", [NT, D], F32, isOutput=True)

    AF = mybir.ActivationFunctionType
    OP = mybir.AluOpType

    with tile.TileContext(nc) as tc:
        with (
            tc.tile_pool(name="misc", bufs=1) as misc,
            tc.tile_pool(name="lnp", bufs=4) as lnp,
            tc.tile_pool(name="pre", bufs=1) as pre,
        ):
            eps_sb = misc.tile([128, 1], F32, tag="eps")
            nc.vector.memset(eps_sb, EPS)
            ident = misc.tile([128, 128], F32, tag="ident")
            make_identity(nc, ident)
            ident_bf = misc.tile([128, 128], BF16, tag="identbf")
            make_identity(nc, ident_bf)
            ident_f8 = misc.tile([128, 128], F8, tag="identf8")
            make_identity(nc, ident_f8)
            # OT8[p, c, q]: head 2c in partitions 0:64, head 2c+1 in 64:128;
            # fp8 hi/lo planes (scaled by OT_PRESCALE) for the 3-term out-proj
            OT8_hi = misc.tile([128, 8, NT], F8, tag="ot8h")

            def ln_apply(x_ap, gbc, bbc):
                stats = lnp.tile([128, 2, 6], F32, tag="lnstats", name="lnstats")
                for sg in range(2):
                    nc.vector.bn_stats(
                        out=stats[:, sg, :], in_=x_ap[:, sg * 512 : sg * 512 + 512]
                    )
                mv = lnp.tile([128, 2], F32, tag="lnmv", name="lnmv")
                nc.vector.bn_aggr(out=mv, in_=stats)
                nc.scalar.activation(
                    out=mv[:, 1:2], in_=mv[:, 1:2], func=AF.Sqrt,
                    bias=eps_sb, scale=1.0,
                )
                nc.vector.reciprocal(out=mv[:, 1:2], in_=mv[:, 1:2])
                nc.vector.tensor_scalar(
                    out=x_ap, in0=x_ap,
                    scalar1=mv[:, 0:1], scalar2=mv[:, 1:2],
                    op0=OP.subtract, op1=OP.mult,
                )
                if gbc is not None:
                    nc.vector.tensor_mul(out=x_ap, in0=x_ap, in1=gbc)
                if bbc is not None:
                    nc.vector.tensor_add(out=x_ap, in0=x_ap, in1=bbc)

            # ============ merged projections + attention ============
            # Single software pipeline: per step s, project K/Q for head pair
            # s, run scores+exp for pair s-1, AV+normalize for pair s-2, and
            # transpose/quantize pair s-3. V projections burst at steps 1/4.
            # AV runs in natural orientation (A = P^T chunk, B = V columns
            # incl. a ones column -> out [128 queries, 64 vdims + den]), so
            # softmax denominators are per-PARTITION and normalization is a
            # plain tensor_scalar; O^T for the out-proj comes from cheap bf16
            # PE transposes of head pairs.
            # wqk opens before att so its released zone sits at the stack
            # bottom: the FFN W1 ring reuses it, anchored on the early
            # K(7)/Q(7) last-uses instead of the late attention tail.
            with (
                tc.tile_pool(name="wqk", bufs=1) as wqk,
                tc.tile_pool(name="att", bufs=1) as att,
            ):
                kaug = att.tile([128, H, N], BF16, tag="kaug")
                qaug = att.tile([128, H, NT], BF16, tag="qaug")
                v_sb = att.tile([128, 8, H * 65], BF16, tag="vsb")
                v4 = v_sb.rearrange("p m (h w) -> p m h w", w=65)
                nc.vector.memset(v4[:, :, :, 64], 1.0)

                P_DS = 1.0 / (S_PRESCALE * WP_PRESCALE)
                # (activation plane, weight plane) product terms; lo*lo skipped
                TERMS = ((0, 0), (1, 0), (0, 1))

                def mm3t(ps, w8, x8, wslice, xslice):
                    i = 0
                    for a, b in TERMS:
                        for dr in range(4):
                            pl = slice(2 * dr, 2 * dr + 2)
                            nc.tensor.matmul(
                                ps,
                                w8[:, b, pl, wslice],
                                x8[:, a, pl, xslice],
                                start=(i == 0), stop=(i == 11),
                                perf_mode=DR,
                            )
                            i += 1

                with tc.tile_pool(name="ph1", bufs=1) as ph1:
                    # DMA emission order tracks first-use order; weights are
                    # host-swizzled so each head pair's block is one
                    # contiguous transfer.
                    wqf = wqk.tile([128, 8, 2, 8, 128], F8, tag="wqf")
                    stf = ph1.tile([128, 2, 8, N], F8, tag="stf")
                    st_vw = srcT8[:, :, :].rearrange("w (c p) n -> p w c n", p=128)
                    wkf = wqk.tile([128, 8, 2, 8, 128], F8, tag="wkf")
                    wvf = ph1.tile([128, 2, 2, 8, 512], F8, tag="wvf")

                    def dma_wdt(dst, src, dt):
                        nc.sync.dma_start(
                            out=dst[:, dt, :, :, :],
                            in_=src[dt, :, :, :].rearrange(
                                "w p (c n) -> p w c n", c=8
                            ),
                        )

                    def dma_aug(h0, h1):
                        # stride-0 broadcast of the shared aug block into
                        # head slots [h0, h1)
                        nc.sync.dma_start(
                            out=kaug[64:128, h0:h1, :],
                            in_=kaug_x[:, :, :].to_broadcast([64, h1 - h0, N]),
                        )
                        nc.sync.dma_start(
                            out=qaug[64:128, h0:h1, :],
                            in_=qaug_x[:, :, :].to_broadcast([64, h1 - h0, NT]),
                        )

                    # need-ordered: Q(0) first (own src columns + wq block 0),
                    # then K(0), aug for early heads, V(dh0), and the rest
                    # staggered against consumption.
                    nc.sync.dma_start(out=stf[:, 0, :, 0:NT], in_=st_vw[:, 0, :, 0:NT])
                    dma_wdt(wqf, WqS8, 0)
                    nc.sync.dma_start(out=stf[:, 1, :, 0:NT], in_=st_vw[:, 1, :, 0:NT])
                    nc.sync.dma_start(out=stf[:, 0, :, NT:N], in_=st_vw[:, 0, :, NT:N])
                    dma_wdt(wkf, WkS8, 0)
                    nc.sync.dma_start(out=stf[:, 1, :, NT:N], in_=st_vw[:, 1, :, NT:N])
                    dma_aug(0, 2)
                    dma_wdt(wqf, WqS8, 1)
                    dma_wdt(wkf, WkS8, 1)
                    for w in range(2):
                        nc.sync.dma_start(
                            out=wvf[:, 0, w, :, :],
                            in_=WvS8[0, w, :, :].rearrange("p (c n) -> p c n", c=8),
                        )
                    dma_aug(2, 4)
                    dma_wdt(wqf, WqS8, 2)
                    dma_wdt(wkf, WkS8, 2)
                    dma_aug(4, 6)
                    dma_wdt(wqf, WqS8, 3)
                    dma_wdt(wkf, WkS8, 3)
                    dma_aug(6, 8)
                    dma_wdt(wqf, WqS8, 4)
                    dma_wdt(wkf, WkS8, 4)
                    dma_aug(8, 12)
                    for w in range(2):
                        nc.sync.dma_start(
                            out=wvf[:, 1, w, :, :],
                            in_=WvS8[1, w, :, :].rearrange("p (c n) -> p c n", c=8),
                        )
                    dma_aug(12, 16)
                    for dt in range(5, 8):
                        dma_wdt(wqf, WqS8, dt)
                        dma_wdt(wkf, WkS8, dt)

                    # phase 3-5 prefetches ride the queue tail; they land
                    # long before the out-proj needs them.
                    if trivial_affine:
                        g1bc = be1bc = b2bc = g2bc = be2bc = None
                    else:
                        g1bc = pre.tile([128, D], F32, tag="g1bc")
                        be1bc = pre.tile([128, D], F32, tag="be1bc")
                        b2bc = pre.tile([128, D], F32, tag="b2bc")
                        g2bc = pre.tile([128, D], F32, tag="g2bc")
                        be2bc = pre.tile([128, D], F32, tag="be2bc")
                        for t_, src_ in (
                            (g1bc, g1), (be1bc, be1),
                            (b2bc, b2), (g2bc, g2), (be2bc, be2),
                        ):
                            nc.sync.dma_start(
                                out=t_, in_=src_[:, :].to_broadcast([128, D])
                            )
                    srar = pre.tile([128, 4, D], F32, tag="srcrows")
                    nc.sync.dma_start(
                        out=srar,
                        in_=src_rows[:, :].rearrange("(nt p) d -> p nt d", p=128),
                    )
                    wof = pre.tile([128, 2, 8, D], F8, tag="wof")
                    wo_vw = WoT8[:, :, :].rearrange("w (c p) n -> p w c n", p=128)
                    for w in range(2):
                        nc.sync.dma_start(out=wof[:, w, :, :], in_=wo_vw[:, w, :, :])
                    b1_sb = pre.tile([128, 32], F32, tag="b1")
                    nc.sync.dma_start(out=b1_sb, in_=b1r[:, :])

                    with (
                        tc.tile_pool(name="ptp", bufs=2) as ptp,
                        tc.tile_pool(name="obp", bufs=3) as obp,
                        tc.tile_pool(name="rcp", bufs=2) as rcp,
                        tc.tile_pool(name="psS", bufs=2, space="PSUM") as psS,
                        tc.tile_pool(name="psP", bufs=2, space="PSUM") as psP,
                        tc.tile_pool(name="psO", bufs=1, space="PSUM") as psO,
                        tc.tile_pool(name="psT", bufs=1, space="PSUM") as psT,
                    ):
                        pts = {}
                        obfs = {}

                        # p-state warmup: the PE needs ~3us of continuous
                        # execution to reach 2.4 GHz; burn the DMA cold-start
                        # on dependency-free dummy matmuls so the first real
                        # projections run at full clock.
                        wu = psP.tile([128, 512], F32, tag="pj", name="warm")
                        for _ in range(8):
                            nc.tensor.matmul(
                                wu[:, 0:128], ident, ident, start=True, stop=True
                            )

                        def mm3p(ps, w8, dt, xslice):
                            i = 0
                            for a, b in TERMS:
                                for dr in range(4):
                                    pl = slice(2 * dr, 2 * dr + 2)
                                    nc.tensor.matmul(
                                        ps,
                                        w8[:, dt, b, pl, :],
                                        stf[:, a, pl, xslice],
                                        start=(i == 0), stop=(i == 11),
                                        perf_mode=DR,
                                    )
                                    i += 1

                        def emit_K(dt, mh):
                            kps = psP.tile([128, 512], F32, tag="pj", name="kps")
                            mm3p(kps, wkf, dt, slice(mh * 512, mh * 512 + 512))
                            ksl = slice(mh * 512, mh * 512 + 512)
                            nc.vector.tensor_scalar_mul(
                                out=kaug[0:64, 2 * dt, ksl],
                                in0=kps[0:64, :], scalar1=P_DS,
                            )
                            nc.vector.tensor_scalar_mul(
                                out=kaug[0:64, 2 * dt + 1, ksl],
                                in0=kps[64:128, :], scalar1=P_DS,
                            )

                        def emit_Q(dt):
                            # own query rows are the FIRST NT columns of stf
                            qps = psP.tile([128, NT], F32, tag="pj", name="qps")
                            mm3p(qps, wqf, dt, slice(0, NT))
                            nc.vector.tensor_scalar_mul(
                                out=qaug[0:64, 2 * dt, :], in0=qps[0:64, :],
                                scalar1=float(SCALE / SLOPES[2 * dt] * P_DS),
                            )
                            nc.vector.tensor_scalar_mul(
                                out=qaug[0:64, 2 * dt + 1, :], in0=qps[64:128, :],
                                scalar1=float(SCALE / SLOPES[2 * dt + 1] * P_DS),
                            )

                        def emit_V_pair(p):
                            # V projection for head pair p only (just-in-time
                            # for AV(2p) one step later; spreads V across the
                            # pipeline and fills the exp-drain tail)
                            dh, hc = p // 4, (p % 4) * 128
                            for mg in range(2):
                                vt = psP.tile(
                                    [128, 4, 128], F32, tag="pj", name="vps"
                                )
                                for lm in range(4):
                                    mt = mg * 4 + lm
                                    i = 0
                                    for a, b in TERMS:
                                        for dr in range(4):
                                            pl = slice(2 * dr, 2 * dr + 2)
                                            nc.tensor.matmul(
                                                vt[:, lm, :],
                                                stf[:, a, pl, mt * 128 : mt * 128 + 128],
                                                wvf[:, dh, b, pl, hc : hc + 128],
                                                start=(i == 0), stop=(i == 11),
                                                perf_mode=DR,
                                            )
                                            i += 1
                                nc.vector.tensor_scalar_mul(
                                    out=v4[:, mg * 4 : mg * 4 + 4, 2 * p : 2 * p + 2, 0:64],
                                    in0=vt.rearrange("p m (h w) -> p m h w", w=64),
                                    scalar1=P_DS,
                                )

                        def emit_S(h, cs):
                            pt = pts[h]
                            for c in cs:
                                stc = psS.tile([128, 2, NT], F32, tag="st", name="stc")
                                for j in range(2):
                                    mt = 2 * c + j
                                    nc.tensor.matmul(
                                        stc[:, j, :],
                                        kaug[:, h, mt * 128 : mt * 128 + 128],
                                        qaug[:, h, :],
                                        start=True, stop=True,
                                    )
                                nc.scalar.activation(
                                    out=pt[:, 2 * c : 2 * c + 2, :], in_=stc,
                                    func=AF.Exp, scale=float(SLOPES[h]),
                                )

                        def emit_AV(h):
                            # natural orientation: out [128 queries, 64 vdims
                            # + den]; col 64 accumulates the softmax denom via
                            # the ones column in V.
                            cc = h // 2
                            if h % 2 == 0:
                                obfs[cc] = obp.tile(
                                    [128, 4, 128], BF16, tag="ob", name="obf"
                                )
                            po = psO.tile([128, 4, 65], F32, tag="po", name="po")
                            for qc in range(4):
                                for mt in range(8):
                                    nc.tensor.matmul(
                                        po[:, qc, :],
                                        pts[h][:, mt, qc * 128 : qc * 128 + 128],
                                        v4[:, mt, h, 0:65],
                                        start=(mt == 0), stop=(mt == 7),
                                    )
                            rc = rcp.tile([128, 4], F32, tag="rc", name="rc")
                            nc.vector.reciprocal(out=rc, in_=po[:, :, 64])
                            nc.vector.tensor_scalar_mul(
                                out=rc, in0=rc, scalar1=OT_PRESCALE
                            )
                            # normalize promptly on DVE (psO has one buffer;
                            # the next AV waits on these reads)
                            hb = (h % 2) * 64
                            for qc in range(4):
                                nc.vector.tensor_scalar_mul(
                                    out=obfs[cc][:, qc, hb : hb + 64],
                                    in0=po[:, qc, 0:64],
                                    scalar1=rc[:, qc : qc + 1],
                                )
                            pts.pop(h)

                        def emit_pair(cc):
                            ob = obfs.pop(cc)
                            tp = psT.tile([128, 4, 128], BF16, tag="tp", name="tp")
                            for qc in range(4):
                                nc.tensor.transpose(
                                    tp[:, qc, :], ob[:, qc, :], ident_bf
                                )
                            tpf = tp.rearrange("p a b -> p (a b)")
                            # gpsimd cannot touch PSUM on hw; Act does the
                            # copy. Single fp8 plane for O (the out-proj runs
                            # 2-term: O_hi x Wo_hi + O_hi x Wo_lo).
                            nc.scalar.activation(
                                out=OT8_hi[:, cc, :], in_=tpf, func=AF.Copy
                            )

                        def step(s):
                            h0, h1 = 2 * (s - 1), 2 * (s - 1) + 1
                            av0, av1 = 2 * (s - 2), 2 * (s - 2) + 1
                            if 2 <= s <= 9:
                                emit_AV(av0)
                            if s < 8:
                                emit_Q(s)
                            if 1 <= s <= 8:
                                pts[h0] = ptp.tile(
                                    [128, 8, NT], BF16, tag="pt", name="pt"
                                )
                                emit_S(h0, (0, 1))
                            if 2 <= s <= 9:
                                emit_AV(av1)
                            if s < 8:
                                emit_K(s, 0)
                            if 1 <= s <= 8:
                                emit_S(h0, (2, 3))
                            if s < 8:
                                emit_K(s, 1)
                            if 1 <= s <= 8:
                                pts[h1] = ptp.tile(
                                    [128, 8, NT], BF16, tag="pt", name="pt"
                                )
                                emit_S(h1, (0, 1))
                            if 1 <= s <= 8:
                                emit_V_pair(s - 1)
                            if 1 <= s <= 8:
                                emit_S(h1, (2, 3))
                            if s >= 3:
                                emit_pair(s - 3)

                        for s in range(11):
                            step(s)

            # ============ post-attention scope ============
            with (
                tc.tile_pool(name="w1p", bufs=20) as w1p,
                tc.tile_pool(name="ffn", bufs=1) as ffn,
            ):
                # w1p opens FIRST so the W1 ring lands on the earliest-freed
                # attention SBUF and its stream starts during the tail.
                # W2 halves [p, dh, w, c, n]: dh0 streams during FFN1, dh1
                # during the FFN2 dh0 pass (keeps FFN1's W1 stream PE-bound)
                W2h = ffn.tile([128, 2, 2, 32, 512], F8, tag="w2")
                w2_v = [
                    W2S[dh, :, :, :].rearrange("w (c p) n -> p w c n", p=128)
                    for dh in range(2)
                ]

                def dma_w2(dh, cg):
                    # one [128, w, 4, 512] chunk (0.25 MB) per call
                    for w in range(2):
                        nc.sync.dma_start(
                            out=W2h[:, dh, w, 4 * cg : 4 * cg + 4, :],
                            in_=w2_v[dh][:, w, 4 * cg : 4 * cg + 4, :],
                        )
                x1_sb = ffn.tile([128, 4, D], F32, tag="x1")
                x1T_hi = ffn.tile([128, 8, NT], F8, tag="x1Th")
                x1T_lo = (
                    ffn.tile([128, 8, NT], F8, tag="x1Tl", name="x1T_lo") if FFN_X_SPLIT else None
                )

                # --- phase 3: out-proj, LN1, transpose ---
                with (
                    tc.tile_pool(name="psS2", bufs=4, space="PSUM") as psS2,
                    tc.tile_pool(name="psT", bufs=2, space="PSUM") as psT,
                    tc.tile_pool(name="xqp", bufs=2) as xqp,
                ):
                    def transposes(nt, xq):
                        # xq holds bf16 16*LN(x1)[nt]; hw forbids plain fp8
                        # transposes, so transpose bf16 and split hi/lo after.
                        nsl = slice(nt * 128, nt * 128 + 128)
                        tp = psT.tile([128, 8, 128], BF16, tag="tp", name="tp")
                        for c in range(8):
                            nc.tensor.transpose(
                                tp[:, c, :],
                                xq[:, c * 128 : c * 128 + 128],
                                ident_bf,
                            )
                        nc.scalar.activation(
                            out=x1T_hi[:, :, nsl], in_=tp, func=AF.Copy
                        )
                        if FFN_X_SPLIT:
                            nc.vector.tensor_sub(
                                out=x1T_lo[:, :, nsl],
                                in0=tp, in1=x1T_hi[:, :, nsl],
                            )

                    def quantize_half(nt, hsl, xq):
                        nc.scalar.activation(
                            out=xq[:, hsl], in_=x1_sb[:, nt, hsl],
                            func=AF.Copy, scale=X_PRESCALE,
                        )

                    # 2-term out-proj: O is a single fp8 plane (error ~fp8
                    # quant of O, emulated ~+0.007 rel; tolerance 0.02)
                    OP_TERMS = ((0, 0), (0, 1))
                    OT8 = (OT8_hi,)
                    O_DS = 1.0 / (OT_PRESCALE * WP_PRESCALE)
                    st1 = [
                        lnp.tile([128, 2, 6], F32, tag=f"ln1s{nt}", name=f"ln1s{nt}")
                        for nt in range(4)
                    ]
                    xqs = {}
                    for nt in range(4):
                        for dh in range(2):
                            s2 = psS2.tile([128, 512], F32, tag="s2", name="s2")
                            # dr-major: only the last 3 matmuls (head pairs
                            # 6-7) wait on the final attention quantize
                            i = 0
                            for dr in range(4):
                                pl = slice(2 * dr, 2 * dr + 2)
                                for a, b in OP_TERMS:
                                    nc.tensor.matmul(
                                        s2,
                                        OT8[a][:, pl, nt * 128 : nt * 128 + 128],
                                        wof[:, b, pl, dh * 512 : dh * 512 + 512],
                                        start=(i == 0), stop=(i == 7),
                                        perf_mode=DR,
                                    )
                                    i += 1
                            hsl = slice(dh * 512, dh * 512 + 512)
                            nc.vector.scalar_tensor_tensor(
                                out=x1_sb[:, nt, hsl],
                                in0=s2, scalar=O_DS,
                                in1=srar[:, nt, hsl],
                                op0=OP.mult, op1=OP.add,
                            )
                            # half-stats immediately: shortens the LN chain
                            nc.vector.bn_stats(
                                out=st1[nt][:, dh, :], in_=x1_sb[:, nt, hsl]
                            )
                        if nt >= 1:
                            transposes(nt - 1, xqs.pop(nt - 1))
                        mv = lnp.tile([128, 2], F32, tag="lnmv", name="lnmv")
                        nc.vector.bn_aggr(out=mv, in_=st1[nt])
                        nc.scalar.activation(
                            out=mv[:, 1:2], in_=mv[:, 1:2], func=AF.Sqrt,
                            bias=eps_sb, scale=1.0,
                        )
                        nc.vector.reciprocal(out=mv[:, 1:2], in_=mv[:, 1:2])
                        # apply + quantize per half so each half's fp8 planes
                        # chain independently (dh0 on DVE, dh1 on Pool)
                        xq = xqp.tile([128, D], BF16, tag="xq", name="xq")
                        xqs[nt] = xq
                        for hh2, eng in ((0, nc.vector), (1, nc.gpsimd)):
                            h2 = slice(hh2 * 512, hh2 * 512 + 512)
                            eng.tensor_scalar(
                                out=x1_sb[:, nt, h2], in0=x1_sb[:, nt, h2],
                                scalar1=mv[:, 0:1], scalar2=mv[:, 1:2],
                                op0=OP.subtract, op1=OP.mult,
                            )
                            if g1bc is not None:
                                eng.tensor_mul(
                                    out=x1_sb[:, nt, h2],
                                    in0=x1_sb[:, nt, h2], in1=g1bc[:, h2],
                                )
                            if be1bc is not None:
                                eng.tensor_add(
                                    out=x1_sb[:, nt, h2],
                                    in0=x1_sb[:, nt, h2], in1=be1bc[:, h2],
                                )
                            quantize_half(nt, h2, xq)
                    transposes(3, xqs.pop(3))

                # --- phase 4: FFN1 (fp8 DoubleRow matmuls, gelu into fp8 h1T) ---
                h1T_hi = ffn.tile([128, 32, NT], F8, tag="h1Th")
                h1T_lo = (
                    ffn.tile([128, 32, NT], F8, tag="h1Tl", name="h1T_lo") if FFN_H_SPLIT else None
                )
                NPRE = 0
                with (
                    tc.tile_pool(name="h1gp", bufs=3) as h1gp,
                    tc.tile_pool(name="psH", bufs=3, space="PSUM") as psH,
                    tc.tile_pool(name="psHp", bufs=max(NPRE, 1), space="PSUM") as psHp,
                ):
                    # term list: (x plane, w plane); lo*lo is negligible
                    # lo-dependent terms last: the first 8 matmuls of each
                    # group only need the hi plane
                    x_terms = [(x1T_hi, 0), (x1T_hi, 1), (x1T_lo, 0)] \
                        if FFN_X_SPLIT else [(x1T_hi, 0), (x1T_hi, 1)]

                    def dma_w1(ft):
                        w1 = w1p.tile([128, 2, 8, 128], F8, tag="w1col", name="w1")
                        # ring-buffer DMAs block SP on their WAR semaphore;
                        # the 20-deep ring keeps the WAR anchor far behind
                        # consumption so SP's queue never stalls.
                        nc.sync.dma_start(
                            out=w1.rearrange("p w c n -> p w (c n)"),
                            in_=W1S[:, ft, :, :].rearrange("w p n -> p w n"),
                        )
                        return w1

                    def mm_ft_nt(hps, w1, ft, nt):
                        nsl = slice(nt * 128, nt * 128 + 128)
                        i = 0
                        for xh, wp in x_terms:
                            for dr in range(4):
                                nc.tensor.matmul(
                                    hps[:, nsl],
                                    w1[:, wp, 2 * dr : 2 * dr + 2, :],
                                    xh[:, 2 * dr : 2 * dr + 2, nsl],
                                    start=(i == 0), stop=(i == 11),
                                    perf_mode=DR,
                                )
                                i += 1

                    def h1_quant(hps, ft):
                        # PSUM holds (X*W1 prescales)*h; descale via gelu's
                        # input scale, rescale the fp8 planes by H_PRESCALE.
                        in_ds = 1.0 / (X_PRESCALE * W1_PRESCALE)
                        if FFN_H_SPLIT:
                            h1g = h1gp.tile([128, NT], BF16, tag="h1g", name="h1g")
                            nc.scalar.activation(
                                out=h1g, in_=hps, func=AF.Gelu,
                                bias=b1_sb[:, ft : ft + 1], scale=in_ds,
                            )
                            nc.vector.tensor_scalar_mul(
                                out=h1T_hi[:, ft, :], in0=h1g, scalar1=H_PRESCALE
                            )
                            nc.vector.scalar_tensor_tensor(
                                out=h1T_lo[:, ft, :], in0=h1g, scalar=H_PRESCALE,
                                in1=h1T_hi[:, ft, :], op0=OP.mult, op1=OP.subtract,
                            )
                        else:
                            nc.scalar.activation(
                                out=h1T_hi[:, ft, :], in_=hps, func=AF.Gelu,
                                bias=b1_sb[:, ft : ft + 1], scale=in_ds,
                            )

                    # The first NPRE fts run nt-sliced and nt-major: their
                    # (ft, nt) groups start as each x1T token tile lands,
                    # filling the PE during the phase-3 LN/quantize drain.
                    pre_w1 = [dma_w1(ft) for ft in range(NPRE)]
                    pre_h = [
                        psHp.tile([128, NT], F32, tag="h1p", name="hpsp")
                        for _ in range(NPRE)
                    ]
                    for nt in range(3):
                        for ft in range(NPRE):
                            mm_ft_nt(pre_h[ft], pre_w1[ft], ft, nt)
                    for ft in range(NPRE):
                        mm_ft_nt(pre_h[ft], pre_w1[ft], ft, 3)
                        h1_quant(pre_h[ft], ft)
                    for ft in range(NPRE, 32):
                        w1 = dma_w1(ft)
                        if NPRE <= ft < NPRE + 8:
                            dma_w2(0, ft - NPRE)
                            dma_w2(1, ft - NPRE)
                        hps = psH.tile([128, NT], F32, tag="h1", name="hps")
                        nmm = 4 * len(x_terms)
                        i = 0
                        for xh, wp in x_terms:
                            for dr in range(4):
                                nc.tensor.matmul(
                                    hps,
                                    w1[:, wp, 2 * dr : 2 * dr + 2, :],
                                    xh[:, 2 * dr : 2 * dr + 2, :],
                                    start=(i == 0), stop=(i == nmm - 1),
                                    perf_mode=DR,
                                )
                                i += 1
                        h1_quant(hps, ft)

                # --- phase 5: FFN2 (dh-major) + residual + LN2 + store ---
                # LN2 stats for the dh0 half are computed during the dh0
                # pass; after the dh1 STT only sg1 stats + apply + store
                # remain on the critical path.
                out_v = out[:, :].rearrange("(nt p) d -> p nt d", p=128)
                with tc.tile_pool(name="psY", bufs=3, space="PSUM") as psY:
                    h_terms = [(h1T_hi, 0), (h1T_hi, 1), (h1T_lo, 0)] \
                        if FFN_H_SPLIT else [(h1T_hi, 0), (h1T_hi, 1)]
                    y_ds = 1.0 / (
                        (H_PRESCALE if FFN_H_SPLIT else 1.0) * W2_PRESCALE
                    )
                    st2 = [
                        lnp.tile([128, 2, 6], F32, tag=f"ln2s{nt}", name=f"ln2s{nt}")
                        for nt in range(4)
                    ]
                    for dh in range(2):
                        for nt in range(4):
                            yps = psY.tile([128, 512], F32, tag="y", name="yps")
                            nmm = 16 * len(h_terms)
                            i = 0
                            for hh, wp in h_terms:
                                for dr in range(16):
                                    nc.tensor.matmul(
                                        yps,
                                        hh[:, 2 * dr : 2 * dr + 2, nt * 128 : nt * 128 + 128],
                                        W2h[:, dh, wp, 2 * dr : 2 * dr + 2, :],
                                        start=(i == 0), stop=(i == nmm - 1),
                                        perf_mode=DR,
                                    )
                                    i += 1
                            hsl = slice(dh * 512, dh * 512 + 512)
                            nc.vector.scalar_tensor_tensor(
                                out=x1_sb[:, nt, hsl],
                                in0=yps, scalar=y_ds,
                                in1=x1_sb[:, nt, hsl],
                                op0=OP.mult, op1=OP.add,
                            )
                            if b2bc is not None:
                                nc.vector.tensor_add(
                                    out=x1_sb[:, nt, hsl],
                                    in0=x1_sb[:, nt, hsl],
                                    in1=b2bc[:, hsl],
                                )
                            nc.vector.bn_stats(
                                out=st2[nt][:, dh, :], in_=x1_sb[:, nt, hsl]
                            )
                            if dh == 1:
                                mv = lnp.tile([128, 2], F32, tag="ln2mv", name="ln2mv")
                                nc.vector.bn_aggr(out=mv, in_=st2[nt])
                                nc.scalar.activation(
                                    out=mv[:, 1:2], in_=mv[:, 1:2], func=AF.Sqrt,
                                    bias=eps_sb, scale=1.0,
                                )
                                nc.vector.reciprocal(out=mv[:, 1:2], in_=mv[:, 1:2])
                                for hh2 in range(2):
                                    h2 = slice(hh2 * 512, hh2 * 512 + 512)
                                    nc.vector.tensor_scalar(
                                        out=x1_sb[:, nt, h2], in0=x1_sb[:, nt, h2],
                                        scalar1=mv[:, 0:1], scalar2=mv[:, 1:2],
                                        op0=OP.subtract, op1=OP.mult,
                                    )
                                    if g2bc is not None:
                                        nc.vector.tensor_mul(
                                            out=x1_sb[:, nt, h2],
                                            in0=x1_sb[:, nt, h2], in1=g2bc[:, h2],
                                        )
                                    if be2bc is not None:
                                        nc.vector.tensor_add(
                                            out=x1_sb[:, nt, h2],
                                            in0=x1_sb[:, nt, h2], in1=be2bc[:, h2],
                                        )
                                    nc.sync.dma_start(
                                        out=out_v[:, nt, h2], in_=x1_sb[:, nt, h2]
                                    )

    nc.finalize()
    return nc


def _hilo8(a):
    """Stack round-to-nearest fp8 hi and residual lo planes: [2, *a.shape]."""
    hi = np.asarray(a, np.float32).astype(F8NP)
    lo = (np.asarray(a, np.float32) - hi.astype(np.float32)).astype(F8NP)
    return np.ascontiguousarray(np.stack([hi, lo], axis=0))


def host_prep(inputs):
    """Build the 8 per-core input maps from the full problem inputs."""
    src = np.asarray(inputs["src"], np.float32)
    coords = np.asarray(inputs["coords"])
    Wq = np.asarray(inputs["Wq"], np.float32)
    Wk = np.asarray(inputs["Wk"], np.float32)
    Wv = np.asarray(inputs["Wv"], np.float32)
    Wo = np.asarray(inputs["Wo"], np.float32)
    W1 = np.asarray(inputs["W1"], np.float32)
    b1 = np.asarray(inputs["b1"], np.float32)
    W2 = np.asarray(inputs["W2"], np.float32)
    b2 = np.asarray(inputs["b2"], np.float32)
    g1 = np.asarray(inputs["g1"], np.float32)
    be1 = np.asarray(inputs["be1"], np.float32)
    g2 = np.asarray(inputs["g2"], np.float32)
    be2 = np.asarray(inputs["be2"], np.float32)

    def _blk8(wt, nblk, blk):
        # [dt, w, p, c*blk + j] from wt.T-like [c*128+p, dt*blk+j]
        x = (WP_PRESCALE * wt.T).reshape(8, 128, nblk, blk)
        x = x.transpose(2, 1, 0, 3).reshape(nblk, 128, 8 * blk)
        return np.ascontiguousarray(_hilo8(x).transpose(1, 0, 2, 3))

    # Projection weights as fp8 hi/lo planes; the per-head SCALE/slope_h for
    # q goes in as the PSUM->qaug copy descale on device.
    shared = {
        "WqS8": _blk8(Wq, 8, 128),
        "WkS8": _blk8(Wk, 8, 128),
        "WvS8": _blk8(Wv, 2, 512),
        "WoT8": _hilo8(WP_PRESCALE * Wo.T),
        # W1S[w, ft, p, dc*128+j] = hi/lo fp8 planes of W1.T[dc*128+p, ft*128+j]
        "W1S": _hilo8(
            (W1_PRESCALE * W1.T)
            .reshape(8, 128, 32, 128).transpose(2, 1, 0, 3).reshape(32, 128, D)
        ),
        # W2S[dh, w, dff, j] = hi/lo planes of W2.T[dff, dh*512+j]
        "W2S": np.ascontiguousarray(
            _hilo8(
                (W2_PRESCALE * W2.T).reshape(DFF, 2, 512).transpose(1, 0, 2)
            ).transpose(1, 0, 2, 3)
        ),
        "b1r": np.ascontiguousarray(b1.reshape(32, 128).T),
        "b2": b2.reshape(1, D),
        "g1": g1.reshape(1, D),
        "be1": be1.reshape(1, D),
        "g2": g2.reshape(1, D),
        "be2": be2.reshape(1, D),
    }

    in_maps = []
    for c in range(NCORES):
        b = c // 2
        half = c % 2
        rows = slice(half * NT, (half + 1) * NT)
        # key-axis permutation: own query rows first (Q proj reads the first
        # NT columns of srcT8); keys are a contraction axis everywhere, so
        # only kaug_x must be permuted consistently.
        perm = np.r_[half * NT : (half + 1) * NT, (1 - half) * NT : (2 - half) * NT]
        x = coords[b, :, 0].astype(np.float64)
        y = coords[b, :, 1].astype(np.float64)
        s = (x + y).astype(np.float32)
        thr = np.arange(1, GRID, dtype=np.float64)
        cx = (x[None, :] >= thr[:, None]).astype(np.float32)
        cy = (y[None, :] >= thr[:, None]).astype(np.float32)
        kaug = np.concatenate(
            [s.reshape(1, N), np.zeros((1, N), np.float32), cx, cy], axis=0
        ).astype(BF)
        qaug = np.empty((64, NT), np.float32)
        qaug[0, :] = 1.0
        qaug[1, :] = 0.0
        qaug[2:33, :] = -2.0 * cx[:, rows]
        qaug[33:64, :] = -2.0 * cy[:, rows]
        srcTb = np.ascontiguousarray(src[b].T[:, perm])
        m = dict(shared)
        m.update(
            {
                "srcT8": _hilo8(S_PRESCALE * srcTb),
                "src_rows": np.ascontiguousarray(src[b, rows, :]),
                "kaug_x": np.ascontiguousarray(kaug[:, perm]).reshape(64, 1, N),
                "qaug_x": qaug.astype(BF).reshape(64, 1, NT),
            }
        )
        in_maps.append(m)
    return in_maps


_NCS = {}
LAST_RUN_S = None


def get_nc(trivial_affine=True):
    if trivial_affine not in _NCS:
        _NCS[trivial_affine] = build_nc(trivial_affine)
    return _NCS[trivial_affine]


def _affine_trivial(inputs):
    return (
        np.all(np.asarray(inputs["g1"]) == 1.0)
        and np.all(np.asarray(inputs["g2"]) == 1.0)
        and not np.any(np.asarray(inputs["be1"]))
        and not np.any(np.asarray(inputs["be2"]))
        and not np.any(np.asarray(inputs["b2"]))
    )


def kernel(**inputs):
    global LAST_RUN_S
    from concourse.bass_utils import run_bass_kernel_spmd

    nc = get_nc(bool(_affine_trivial(inputs)))
    in_maps = host_prep(inputs)
    t0 = time.monotonic()
    res = run_bass_kernel_spmd(nc, in_maps, list(range(NCORES)))
    LAST_RUN_S = time.monotonic() - t0
    full = np.empty((B, N, D), np.float32)
    for c in range(NCORES):
        b = c // 2
        half = c % 2
        full[b, half * NT : (half + 1) * NT, :] = res.results[c]["out"]
    return full



# revision 73
# speedup vs baseline: 1.2204x; 1.0059x over previous
"""Fused transformer encoder layer (attention w/ 2D-ALiBi bias + FFN) on 8 trn2 cores.

Sharding: core c handles batch b = c//2, token half h = c%2 (512 query rows).
K/V are computed per-core for the full 1024-token sequence of its batch
(duplicated across the 2 cores sharing a batch); outputs are disjoint row
slices of the final tensor, so no collectives are needed.

Bias trick: the alibi_2d bias slope_h*(|xi-xj|+|yi-yj|) is folded into the
QK^T contraction. |xi-xj| = xi + xj - 2*a_i.a_j with a_i in {0,1}^31 the
threshold indicators of xi, so dist(i,j) = s_i + s_j - 2*c_i.c_j (c = 62-dim
indicator, s = x+y). The per-query term slope*s_i is constant along the
softmax axis and is dropped. Q/K are augmented with 64 extra contraction dims
(s_j / pad / c_j on the K side; 1 / 0 / -2*c_i on the Q side), making the
score contraction K = 64+64 = 128 exactly — full PE array, bias for free.

bf16 precision care: the aug rows are small integers / {0,-2} — exact in
bf16. The attention scale AND the per-head slope are folded out of the bf16
data: Q-projection weights carry scale/slope_h per head (so scores come out
as S/slope_h) and the exact fp32 slope_h is re-applied as the exp()
activation's scale immediate. exp needs no max-subtraction (|S| <= ~50 by
construction).

Scores are computed keys-on-partitions (S^T layout) so the exp() output is
already P^T for the AV matmul (no transpose). Softmax denominators come from
an appended ones-column in V; normalization is deferred past the (linear)
output projection boundary: each head's O^T rows are scaled by a broadcasted
1/den (built with a small fp32 selector matmul) before the head-summing
projection.
"""

import math
import sys
import time

for _p in ("/opt/trn_rl_repo",):
    if _p not in sys.path:
        sys.path.insert(0, _p)

import numpy as np
import ml_dtypes

import concourse.bass as bass
import concourse.tile as tile
from concourse import bacc, mybir
from concourse.masks import make_identity

F32 = mybir.dt.float32
F32R = mybir.dt.float32r
BF16 = mybir.dt.bfloat16
F8 = mybir.dt.float8e4
BF = ml_dtypes.bfloat16
F8NP = ml_dtypes.float8_e4m3
DR = mybir.MatmulPerfMode.DoubleRow

# fp8 FFN config: activations split into fp8 hi+lo planes (quantization error
# feedback), weights plain fp8. Splitting halves the DoubleRow speedup for
# that operand but removes its quantization error from the output.
FFN_X_SPLIT = True   # x1T (FFN1 input) hi/lo
FFN_H_SPLIT = True   # h1T (FFN2 input) hi/lo

# fp8e4m3 normals span [2^-6, 448]; W1/W2 entries (sigma ~ 1/32) and the lo
# planes would otherwise land in the subnormal range and lose most precision,
# so everything is pre-scaled up into the normal range and the product scale
# is divided back out at the PSUM->SBUF boundary (gelu scale / y descale).
W1_PRESCALE = 256.0
W2_PRESCALE = 256.0
X_PRESCALE = 16.0
H_PRESCALE = 32.0
S_PRESCALE = 8.0     # src (sigma 1) for the QKV projections
WP_PRESCALE = 256.0  # Wq/Wk/Wv/Wo (sigma 1/32)
OT_PRESCALE = 16.0   # normalized attention outputs (sigma ~1)

D = 1024          # d_model
H = 16            # heads
HD = 64           # head dim
DFF = 4096
B = 4
N = 1024          # sequence length
NT = 512          # tokens (query rows) per core
GRID = 32
EPS = 1e-5
NCORES = 8
SCALE = HD ** -0.5


def _alibi_slopes(n):
    def pow2(n_):
        start = 2.0 ** (-(2.0 ** -(math.log2(n_) - 3)))
        return [start * start ** i for i in range(n_)]
    if math.log2(n).is_integer():
        return np.array(pow2(n), dtype=np.float64)
    m = 2 ** math.floor(math.log2(n))
    s = pow2(m)
    s += [s[-1] * 0.5 ** (i + 1) for i in range(n - m)]
    return np.array(s, dtype=np.float64)


SLOPES = _alibi_slopes(H)


def build_nc(trivial_affine=False):
    """trivial_affine: g1/g2 all-ones and be1/be2/b2 all-zeros -> skip those ops."""
    nc = bacc.Bacc()

    # srcT8 columns are permuted per-core so the core's own 512 query rows
    # come FIRST (Q proj reads stf[:, :, :, 0:NT]); key order is a contraction
    # axis everywhere else, so the permutation is invisible provided kaug_x
    # and the V layout use the same order (host_prep keeps them consistent).
    srcT8 = nc.declare_dram_parameter("srcT8", [2, D, N], F8, isOutput=False)
    src_rows = nc.declare_dram_parameter("src_rows", [NT, D], F32, isOutput=False)
    # Wq/Wk swizzled per output-block dt (head pair): WqS8[dt, w, p, c*128+j]
    # = Wq.T[c*128+p, dt*128+j], so each dt block (both planes) is one
    # contiguous DMA and head pair dt can project as soon as it lands.
    WqS8 = nc.declare_dram_parameter("WqS8", [8, 2, 128, D], F8, isOutput=False)
    WkS8 = nc.declare_dram_parameter("WkS8", [8, 2, 128, D], F8, isOutput=False)
    # Wv swizzled by dh half: WvS8[dh, w, p, c*512+j] = Wv.T[c*128+p, dh*512+j]
    WvS8 = nc.declare_dram_parameter("WvS8", [2, 2, 128, 8 * 512], F8, isOutput=False)
    WoT8 = nc.declare_dram_parameter("WoT8", [2, D, D], F8, isOutput=False)
    # W1 pre-swizzled on host: W1S[w, ft, p, dc*128+j] = W1.T[dc*128+p, ft*128+j]
    # (w = fp8 hi/lo plane) so each FFN1 weight block is contiguous per plane.
    W1S = nc.declare_dram_parameter("W1S", [2, 32, 128, D], F8, isOutput=False)
    # W2 split by output half dh so dh0 can stream during FFN1 and dh1
    # during the FFN2 dh0 pass: W2S[dh, w, dff, j] = W2.T[dff, dh*512+j]
    W2S = nc.declare_dram_parameter("W2S", [2, 2, DFF, 512], F8, isOutput=False)
    # aug blocks are head-independent; the singleton dim enables stride-0
    # broadcast DMAs into all head slots.
    kaug_x = nc.declare_dram_parameter("kaug_x", [64, 1, N], BF16, isOutput=False)
    qaug_x = nc.declare_dram_parameter("qaug_x", [64, 1, NT], BF16, isOutput=False)
    b1r = nc.declare_dram_parameter("b1r", [128, 32], F32, isOutput=False)
    b2 = nc.declare_dram_parameter("b2", [1, D], F32, isOutput=False)
    g1 = nc.declare_dram_parameter("g1", [1, D], F32, isOutput=False)
    be1 = nc.declare_dram_parameter("be1", [1, D], F32, isOutput=False)
    g2 = nc.declare_dram_parameter("g2", [1, D], F32, isOutput=False)
    be2 = nc.declare_dram_parameter("be2", [1, D], F32, isOutput=False)
    out = nc.declare_dram_parameter("out", [NT, D], F32, isOutput=True)

    AF = mybir.ActivationFunctionType
    OP = mybir.AluOpType

    with tile.TileContext(nc) as tc:
        with (
            tc.tile_pool(name="misc", bufs=1) as misc,
            tc.tile_pool(name="lnp", bufs=4) as lnp,
            tc.tile_pool(name="pre", bufs=1) as pre,
        ):
            eps_sb = misc.tile([128, 1], F32, tag="eps")
            nc.vector.memset(eps_sb, EPS)
            ident = misc.tile([128, 128], F32, tag="ident")
            make_identity(nc, ident)
            ident_bf = misc.tile([128, 128], BF16, tag="identbf")
            make_identity(nc, ident_bf)
            ident_f8 = misc.tile([128, 128], F8, tag="identf8")
            make_identity(nc, ident_f8)
            # OT8[p, c, q]: head 2c in partitions 0:64, head 2c+1 in 64:128;
            # fp8 hi/lo planes (scaled by OT_PRESCALE) for the 3-term out-proj
            OT8_hi = misc.tile([128, 8, NT], F8, tag="ot8h")

            def ln_apply(x_ap, gbc, bbc):
                stats = lnp.tile([128, 2, 6], F32, tag="lnstats", name="lnstats")
                for sg in range(2):
                    nc.vector.bn_stats(
                        out=stats[:, sg, :], in_=x_ap[:, sg * 512 : sg * 512 + 512]
                    )
                mv = lnp.tile([128, 2], F32, tag="lnmv", name="lnmv")
                nc.vector.bn_aggr(out=mv, in_=stats)
                nc.scalar.activation(
                    out=mv[:, 1:2], in_=mv[:, 1:2], func=AF.Sqrt,
                    bias=eps_sb, scale=1.0,
                )
                nc.vector.reciprocal(out=mv[:, 1:2], in_=mv[:, 1:2])
                nc.vector.tensor_scalar(
                    out=x_ap, in0=x_ap,
                    scalar1=mv[:, 0:1], scalar2=mv[:, 1:2],
                    op0=OP.subtract, op1=OP.mult,
                )
                if gbc is not None:
                    nc.vector.tensor_mul(out=x_ap, in0=x_ap, in1=gbc)
                if bbc is not None:
                    nc.vector.tensor_add(out=x_ap, in0=x_ap, in1=bbc)

            # ============ merged projections + attention ============
            # Single software pipeline: per step s, project K/Q for head pair
            # s, run scores+exp for pair s-1, AV+normalize for pair s-2, and
            # transpose/quantize pair s-3. V projections burst at steps 1/4.
            # AV runs in natural orientation (A = P^T chunk, B = V columns
            # incl. a ones column -> out [128 queries, 64 vdims + den]), so
            # softmax denominators are per-PARTITION and normalization is a
            # plain tensor_scalar; O^T for the out-proj comes from cheap bf16
            # PE transposes of head pairs.
            # wqk opens before att so its released zone sits at the stack
            # bottom: the FFN W1 ring reuses it, anchored on the early
            # K(7)/Q(7) last-uses instead of the late attention tail.
            with (
                tc.tile_pool(name="wqk", bufs=1) as wqk,
                tc.tile_pool(name="att", bufs=1) as att,
            ):
                kaug = att.tile([128, H, N], BF16, tag="kaug")
                qaug = att.tile([128, H, NT], BF16, tag="qaug")
                v_sb = att.tile([128, 8, H * 65], BF16, tag="vsb")
                v4 = v_sb.rearrange("p m (h w) -> p m h w", w=65)
                nc.vector.memset(v4[:, :, :, 64], 1.0)

                P_DS = 1.0 / (S_PRESCALE * WP_PRESCALE)
                # (activation plane, weight plane) product terms; lo*lo skipped
                TERMS = ((0, 0), (1, 0), (0, 1))

                def mm3t(ps, w8, x8, wslice, xslice):
                    i = 0
                    for a, b in TERMS:
                        for dr in range(4):
                            pl = slice(2 * dr, 2 * dr + 2)
                            nc.tensor.matmul(
                                ps,
                                w8[:, b, pl, wslice],
                                x8[:, a, pl, xslice],
                                start=(i == 0), stop=(i == 11),
                                perf_mode=DR,
                            )
                            i += 1

                with tc.tile_pool(name="ph1", bufs=1) as ph1:
                    # DMA emission order tracks first-use order; weights are
                    # host-swizzled so each head pair's block is one
                    # contiguous transfer.
                    wqf = wqk.tile([128, 8, 2, 8, 128], F8, tag="wqf")
                    stf = ph1.tile([128, 2, 8, N], F8, tag="stf")
                    st_vw = srcT8[:, :, :].rearrange("w (c p) n -> p w c n", p=128)
                    wkf = wqk.tile([128, 8, 2, 8, 128], F8, tag="wkf")
                    wvf = ph1.tile([128, 2, 2, 8, 512], F8, tag="wvf")

                    def dma_wdt(dst, src, dt):
                        nc.sync.dma_start(
                            out=dst[:, dt, :, :, :],
                            in_=src[dt, :, :, :].rearrange(
                                "w p (c n) -> p w c n", c=8
                            ),
                        )

                    def dma_aug(h0, h1):
                        # stride-0 broadcast of the shared aug block into
                        # head slots [h0, h1)
                        nc.sync.dma_start(
                            out=kaug[64:128, h0:h1, :],
                            in_=kaug_x[:, :, :].to_broadcast([64, h1 - h0, N]),
                        )
                        nc.sync.dma_start(
                            out=qaug[64:128, h0:h1, :],
                            in_=qaug_x[:, :, :].to_broadcast([64, h1 - h0, NT]),
                        )

                    # need-ordered: Q(0) first (own src columns + wq block 0),
                    # then K(0), aug for early heads, V(dh0), and the rest
                    # staggered against consumption.
                    nc.sync.dma_start(out=stf[:, 0, :, 0:NT], in_=st_vw[:, 0, :, 0:NT])
                    dma_wdt(wqf, WqS8, 0)
                    nc.sync.dma_start(out=stf[:, 1, :, 0:NT], in_=st_vw[:, 1, :, 0:NT])
                    nc.sync.dma_start(out=stf[:, 0, :, NT:N], in_=st_vw[:, 0, :, NT:N])
                    dma_wdt(wkf, WkS8, 0)
                    nc.sync.dma_start(out=stf[:, 1, :, NT:N], in_=st_vw[:, 1, :, NT:N])
                    dma_aug(0, 2)
                    dma_wdt(wqf, WqS8, 1)
                    dma_wdt(wkf, WkS8, 1)
                    for w in range(2):
                        nc.sync.dma_start(
                            out=wvf[:, 0, w, :, :],
                            in_=WvS8[0, w, :, :].rearrange("p (c n) -> p c n", c=8),
                        )
                    dma_aug(2, 4)
                    dma_wdt(wqf, WqS8, 2)
                    dma_wdt(wkf, WkS8, 2)
                    dma_aug(4, 6)
                    dma_wdt(wqf, WqS8, 3)
                    dma_wdt(wkf, WkS8, 3)
                    dma_aug(6, 8)
                    dma_wdt(wqf, WqS8, 4)
                    dma_wdt(wkf, WkS8, 4)
                    dma_aug(8, 12)
                    for w in range(2):
                        nc.sync.dma_start(
                            out=wvf[:, 1, w, :, :],
                            in_=WvS8[1, w, :, :].rearrange("p (c n) -> p c n", c=8),
                        )
                    dma_aug(12, 16)
                    for dt in range(5, 8):
                        dma_wdt(wqf, WqS8, dt)
                        dma_wdt(wkf, WkS8, dt)

                    # phase 3-5 prefetches ride the queue tail; they land
                    # long before the out-proj needs them.
                    if trivial_affine:
                        g1bc = be1bc = b2bc = g2bc = be2bc = None
                    else:
                        g1bc = pre.tile([128, D], F32, tag="g1bc")
                        be1bc = pre.tile([128, D], F32, tag="be1bc")
                        b2bc = pre.tile([128, D], F32, tag="b2bc")
                        g2bc = pre.tile([128, D], F32, tag="g2bc")
                        be2bc = pre.tile([128, D], F32, tag="be2bc")
                        for t_, src_ in (
                            (g1bc, g1), (be1bc, be1),
                            (b2bc, b2), (g2bc, g2), (be2bc, be2),
                        ):
                            nc.sync.dma_start(
                                out=t_, in_=src_[:, :].to_broadcast([128, D])
                            )
                    srar = pre.tile([128, 4, D], F32, tag="srcrows")
                    nc.sync.dma_start(
                        out=srar,
                        in_=src_rows[:, :].rearrange("(nt p) d -> p nt d", p=128),
                    )
                    wof = pre.tile([128, 2, 8, D], F8, tag="wof")
                    wo_vw = WoT8[:, :, :].rearrange("w (c p) n -> p w c n", p=128)
                    for w in range(2):
                        nc.sync.dma_start(out=wof[:, w, :, :], in_=wo_vw[:, w, :, :])
                    b1_sb = pre.tile([128, 32], F32, tag="b1")
                    nc.sync.dma_start(out=b1_sb, in_=b1r[:, :])

                    with (
                        tc.tile_pool(name="ptp", bufs=2) as ptp,
                        tc.tile_pool(name="obp", bufs=3) as obp,
                        tc.tile_pool(name="rcp", bufs=2) as rcp,
                        tc.tile_pool(name="psS", bufs=2, space="PSUM") as psS,
                        tc.tile_pool(name="psP", bufs=2, space="PSUM") as psP,
                        tc.tile_pool(name="psO", bufs=1, space="PSUM") as psO,
                        tc.tile_pool(name="psT", bufs=1, space="PSUM") as psT,
                    ):
                        pts = {}
                        obfs = {}

                        # p-state warmup: the PE needs ~3us of continuous
                        # execution to reach 2.4 GHz; burn the DMA cold-start
                        # on dependency-free dummy matmuls so the first real
                        # projections run at full clock.
                        wu = psP.tile([128, 512], F32, tag="pj", name="warm")
                        for _ in range(8):
                            nc.tensor.matmul(
                                wu[:, 0:128], ident, ident, start=True, stop=True
                            )

                        def mm3p(ps, w8, dt, xslice):
                            i = 0
                            for a, b in TERMS:
                                for dr in range(4):
                                    pl = slice(2 * dr, 2 * dr + 2)
                                    nc.tensor.matmul(
                                        ps,
                                        w8[:, dt, b, pl, :],
                                        stf[:, a, pl, xslice],
                                        start=(i == 0), stop=(i == 11),
                                        perf_mode=DR,
                                    )
                                    i += 1

                        def emit_K(dt, mh):
                            kps = psP.tile([128, 512], F32, tag="pj", name="kps")
                            mm3p(kps, wkf, dt, slice(mh * 512, mh * 512 + 512))
                            ksl = slice(mh * 512, mh * 512 + 512)
                            nc.vector.tensor_scalar_mul(
                                out=kaug[0:64, 2 * dt, ksl],
                                in0=kps[0:64, :], scalar1=P_DS,
                            )
                            nc.vector.tensor_scalar_mul(
                                out=kaug[0:64, 2 * dt + 1, ksl],
                                in0=kps[64:128, :], scalar1=P_DS,
                            )

                        def emit_Q(dt):
                            # own query rows are the FIRST NT columns of stf
                            qps = psP.tile([128, NT], F32, tag="pj", name="qps")
                            mm3p(qps, wqf, dt, slice(0, NT))
                            nc.vector.tensor_scalar_mul(
                                out=qaug[0:64, 2 * dt, :], in0=qps[0:64, :],
                                scalar1=float(SCALE / SLOPES[2 * dt] * P_DS),
                            )
                            nc.vector.tensor_scalar_mul(
                                out=qaug[0:64, 2 * dt + 1, :], in0=qps[64:128, :],
                                scalar1=float(SCALE / SLOPES[2 * dt + 1] * P_DS),
                            )

                        def emit_V_pair(p):
                            # V projection for head pair p only (just-in-time
                            # for AV(2p) one step later; spreads V across the
                            # pipeline and fills the exp-drain tail)
                            dh, hc = p // 4, (p % 4) * 128
                            for mg in range(2):
                                vt = psP.tile(
                                    [128, 4, 128], F32, tag="pj", name="vps"
                                )
                                for lm in range(4):
                                    mt = mg * 4 + lm
                                    i = 0
                                    for a, b in TERMS:
                                        for dr in range(4):
                                            pl = slice(2 * dr, 2 * dr + 2)
                                            nc.tensor.matmul(
                                                vt[:, lm, :],
                                                stf[:, a, pl, mt * 128 : mt * 128 + 128],
                                                wvf[:, dh, b, pl, hc : hc + 128],
                                                start=(i == 0), stop=(i == 11),
                                                perf_mode=DR,
                                            )
                                            i += 1
                                nc.vector.tensor_scalar_mul(
                                    out=v4[:, mg * 4 : mg * 4 + 4, 2 * p : 2 * p + 2, 0:64],
                                    in0=vt.rearrange("p m (h w) -> p m h w", w=64),
                                    scalar1=P_DS,
                                )

                        def emit_S(h, cs):
                            pt = pts[h]
                            for c in cs:
                                stc = psS.tile([128, 2, NT], F32, tag="st", name="stc")
                                for j in range(2):
                                    mt = 2 * c + j
                                    nc.tensor.matmul(
                                        stc[:, j, :],
                                        kaug[:, h, mt * 128 : mt * 128 + 128],
                                        qaug[:, h, :],
                                        start=True, stop=True,
                                    )
                                nc.scalar.activation(
                                    out=pt[:, 2 * c : 2 * c + 2, :], in_=stc,
                                    func=AF.Exp, scale=float(SLOPES[h]),
                                )

                        def emit_AV(h):
                            # natural orientation: out [128 queries, 64 vdims
                            # + den]; col 64 accumulates the softmax denom via
                            # the ones column in V.
                            cc = h // 2
                            if h % 2 == 0:
                                obfs[cc] = obp.tile(
                                    [128, 4, 128], BF16, tag="ob", name="obf"
                                )
                            po = psO.tile([128, 4, 65], F32, tag="po", name="po")
                            for qc in range(4):
                                for mt in range(8):
                                    nc.tensor.matmul(
                                        po[:, qc, :],
                                        pts[h][:, mt, qc * 128 : qc * 128 + 128],
                                        v4[:, mt, h, 0:65],
                                        start=(mt == 0), stop=(mt == 7),
                                    )
                            rc = rcp.tile([128, 4], F32, tag="rc", name="rc")
                            nc.vector.reciprocal(out=rc, in_=po[:, :, 64])
                            nc.vector.tensor_scalar_mul(
                                out=rc, in0=rc, scalar1=OT_PRESCALE
                            )
                            # normalize promptly on DVE (psO has one buffer;
                            # the next AV waits on these reads)
                            hb = (h % 2) * 64
                            for qc in range(4):
                                nc.vector.tensor_scalar_mul(
                                    out=obfs[cc][:, qc, hb : hb + 64],
                                    in0=po[:, qc, 0:64],
                                    scalar1=rc[:, qc : qc + 1],
                                )
                            pts.pop(h)

                        def emit_pair(cc):
                            ob = obfs.pop(cc)
                            tp = psT.tile([128, 4, 128], BF16, tag="tp", name="tp")
                            for qc in range(4):
                                nc.tensor.transpose(
                                    tp[:, qc, :], ob[:, qc, :], ident_bf
                                )
                            tpf = tp.rearrange("p a b -> p (a b)")
                            # gpsimd cannot touch PSUM on hw; Act does the
                            # copy. Single fp8 plane for O (the out-proj runs
                            # 2-term: O_hi x Wo_hi + O_hi x Wo_lo).
                            nc.scalar.activation(
                                out=OT8_hi[:, cc, :], in_=tpf, func=AF.Copy
                            )

                        def step(s):
                            h0, h1 = 2 * (s - 1), 2 * (s - 1) + 1
                            av0, av1 = 2 * (s - 2), 2 * (s - 2) + 1
                            if 2 <= s <= 9:
                                emit_AV(av0)
                            if s < 8:
                                emit_Q(s)
                            if 1 <= s <= 8:
                                pts[h0] = ptp.tile(
                                    [128, 8, NT], BF16, tag="pt", name="pt"
                                )
                                emit_S(h0, (0, 1))
                            if 2 <= s <= 9:
                                emit_AV(av1)
                            if s < 8:
                                emit_K(s, 0)
                            if 1 <= s <= 8:
                                emit_S(h0, (2, 3))
                            if s < 8:
                                emit_K(s, 1)
                            if 1 <= s <= 8:
                                pts[h1] = ptp.tile(
                                    [128, 8, NT], BF16, tag="pt", name="pt"
                                )
                                emit_S(h1, (0, 1))
                            if 1 <= s <= 8:
                                emit_V_pair(s - 1)
                            if 1 <= s <= 8:
                                emit_S(h1, (2, 3))
                            if s >= 3:
                                emit_pair(s - 3)

                        for s in range(11):
                            step(s)

            # ============ post-attention scope ============
            with (
                tc.tile_pool(name="w1p", bufs=20) as w1p,
                tc.tile_pool(name="ffn", bufs=1) as ffn,
            ):
                # w1p opens FIRST so the W1 ring lands on the earliest-freed
                # attention SBUF and its stream starts during the tail.
                # W2 halves [p, dh, w, c, n]: dh0 streams during FFN1, dh1
                # during the FFN2 dh0 pass (keeps FFN1's W1 stream PE-bound)
                W2h = ffn.tile([128, 2, 2, 32, 512], F8, tag="w2")
                w2_v = [
                    W2S[dh, :, :, :].rearrange("w (c p) n -> p w c n", p=128)
                    for dh in range(2)
                ]

                def dma_w2(dh, cg):
                    # one [128, w, 4, 512] chunk (0.25 MB) per call
                    for w in range(2):
                        nc.sync.dma_start(
                            out=W2h[:, dh, w, 4 * cg : 4 * cg + 4, :],
                            in_=w2_v[dh][:, w, 4 * cg : 4 * cg + 4, :],
                        )
                x1_sb = ffn.tile([128, 4, D], F32, tag="x1")
                x1T_hi = ffn.tile([128, 8, NT], F8, tag="x1Th")
                x1T_lo = (
                    ffn.tile([128, 8, NT], F8, tag="x1Tl", name="x1T_lo") if FFN_X_SPLIT else None
                )

                # --- phase 3: out-proj, LN1, transpose ---
                with (
                    tc.tile_pool(name="psS2", bufs=4, space="PSUM") as psS2,
                    tc.tile_pool(name="psT", bufs=2, space="PSUM") as psT,
                    tc.tile_pool(name="xqp", bufs=2) as xqp,
                ):
                    def transposes(nt, xq):
                        # xq holds bf16 16*LN(x1)[nt]; hw forbids plain fp8
                        # transposes, so transpose bf16 and split hi/lo after.
                        nsl = slice(nt * 128, nt * 128 + 128)
                        tp = psT.tile([128, 8, 128], BF16, tag="tp", name="tp")
                        for c in range(8):
                            nc.tensor.transpose(
                                tp[:, c, :],
                                xq[:, c * 128 : c * 128 + 128],
                                ident_bf,
                            )
                        nc.scalar.activation(
                            out=x1T_hi[:, :, nsl], in_=tp, func=AF.Copy
                        )
                        if FFN_X_SPLIT:
                            nc.vector.tensor_sub(
                                out=x1T_lo[:, :, nsl],
                                in0=tp, in1=x1T_hi[:, :, nsl],
                            )

                    def quantize_half(nt, hsl, xq):
                        nc.scalar.activation(
                            out=xq[:, hsl], in_=x1_sb[:, nt, hsl],
                            func=AF.Copy, scale=X_PRESCALE,
                        )

                    # 2-term out-proj: O is a single fp8 plane (error ~fp8
                    # quant of O, emulated ~+0.007 rel; tolerance 0.02)
                    OP_TERMS = ((0, 0), (0, 1))
                    OT8 = (OT8_hi,)
                    O_DS = 1.0 / (OT_PRESCALE * WP_PRESCALE)
                    st1 = [
                        lnp.tile([128, 2, 6], F32, tag=f"ln1s{nt}", name=f"ln1s{nt}")
                        for nt in range(4)
                    ]
                    xqs = {}
                    for nt in range(4):
                        for dh in range(2):
                            s2 = psS2.tile([128, 512], F32, tag="s2", name="s2")
                            # dr-major: only the last 3 matmuls (head pairs
                            # 6-7) wait on the final attention quantize
                            i = 0
                            for dr in range(4):
                                pl = slice(2 * dr, 2 * dr + 2)
                                for a, b in OP_TERMS:
                                    nc.tensor.matmul(
                                        s2,
                                        OT8[a][:, pl, nt * 128 : nt * 128 + 128],
                                        wof[:, b, pl, dh * 512 : dh * 512 + 512],
                                        start=(i == 0), stop=(i == 7),
                                        perf_mode=DR,
                                    )
                                    i += 1
                            hsl = slice(dh * 512, dh * 512 + 512)
                            nc.vector.scalar_tensor_tensor(
                                out=x1_sb[:, nt, hsl],
                                in0=s2, scalar=O_DS,
                                in1=srar[:, nt, hsl],
                                op0=OP.mult, op1=OP.add,
                            )
                            # half-stats immediately: shortens the LN chain
                            nc.vector.bn_stats(
                                out=st1[nt][:, dh, :], in_=x1_sb[:, nt, hsl]
                            )
                        mv = lnp.tile([128, 2], F32, tag="lnmv", name="lnmv")
                        nc.vector.bn_aggr(out=mv, in_=st1[nt])
                        nc.scalar.activation(
                            out=mv[:, 1:2], in_=mv[:, 1:2], func=AF.Sqrt,
                            bias=eps_sb, scale=1.0,
                        )
                        nc.vector.reciprocal(out=mv[:, 1:2], in_=mv[:, 1:2])
                        # apply + quantize per half so each half's fp8 planes
                        # chain independently (dh0 on DVE, dh1 on Pool)
                        xq = xqp.tile([128, D], BF16, tag="xq", name="xq")
                        xqs[nt] = xq
                        for hh2, eng in ((0, nc.vector), (1, nc.gpsimd)):
                            h2 = slice(hh2 * 512, hh2 * 512 + 512)
                            eng.tensor_scalar(
                                out=x1_sb[:, nt, h2], in0=x1_sb[:, nt, h2],
                                scalar1=mv[:, 0:1], scalar2=mv[:, 1:2],
                                op0=OP.subtract, op1=OP.mult,
                            )
                            if g1bc is not None:
                                eng.tensor_mul(
                                    out=x1_sb[:, nt, h2],
                                    in0=x1_sb[:, nt, h2], in1=g1bc[:, h2],
                                )
                            if be1bc is not None:
                                eng.tensor_add(
                                    out=x1_sb[:, nt, h2],
                                    in0=x1_sb[:, nt, h2], in1=be1bc[:, h2],
                                )
                            quantize_half(nt, h2, xq)
                        # emitted AFTER the LN/quantize block: the next nt's
                        # gating chain gets queue priority over these copies
                        if nt >= 1:
                            transposes(nt - 1, xqs.pop(nt - 1))
                    transposes(3, xqs.pop(3))

                # --- phase 4: FFN1 (fp8 DoubleRow matmuls, gelu into fp8 h1T) ---
                h1T_hi = ffn.tile([128, 32, NT], F8, tag="h1Th")
                h1T_lo = (
                    ffn.tile([128, 32, NT], F8, tag="h1Tl", name="h1T_lo") if FFN_H_SPLIT else None
                )
                NPRE = 0
                with (
                    tc.tile_pool(name="h1gp", bufs=3) as h1gp,
                    tc.tile_pool(name="psH", bufs=3, space="PSUM") as psH,
                    tc.tile_pool(name="psHp", bufs=max(NPRE, 1), space="PSUM") as psHp,
                ):
                    # term list: (x plane, w plane); lo*lo is negligible
                    # lo-dependent terms last: the first 8 matmuls of each
                    # group only need the hi plane
                    x_terms = [(x1T_hi, 0), (x1T_hi, 1), (x1T_lo, 0)] \
                        if FFN_X_SPLIT else [(x1T_hi, 0), (x1T_hi, 1)]

                    def dma_w1(ft):
                        w1 = w1p.tile([128, 2, 8, 128], F8, tag="w1col", name="w1")
                        # ring-buffer DMAs block SP on their WAR semaphore;
                        # the 20-deep ring keeps the WAR anchor far behind
                        # consumption so SP's queue never stalls.
                        nc.sync.dma_start(
                            out=w1.rearrange("p w c n -> p w (c n)"),
                            in_=W1S[:, ft, :, :].rearrange("w p n -> p w n"),
                        )
                        return w1

                    def mm_ft_nt(hps, w1, ft, nt):
                        nsl = slice(nt * 128, nt * 128 + 128)
                        i = 0
                        for xh, wp in x_terms:
                            for dr in range(4):
                                nc.tensor.matmul(
                                    hps[:, nsl],
                                    w1[:, wp, 2 * dr : 2 * dr + 2, :],
                                    xh[:, 2 * dr : 2 * dr + 2, nsl],
                                    start=(i == 0), stop=(i == 11),
                                    perf_mode=DR,
                                )
                                i += 1

                    def h1_quant(hps, ft):
                        # PSUM holds (X*W1 prescales)*h; descale via gelu's
                        # input scale, rescale the fp8 planes by H_PRESCALE.
                        in_ds = 1.0 / (X_PRESCALE * W1_PRESCALE)
                        if FFN_H_SPLIT:
                            h1g = h1gp.tile([128, NT], BF16, tag="h1g", name="h1g")
                            nc.scalar.activation(
                                out=h1g, in_=hps, func=AF.Gelu,
                                bias=b1_sb[:, ft : ft + 1], scale=in_ds,
                            )
                            nc.vector.tensor_scalar_mul(
                                out=h1T_hi[:, ft, :], in0=h1g, scalar1=H_PRESCALE
                            )
                            nc.vector.scalar_tensor_tensor(
                                out=h1T_lo[:, ft, :], in0=h1g, scalar=H_PRESCALE,
                                in1=h1T_hi[:, ft, :], op0=OP.mult, op1=OP.subtract,
                            )
                        else:
                            nc.scalar.activation(
                                out=h1T_hi[:, ft, :], in_=hps, func=AF.Gelu,
                                bias=b1_sb[:, ft : ft + 1], scale=in_ds,
                            )

                    # The first NPRE fts run nt-sliced and nt-major: their
                    # (ft, nt) groups start as each x1T token tile lands,
                    # filling the PE during the phase-3 LN/quantize drain.
                    pre_w1 = [dma_w1(ft) for ft in range(NPRE)]
                    pre_h = [
                        psHp.tile([128, NT], F32, tag="h1p", name="hpsp")
                        for _ in range(NPRE)
                    ]
                    for nt in range(3):
                        for ft in range(NPRE):
                            mm_ft_nt(pre_h[ft], pre_w1[ft], ft, nt)
                    for ft in range(NPRE):
                        mm_ft_nt(pre_h[ft], pre_w1[ft], ft, 3)
                        h1_quant(pre_h[ft], ft)
                    for ft in range(NPRE, 32):
                        w1 = dma_w1(ft)
                        if NPRE <= ft < NPRE + 8:
                            dma_w2(0, ft - NPRE)
                            dma_w2(1, ft - NPRE)
                        hps = psH.tile([128, NT], F32, tag="h1", name="hps")
                        nmm = 4 * len(x_terms)
                        i = 0
                        for xh, wp in x_terms:
                            for dr in range(4):
                                nc.tensor.matmul(
                                    hps,
                                    w1[:, wp, 2 * dr : 2 * dr + 2, :],
                                    xh[:, 2 * dr : 2 * dr + 2, :],
                                    start=(i == 0), stop=(i == nmm - 1),
                                    perf_mode=DR,
                                )
                                i += 1
                        h1_quant(hps, ft)

                # --- phase 5: FFN2 (dh-major) + residual + LN2 + store ---
                # LN2 stats for the dh0 half are computed during the dh0
                # pass; after the dh1 STT only sg1 stats + apply + store
                # remain on the critical path.
                out_v = out[:, :].rearrange("(nt p) d -> p nt d", p=128)
                with tc.tile_pool(name="psY", bufs=3, space="PSUM") as psY:
                    h_terms = [(h1T_hi, 0), (h1T_hi, 1), (h1T_lo, 0)] \
                        if FFN_H_SPLIT else [(h1T_hi, 0), (h1T_hi, 1)]
                    y_ds = 1.0 / (
                        (H_PRESCALE if FFN_H_SPLIT else 1.0) * W2_PRESCALE
                    )
                    st2 = [
                        lnp.tile([128, 2, 6], F32, tag=f"ln2s{nt}", name=f"ln2s{nt}")
                        for nt in range(4)
                    ]
                    for dh in range(2):
                        for nt in range(4):
                            yps = psY.tile([128, 512], F32, tag="y", name="yps")
                            nmm = 16 * len(h_terms)
                            i = 0
                            for hh, wp in h_terms:
                                for dr in range(16):
                                    nc.tensor.matmul(
                                        yps,
                                        hh[:, 2 * dr : 2 * dr + 2, nt * 128 : nt * 128 + 128],
                                        W2h[:, dh, wp, 2 * dr : 2 * dr + 2, :],
                                        start=(i == 0), stop=(i == nmm - 1),
                                        perf_mode=DR,
                                    )
                                    i += 1
                            hsl = slice(dh * 512, dh * 512 + 512)
                            nc.vector.scalar_tensor_tensor(
                                out=x1_sb[:, nt, hsl],
                                in0=yps, scalar=y_ds,
                                in1=x1_sb[:, nt, hsl],
                                op0=OP.mult, op1=OP.add,
                            )
                            if b2bc is not None:
                                nc.vector.tensor_add(
                                    out=x1_sb[:, nt, hsl],
                                    in0=x1_sb[:, nt, hsl],
                                    in1=b2bc[:, hsl],
                                )
                            nc.vector.bn_stats(
                                out=st2[nt][:, dh, :], in_=x1_sb[:, nt, hsl]
                            )
                            if dh == 1:
                                mv = lnp.tile([128, 2], F32, tag="ln2mv", name="ln2mv")
                                nc.vector.bn_aggr(out=mv, in_=st2[nt])
                                nc.scalar.activation(
                                    out=mv[:, 1:2], in_=mv[:, 1:2], func=AF.Sqrt,
                                    bias=eps_sb, scale=1.0,
                                )
                                nc.vector.reciprocal(out=mv[:, 1:2], in_=mv[:, 1:2])
                                for hh2 in range(2):
                                    h2 = slice(hh2 * 512, hh2 * 512 + 512)
                                    nc.vector.tensor_scalar(
                                        out=x1_sb[:, nt, h2], in0=x1_sb[:, nt, h2],
                                        scalar1=mv[:, 0:1], scalar2=mv[:, 1:2],
                                        op0=OP.subtract, op1=OP.mult,
                                    )
                                    if g2bc is not None:
                                        nc.vector.tensor_mul(
                                            out=x1_sb[:, nt, h2],
                                            in0=x1_sb[:, nt, h2], in1=g2bc[:, h2],
                                        )
                                    if be2bc is not None:
                                        nc.vector.tensor_add(
                                            out=x1_sb[:, nt, h2],
                                            in0=x1_sb[:, nt, h2], in1=be2bc[:, h2],
                                        )
                                    nc.sync.dma_start(
                                        out=out_v[:, nt, h2], in_=x1_sb[:, nt, h2]
                                    )

    nc.finalize()
    return nc


def _hilo8(a):
    """Stack round-to-nearest fp8 hi and residual lo planes: [2, *a.shape]."""
    hi = np.asarray(a, np.float32).astype(F8NP)
    lo = (np.asarray(a, np.float32) - hi.astype(np.float32)).astype(F8NP)
    return np.ascontiguousarray(np.stack([hi, lo], axis=0))


def host_prep(inputs):
    """Build the 8 per-core input maps from the full problem inputs."""
    src = np.asarray(inputs["src"], np.float32)
    coords = np.asarray(inputs["coords"])
    Wq = np.asarray(inputs["Wq"], np.float32)
    Wk = np.asarray(inputs["Wk"], np.float32)
    Wv = np.asarray(inputs["Wv"], np.float32)
    Wo = np.asarray(inputs["Wo"], np.float32)
    W1 = np.asarray(inputs["W1"], np.float32)
    b1 = np.asarray(inputs["b1"], np.float32)
    W2 = np.asarray(inputs["W2"], np.float32)
    b2 = np.asarray(inputs["b2"], np.float32)
    g1 = np.asarray(inputs["g1"], np.float32)
    be1 = np.asarray(inputs["be1"], np.float32)
    g2 = np.asarray(inputs["g2"], np.float32)
    be2 = np.asarray(inputs["be2"], np.float32)

    def _blk8(wt, nblk, blk):
        # [dt, w, p, c*blk + j] from wt.T-like [c*128+p, dt*blk+j]
        x = (WP_PRESCALE * wt.T).reshape(8, 128, nblk, blk)
        x = x.transpose(2, 1, 0, 3).reshape(nblk, 128, 8 * blk)
        return np.ascontiguousarray(_hilo8(x).transpose(1, 0, 2, 3))

    # Projection weights as fp8 hi/lo planes; the per-head SCALE/slope_h for
    # q goes in as the PSUM->qaug copy descale on device.
    shared = {
        "WqS8": _blk8(Wq, 8, 128),
        "WkS8": _blk8(Wk, 8, 128),
        "WvS8": _blk8(Wv, 2, 512),
        "WoT8": _hilo8(WP_PRESCALE * Wo.T),
        # W1S[w, ft, p, dc*128+j] = hi/lo fp8 planes of W1.T[dc*128+p, ft*128+j]
        "W1S": _hilo8(
            (W1_PRESCALE * W1.T)
            .reshape(8, 128, 32, 128).transpose(2, 1, 0, 3).reshape(32, 128, D)
        ),
        # W2S[dh, w, dff, j] = hi/lo planes of W2.T[dff, dh*512+j]
        "W2S": np.ascontiguousarray(
            _hilo8(
                (W2_PRESCALE * W2.T).reshape(DFF, 2, 512).transpose(1, 0, 2)
            ).transpose(1, 0, 2, 3)
        ),
        "b1r": np.ascontiguousarray(b1.reshape(32, 128).T),
        "b2": b2.reshape(1, D),
        "g1": g1.reshape(1, D),
        "be1": be1.reshape(1, D),
        "g2": g2.reshape(1, D),
        "be2": be2.reshape(1, D),
    }

    in_maps = []
    for c in range(NCORES):
        b = c // 2
        half = c % 2
        rows = slice(half * NT, (half + 1) * NT)
        # key-axis permutation: own query rows first (Q proj reads the first
        # NT columns of srcT8); keys are a contraction axis everywhere, so
        # only kaug_x must be permuted consistently.
        perm = np.r_[half * NT : (half + 1) * NT, (1 - half) * NT : (2 - half) * NT]
        x = coords[b, :, 0].astype(np.float64)
        y = coords[b, :, 1].astype(np.float64)
        s = (x + y).astype(np.float32)
        thr = np.arange(1, GRID, dtype=np.float64)
        cx = (x[None, :] >= thr[:, None]).astype(np.float32)
        cy = (y[None, :] >= thr[:, None]).astype(np.float32)
        kaug = np.concatenate(
            [s.reshape(1, N), np.zeros((1, N), np.float32), cx, cy], axis=0
        ).astype(BF)
        qaug = np.empty((64, NT), np.float32)
        qaug[0, :] = 1.0
        qaug[1, :] = 0.0
        qaug[2:33, :] = -2.0 * cx[:, rows]
        qaug[33:64, :] = -2.0 * cy[:, rows]
        srcTb = np.ascontiguousarray(src[b].T[:, perm])
        m = dict(shared)
        m.update(
            {
                "srcT8": _hilo8(S_PRESCALE * srcTb),
                "src_rows": np.ascontiguousarray(src[b, rows, :]),
                "kaug_x": np.ascontiguousarray(kaug[:, perm]).reshape(64, 1, N),
                "qaug_x": qaug.astype(BF).reshape(64, 1, NT),
            }
        )
        in_maps.append(m)
    return in_maps


_NCS = {}
LAST_RUN_S = None


def get_nc(trivial_affine=True):
    if trivial_affine not in _NCS:
        _NCS[trivial_affine] = build_nc(trivial_affine)
    return _NCS[trivial_affine]


def _affine_trivial(inputs):
    return (
        np.all(np.asarray(inputs["g1"]) == 1.0)
        and np.all(np.asarray(inputs["g2"]) == 1.0)
        and not np.any(np.asarray(inputs["be1"]))
        and not np.any(np.asarray(inputs["be2"]))
        and not np.any(np.asarray(inputs["b2"]))
    )


def kernel(**inputs):
    global LAST_RUN_S
    from concourse.bass_utils import run_bass_kernel_spmd

    nc = get_nc(bool(_affine_trivial(inputs)))
    in_maps = host_prep(inputs)
    t0 = time.monotonic()
    res = run_bass_kernel_spmd(nc, in_maps, list(range(NCORES)))
    LAST_RUN_S = time.monotonic() - t0
    full = np.empty((B, N, D), np.float32)
    for c in range(NCORES):
        b = c // 2
        half = c % 2
        full[b, half * NT : (half + 1) * NT, :] = res.results[c]["out"]
    return full



# revision 90
# speedup vs baseline: 1.2264x; 1.0049x over previous
"""Fused transformer encoder layer (attention w/ 2D-ALiBi bias + FFN) on 8 trn2 cores.

Sharding: core c handles batch b = c//2, token half h = c%2 (512 query rows).
K/V are computed per-core for the full 1024-token sequence of its batch
(duplicated across the 2 cores sharing a batch); outputs are disjoint row
slices of the final tensor, so no collectives are needed.

Bias trick: the alibi_2d bias slope_h*(|xi-xj|+|yi-yj|) is folded into the
QK^T contraction. |xi-xj| = xi + xj - 2*a_i.a_j with a_i in {0,1}^31 the
threshold indicators of xi, so dist(i,j) = s_i + s_j - 2*c_i.c_j (c = 62-dim
indicator, s = x+y). The per-query term slope*s_i is constant along the
softmax axis and is dropped. Q/K are augmented with 64 extra contraction dims
(s_j / pad / c_j on the K side; 1 / 0 / -2*c_i on the Q side), making the
score contraction K = 64+64 = 128 exactly — full PE array, bias for free.

bf16 precision care: the aug rows are small integers / {0,-2} — exact in
bf16. The attention scale AND the per-head slope are folded out of the bf16
data: Q-projection weights carry scale/slope_h per head (so scores come out
as S/slope_h) and the exact fp32 slope_h is re-applied as the exp()
activation's scale immediate. exp needs no max-subtraction (|S| <= ~50 by
construction).

Scores are computed keys-on-partitions (S^T layout) so the exp() output is
already P^T for the AV matmul (no transpose). Softmax denominators come from
an appended ones-column in V; normalization is deferred past the (linear)
output projection boundary: each head's O^T rows are scaled by a broadcasted
1/den (built with a small fp32 selector matmul) before the head-summing
projection.
"""

import math
import sys
import time

for _p in ("/opt/trn_rl_repo",):
    if _p not in sys.path:
        sys.path.insert(0, _p)

import numpy as np
import ml_dtypes

import concourse.bass as bass
import concourse.tile as tile
from concourse import bacc, mybir
from concourse.masks import make_identity

F32 = mybir.dt.float32
F32R = mybir.dt.float32r
BF16 = mybir.dt.bfloat16
F8 = mybir.dt.float8e4
BF = ml_dtypes.bfloat16
F8NP = ml_dtypes.float8_e4m3
DR = mybir.MatmulPerfMode.DoubleRow

# fp8 FFN config: activations split into fp8 hi+lo planes (quantization error
# feedback), weights plain fp8. Splitting halves the DoubleRow speedup for
# that operand but removes its quantization error from the output.
FFN_X_SPLIT = True   # x1T (FFN1 input) hi/lo
FFN_H_SPLIT = True   # h1T (FFN2 input) hi/lo

# fp8e4m3 normals span [2^-6, 448]; W1/W2 entries (sigma ~ 1/32) and the lo
# planes would otherwise land in the subnormal range and lose most precision,
# so everything is pre-scaled up into the normal range and the product scale
# is divided back out at the PSUM->SBUF boundary (gelu scale / y descale).
W1_PRESCALE = 256.0
W2_PRESCALE = 256.0
X_PRESCALE = 16.0
H_PRESCALE = 32.0
S_PRESCALE = 8.0     # src (sigma 1) for the QKV projections
WP_PRESCALE = 256.0  # Wq/Wk/Wv/Wo (sigma 1/32)
OT_PRESCALE = 16.0   # normalized attention outputs (sigma ~1)

D = 1024          # d_model
H = 16            # heads
HD = 64           # head dim
DFF = 4096
B = 4
N = 1024          # sequence length
NT = 512          # tokens (query rows) per core
GRID = 32
EPS = 1e-5
NCORES = 8
SCALE = HD ** -0.5


def _alibi_slopes(n):
    def pow2(n_):
        start = 2.0 ** (-(2.0 ** -(math.log2(n_) - 3)))
        return [start * start ** i for i in range(n_)]
    if math.log2(n).is_integer():
        return np.array(pow2(n), dtype=np.float64)
    m = 2 ** math.floor(math.log2(n))
    s = pow2(m)
    s += [s[-1] * 0.5 ** (i + 1) for i in range(n - m)]
    return np.array(s, dtype=np.float64)


SLOPES = _alibi_slopes(H)


def build_nc(trivial_affine=False):
    """trivial_affine: g1/g2 all-ones and be1/be2/b2 all-zeros -> skip those ops."""
    nc = bacc.Bacc()

    # srcT8 columns are permuted per-core so the core's own 512 query rows
    # come FIRST (Q proj reads stf[:, :, :, 0:NT]); key order is a contraction
    # axis everywhere else, so the permutation is invisible provided kaug_x
    # and the V layout use the same order (host_prep keeps them consistent).
    srcT8 = nc.declare_dram_parameter("srcT8", [2, D, N], F8, isOutput=False)
    src_rows = nc.declare_dram_parameter("src_rows", [NT, D], F32, isOutput=False)
    # Wq/Wk swizzled per output-block dt (head pair): WqS8[dt, w, p, c*128+j]
    # = Wq.T[c*128+p, dt*128+j], so each dt block (both planes) is one
    # contiguous DMA and head pair dt can project as soon as it lands.
    WqS8 = nc.declare_dram_parameter("WqS8", [8, 2, 128, D], F8, isOutput=False)
    WkS8 = nc.declare_dram_parameter("WkS8", [8, 2, 128, D], F8, isOutput=False)
    # Wv swizzled by dh half: WvS8[dh, w, p, c*512+j] = Wv.T[c*128+p, dh*512+j]
    WvS8 = nc.declare_dram_parameter("WvS8", [2, 2, 128, 8 * 512], F8, isOutput=False)
    WoT8 = nc.declare_dram_parameter("WoT8", [2, D, D], F8, isOutput=False)
    # W1 pre-swizzled on host: W1S[w, ft, p, dc*128+j] = W1.T[dc*128+p, ft*128+j]
    # (w = fp8 hi/lo plane) so each FFN1 weight block is contiguous per plane.
    W1S = nc.declare_dram_parameter("W1S", [2, 32, 128, D], F8, isOutput=False)
    # W2 split by output half dh so dh0 can stream during FFN1 and dh1
    # during the FFN2 dh0 pass: W2S[dh, w, dff, j] = W2.T[dff, dh*512+j]
    W2S = nc.declare_dram_parameter("W2S", [2, 2, DFF, 512], F8, isOutput=False)
    # aug blocks are head-independent; the singleton dim enables stride-0
    # broadcast DMAs into all head slots.
    kaug_x = nc.declare_dram_parameter("kaug_x", [64, 1, N], BF16, isOutput=False)
    qaug_x = nc.declare_dram_parameter("qaug_x", [64, 1, NT], BF16, isOutput=False)
    b1r = nc.declare_dram_parameter("b1r", [128, 32], F32, isOutput=False)
    b2 = nc.declare_dram_parameter("b2", [1, D], F32, isOutput=False)
    g1 = nc.declare_dram_parameter("g1", [1, D], F32, isOutput=False)
    be1 = nc.declare_dram_parameter("be1", [1, D], F32, isOutput=False)
    g2 = nc.declare_dram_parameter("g2", [1, D], F32, isOutput=False)
    be2 = nc.declare_dram_parameter("be2", [1, D], F32, isOutput=False)
    out = nc.declare_dram_parameter("out", [NT, D], F32, isOutput=True)

    AF = mybir.ActivationFunctionType
    OP = mybir.AluOpType

    with tile.TileContext(nc) as tc:
        with (
            tc.tile_pool(name="misc", bufs=1) as misc,
            tc.tile_pool(name="lnp", bufs=4) as lnp,
            tc.tile_pool(name="pre", bufs=1) as pre,
        ):
            eps_sb = misc.tile([128, 1], F32, tag="eps")
            nc.vector.memset(eps_sb, EPS)
            ident = misc.tile([128, 128], F32, tag="ident")
            make_identity(nc, ident)
            ident_bf = misc.tile([128, 128], BF16, tag="identbf")
            make_identity(nc, ident_bf)
            # OT8[p, c, q]: head 2c in partitions 0:64, head 2c+1 in 64:128;
            # fp8 hi/lo planes (scaled by OT_PRESCALE) for the 3-term out-proj
            OT8_hi = misc.tile([128, 8, NT], F8, tag="ot8h")

            def ln_apply(x_ap, gbc, bbc):
                stats = lnp.tile([128, 2, 6], F32, tag="lnstats", name="lnstats")
                for sg in range(2):
                    nc.vector.bn_stats(
                        out=stats[:, sg, :], in_=x_ap[:, sg * 512 : sg * 512 + 512]
                    )
                mv = lnp.tile([128, 2], F32, tag="lnmv", name="lnmv")
                nc.vector.bn_aggr(out=mv, in_=stats)
                nc.scalar.activation(
                    out=mv[:, 1:2], in_=mv[:, 1:2], func=AF.Sqrt,
                    bias=eps_sb, scale=1.0,
                )
                nc.vector.reciprocal(out=mv[:, 1:2], in_=mv[:, 1:2])
                nc.vector.tensor_scalar(
                    out=x_ap, in0=x_ap,
                    scalar1=mv[:, 0:1], scalar2=mv[:, 1:2],
                    op0=OP.subtract, op1=OP.mult,
                )
                if gbc is not None:
                    nc.vector.tensor_mul(out=x_ap, in0=x_ap, in1=gbc)
                if bbc is not None:
                    nc.vector.tensor_add(out=x_ap, in0=x_ap, in1=bbc)

            # ============ merged projections + attention ============
            # Single software pipeline: per step s, project K/Q for head pair
            # s, run scores+exp for pair s-1, AV+normalize for pair s-2, and
            # transpose/quantize pair s-3. V projections burst at steps 1/4.
            # AV runs in natural orientation (A = P^T chunk, B = V columns
            # incl. a ones column -> out [128 queries, 64 vdims + den]), so
            # softmax denominators are per-PARTITION and normalization is a
            # plain tensor_scalar; O^T for the out-proj comes from cheap bf16
            # PE transposes of head pairs.
            # wqk opens before att so its released zone sits at the stack
            # bottom: the FFN W1 ring reuses it, anchored on the early
            # K(7)/Q(7) last-uses instead of the late attention tail.
            with (
                tc.tile_pool(name="wqk", bufs=1) as wqk,
                tc.tile_pool(name="att", bufs=1) as att,
            ):
                kaug = att.tile([128, H, N], BF16, tag="kaug")
                qaug = att.tile([128, H, NT], BF16, tag="qaug")
                v_sb = att.tile([128, 8, H * 65], BF16, tag="vsb")
                v4 = v_sb.rearrange("p m (h w) -> p m h w", w=65)
                nc.vector.memset(v4[:, :, :, 64], 1.0)

                P_DS = 1.0 / (S_PRESCALE * WP_PRESCALE)
                # (activation plane, weight plane) product terms; lo*lo skipped
                TERMS = ((0, 0), (1, 0), (0, 1))

                def mm3t(ps, w8, x8, wslice, xslice):
                    i = 0
                    for a, b in TERMS:
                        for dr in range(4):
                            pl = slice(2 * dr, 2 * dr + 2)
                            nc.tensor.matmul(
                                ps,
                                w8[:, b, pl, wslice],
                                x8[:, a, pl, xslice],
                                start=(i == 0), stop=(i == 11),
                                perf_mode=DR,
                            )
                            i += 1

                with tc.tile_pool(name="ph1", bufs=1) as ph1:
                    # DMA emission order tracks first-use order; weights are
                    # host-swizzled so each head pair's block is one
                    # contiguous transfer.
                    wqf = wqk.tile([128, 8, 2, 8, 128], F8, tag="wqf")
                    stf = ph1.tile([128, 2, 8, N], F8, tag="stf")
                    st_vw = srcT8[:, :, :].rearrange("w (c p) n -> p w c n", p=128)
                    wkf = wqk.tile([128, 8, 2, 8, 128], F8, tag="wkf")
                    wvf = ph1.tile([128, 2, 2, 8, 512], F8, tag="wvf")

                    def dma_wdt(dst, src, dt):
                        nc.sync.dma_start(
                            out=dst[:, dt, :, :, :],
                            in_=src[dt, :, :, :].rearrange(
                                "w p (c n) -> p w c n", c=8
                            ),
                        )

                    def dma_aug(h0, h1):
                        # stride-0 broadcast of the shared aug block into
                        # head slots [h0, h1)
                        nc.sync.dma_start(
                            out=kaug[64:128, h0:h1, :],
                            in_=kaug_x[:, :, :].to_broadcast([64, h1 - h0, N]),
                        )
                        nc.sync.dma_start(
                            out=qaug[64:128, h0:h1, :],
                            in_=qaug_x[:, :, :].to_broadcast([64, h1 - h0, NT]),
                        )

                    # need-ordered: Q(0) first (own src columns + wq block 0),
                    # then K(0), aug for early heads, V(dh0), and the rest
                    # staggered against consumption.
                    nc.sync.dma_start(out=stf[:, 0, :, 0:NT], in_=st_vw[:, 0, :, 0:NT])
                    dma_wdt(wqf, WqS8, 0)
                    nc.sync.dma_start(out=stf[:, 1, :, 0:NT], in_=st_vw[:, 1, :, 0:NT])
                    nc.sync.dma_start(out=stf[:, 0, :, NT:N], in_=st_vw[:, 0, :, NT:N])
                    dma_wdt(wkf, WkS8, 0)
                    nc.sync.dma_start(out=stf[:, 1, :, NT:N], in_=st_vw[:, 1, :, NT:N])
                    dma_aug(0, 2)
                    dma_wdt(wqf, WqS8, 1)
                    dma_wdt(wkf, WkS8, 1)
                    for w in range(2):
                        nc.sync.dma_start(
                            out=wvf[:, 0, w, :, :],
                            in_=WvS8[0, w, :, :].rearrange("p (c n) -> p c n", c=8),
                        )
                    dma_aug(2, 4)
                    dma_wdt(wqf, WqS8, 2)
                    dma_wdt(wkf, WkS8, 2)
                    dma_aug(4, 6)
                    dma_wdt(wqf, WqS8, 3)
                    dma_wdt(wkf, WkS8, 3)
                    dma_aug(6, 8)
                    dma_wdt(wqf, WqS8, 4)
                    dma_wdt(wkf, WkS8, 4)
                    dma_aug(8, 12)
                    for w in range(2):
                        nc.sync.dma_start(
                            out=wvf[:, 1, w, :, :],
                            in_=WvS8[1, w, :, :].rearrange("p (c n) -> p c n", c=8),
                        )
                    dma_aug(12, 16)
                    for dt in range(5, 8):
                        dma_wdt(wqf, WqS8, dt)
                        dma_wdt(wkf, WkS8, dt)

                    # phase 3-5 prefetches ride the queue tail; they land
                    # long before the out-proj needs them.
                    if trivial_affine:
                        g1bc = be1bc = b2bc = g2bc = be2bc = None
                    else:
                        g1bc = pre.tile([128, D], F32, tag="g1bc")
                        be1bc = pre.tile([128, D], F32, tag="be1bc")
                        b2bc = pre.tile([128, D], F32, tag="b2bc")
                        g2bc = pre.tile([128, D], F32, tag="g2bc")
                        be2bc = pre.tile([128, D], F32, tag="be2bc")
                        for t_, src_ in (
                            (g1bc, g1), (be1bc, be1),
                            (b2bc, b2), (g2bc, g2), (be2bc, be2),
                        ):
                            nc.sync.dma_start(
                                out=t_, in_=src_[:, :].to_broadcast([128, D])
                            )
                    srar = pre.tile([128, 4, D], F32, tag="srcrows")
                    nc.sync.dma_start(
                        out=srar,
                        in_=src_rows[:, :].rearrange("(nt p) d -> p nt d", p=128),
                    )
                    wof = pre.tile([128, 2, 8, D], F8, tag="wof")
                    wo_vw = WoT8[:, :, :].rearrange("w (c p) n -> p w c n", p=128)
                    for w in range(2):
                        nc.sync.dma_start(out=wof[:, w, :, :], in_=wo_vw[:, w, :, :])
                    b1_sb = pre.tile([128, 32], F32, tag="b1")
                    nc.sync.dma_start(out=b1_sb, in_=b1r[:, :])

                    with (
                        tc.tile_pool(name="ptp", bufs=2) as ptp,
                        tc.tile_pool(name="obp", bufs=3) as obp,
                        tc.tile_pool(name="rcp", bufs=2) as rcp,
                        tc.tile_pool(name="psS", bufs=2, space="PSUM") as psS,
                        tc.tile_pool(name="psP", bufs=2, space="PSUM") as psP,
                        tc.tile_pool(name="psO", bufs=1, space="PSUM") as psO,
                        tc.tile_pool(name="psT", bufs=1, space="PSUM") as psT,
                    ):
                        pts = {}
                        obfs = {}

                        # p-state warmup: the PE needs ~3us of continuous
                        # execution to reach 2.4 GHz; burn the DMA cold-start
                        # on dependency-free dummy matmuls so the first real
                        # projections run at full clock.
                        wu = psP.tile([128, 512], F32, tag="pj", name="warm")
                        for _ in range(8):
                            nc.tensor.matmul(
                                wu[:, 0:128], ident, ident, start=True, stop=True
                            )

                        def mm3p(ps, w8, dt, xslice):
                            i = 0
                            for a, b in TERMS:
                                for dr in range(4):
                                    pl = slice(2 * dr, 2 * dr + 2)
                                    nc.tensor.matmul(
                                        ps,
                                        w8[:, dt, b, pl, :],
                                        stf[:, a, pl, xslice],
                                        start=(i == 0), stop=(i == 11),
                                        perf_mode=DR,
                                    )
                                    i += 1

                        def emit_K(dt, mh):
                            kps = psP.tile([128, 512], F32, tag="pj", name="kps")
                            mm3p(kps, wkf, dt, slice(mh * 512, mh * 512 + 512))
                            ksl = slice(mh * 512, mh * 512 + 512)
                            nc.vector.tensor_scalar_mul(
                                out=kaug[0:64, 2 * dt, ksl],
                                in0=kps[0:64, :], scalar1=P_DS,
                            )
                            nc.vector.tensor_scalar_mul(
                                out=kaug[0:64, 2 * dt + 1, ksl],
                                in0=kps[64:128, :], scalar1=P_DS,
                            )

                        def emit_Q(dt):
                            # own query rows are the FIRST NT columns of stf
                            qps = psP.tile([128, NT], F32, tag="pj", name="qps")
                            mm3p(qps, wqf, dt, slice(0, NT))
                            nc.vector.tensor_scalar_mul(
                                out=qaug[0:64, 2 * dt, :], in0=qps[0:64, :],
                                scalar1=float(SCALE / SLOPES[2 * dt] * P_DS),
                            )
                            nc.vector.tensor_scalar_mul(
                                out=qaug[0:64, 2 * dt + 1, :], in0=qps[64:128, :],
                                scalar1=float(SCALE / SLOPES[2 * dt + 1] * P_DS),
                            )

                        def emit_V_pair(p):
                            # V projection for head pair p only (just-in-time
                            # for AV(2p) one step later; spreads V across the
                            # pipeline and fills the exp-drain tail)
                            dh, hc = p // 4, (p % 4) * 128
                            for mg in range(2):
                                vt = psP.tile(
                                    [128, 4, 128], F32, tag="pj", name="vps"
                                )
                                for lm in range(4):
                                    mt = mg * 4 + lm
                                    i = 0
                                    for a, b in TERMS:
                                        for dr in range(4):
                                            pl = slice(2 * dr, 2 * dr + 2)
                                            nc.tensor.matmul(
                                                vt[:, lm, :],
                                                stf[:, a, pl, mt * 128 : mt * 128 + 128],
                                                wvf[:, dh, b, pl, hc : hc + 128],
                                                start=(i == 0), stop=(i == 11),
                                                perf_mode=DR,
                                            )
                                            i += 1
                                nc.vector.tensor_scalar_mul(
                                    out=v4[:, mg * 4 : mg * 4 + 4, 2 * p : 2 * p + 2, 0:64],
                                    in0=vt.rearrange("p m (h w) -> p m h w", w=64),
                                    scalar1=P_DS,
                                )

                        def emit_S(h, cs):
                            pt = pts[h]
                            for c in cs:
                                stc = psS.tile([128, 2, NT], F32, tag="st", name="stc")
                                for j in range(2):
                                    mt = 2 * c + j
                                    nc.tensor.matmul(
                                        stc[:, j, :],
                                        kaug[:, h, mt * 128 : mt * 128 + 128],
                                        qaug[:, h, :],
                                        start=True, stop=True,
                                    )
                                nc.scalar.activation(
                                    out=pt[:, 2 * c : 2 * c + 2, :], in_=stc,
                                    func=AF.Exp, scale=float(SLOPES[h]),
                                )

                        def emit_AV(h):
                            # natural orientation: out [128 queries, 64 vdims
                            # + den]; col 64 accumulates the softmax denom via
                            # the ones column in V.
                            cc = h // 2
                            if h % 2 == 0:
                                obfs[cc] = obp.tile(
                                    [128, 4, 128], BF16, tag="ob", name="obf"
                                )
                            po = psO.tile([128, 4, 65], F32, tag="po", name="po")
                            for qc in range(4):
                                for mt in range(8):
                                    nc.tensor.matmul(
                                        po[:, qc, :],
                                        pts[h][:, mt, qc * 128 : qc * 128 + 128],
                                        v4[:, mt, h, 0:65],
                                        start=(mt == 0), stop=(mt == 7),
                                    )
                            rc = rcp.tile([128, 4], F32, tag="rc", name="rc")
                            nc.vector.reciprocal(out=rc, in_=po[:, :, 64])
                            nc.vector.tensor_scalar_mul(
                                out=rc, in0=rc, scalar1=OT_PRESCALE
                            )
                            # normalize promptly on DVE (psO has one buffer;
                            # the next AV waits on these reads)
                            hb = (h % 2) * 64
                            for qc in range(4):
                                nc.vector.tensor_scalar_mul(
                                    out=obfs[cc][:, qc, hb : hb + 64],
                                    in0=po[:, qc, 0:64],
                                    scalar1=rc[:, qc : qc + 1],
                                )
                            pts.pop(h)

                        def emit_pair(cc):
                            ob = obfs.pop(cc)
                            tp = psT.tile([128, 4, 128], BF16, tag="tp", name="tp")
                            for qc in range(4):
                                nc.tensor.transpose(
                                    tp[:, qc, :], ob[:, qc, :], ident_bf
                                )
                            tpf = tp.rearrange("p a b -> p (a b)")
                            # gpsimd cannot touch PSUM on hw; Act does the
                            # copy. Single fp8 plane for O (the out-proj runs
                            # 2-term: O_hi x Wo_hi + O_hi x Wo_lo).
                            nc.scalar.activation(
                                out=OT8_hi[:, cc, :], in_=tpf, func=AF.Copy
                            )

                        def step(s):
                            h0, h1 = 2 * (s - 1), 2 * (s - 1) + 1
                            av0, av1 = 2 * (s - 2), 2 * (s - 2) + 1
                            if 2 <= s <= 9:
                                emit_AV(av0)
                            if s < 8:
                                emit_Q(s)
                            if 1 <= s <= 8:
                                pts[h0] = ptp.tile(
                                    [128, 8, NT], BF16, tag="pt", name="pt"
                                )
                                emit_S(h0, (0, 1))
                            if 2 <= s <= 9:
                                emit_AV(av1)
                            if s < 8:
                                emit_K(s, 0)
                            if 1 <= s <= 8:
                                emit_S(h0, (2, 3))
                            if s < 8:
                                emit_K(s, 1)
                            if 1 <= s <= 8:
                                pts[h1] = ptp.tile(
                                    [128, 8, NT], BF16, tag="pt", name="pt"
                                )
                                emit_S(h1, (0, 1))
                            if 1 <= s <= 8:
                                emit_V_pair(s - 1)
                            if 1 <= s <= 8:
                                emit_S(h1, (2, 3))
                            if s >= 3:
                                emit_pair(s - 3)

                        for s in range(11):
                            step(s)

            # ============ post-attention scope ============
            with (
                tc.tile_pool(name="w1p", bufs=20 if trivial_affine else 10) as w1p,
                tc.tile_pool(name="ffn", bufs=1) as ffn,
            ):
                # w1p opens FIRST so the W1 ring lands on the earliest-freed
                # attention SBUF and its stream starts during the tail.
                # W2 halves [p, dh, w, c, n]: dh0 streams during FFN1, dh1
                # during the FFN2 dh0 pass (keeps FFN1's W1 stream PE-bound)
                W2h = ffn.tile([128, 2, 2, 32, 512], F8, tag="w2")
                w2_v = [
                    W2S[dh, :, :, :].rearrange("w (c p) n -> p w c n", p=128)
                    for dh in range(2)
                ]

                def dma_w2(dh, cg):
                    # one [128, w, 4, 512] chunk (0.25 MB) per call
                    for w in range(2):
                        nc.sync.dma_start(
                            out=W2h[:, dh, w, 4 * cg : 4 * cg + 4, :],
                            in_=w2_v[dh][:, w, 4 * cg : 4 * cg + 4, :],
                        )
                x1_sb = ffn.tile([128, 4, D], F32, tag="x1")
                x1T_hi = ffn.tile([128, 8, NT], F8, tag="x1Th")
                x1T_lo = (
                    ffn.tile([128, 8, NT], F8, tag="x1Tl", name="x1T_lo") if FFN_X_SPLIT else None
                )

                # --- phase 3: out-proj, LN1, transpose ---
                with (
                    tc.tile_pool(name="psS2", bufs=4, space="PSUM") as psS2,
                    tc.tile_pool(name="psT", bufs=2, space="PSUM") as psT,
                    tc.tile_pool(name="xqp", bufs=2) as xqp,
                ):
                    def transposes(nt, xq):
                        # xq holds bf16 16*LN(x1)[nt]; hw forbids plain fp8
                        # transposes, so transpose bf16 and split hi/lo after.
                        nsl = slice(nt * 128, nt * 128 + 128)
                        tp = psT.tile([128, 8, 128], BF16, tag="tp", name="tp")
                        for c in range(8):
                            nc.tensor.transpose(
                                tp[:, c, :],
                                xq[:, c * 128 : c * 128 + 128],
                                ident_bf,
                            )
                        nc.scalar.activation(
                            out=x1T_hi[:, :, nsl], in_=tp, func=AF.Copy
                        )
                        if FFN_X_SPLIT:
                            nc.vector.tensor_sub(
                                out=x1T_lo[:, :, nsl],
                                in0=tp, in1=x1T_hi[:, :, nsl],
                            )

                    def quantize_half(nt, hsl, xq):
                        nc.scalar.activation(
                            out=xq[:, hsl], in_=x1_sb[:, nt, hsl],
                            func=AF.Copy, scale=X_PRESCALE,
                        )

                    # 2-term out-proj: O is a single fp8 plane (error ~fp8
                    # quant of O, emulated ~+0.007 rel; tolerance 0.02)
                    OP_TERMS = ((0, 0), (0, 1))
                    OT8 = (OT8_hi,)
                    O_DS = 1.0 / (OT_PRESCALE * WP_PRESCALE)
                    st1 = [
                        lnp.tile([128, 2, 6], F32, tag=f"ln1s{nt}", name=f"ln1s{nt}")
                        for nt in range(4)
                    ]
                    xqs = {}
                    deferred_applies = []
                    for nt in range(4):
                        for dh in range(2):
                            s2 = psS2.tile([128, 512], F32, tag="s2", name="s2")
                            # dr-major: only the last 3 matmuls (head pairs
                            # 6-7) wait on the final attention quantize
                            i = 0
                            for dr in range(4):
                                pl = slice(2 * dr, 2 * dr + 2)
                                for a, b in OP_TERMS:
                                    nc.tensor.matmul(
                                        s2,
                                        OT8[a][:, pl, nt * 128 : nt * 128 + 128],
                                        wof[:, b, pl, dh * 512 : dh * 512 + 512],
                                        start=(i == 0), stop=(i == 7),
                                        perf_mode=DR,
                                    )
                                    i += 1
                            hsl = slice(dh * 512, dh * 512 + 512)
                            nc.vector.scalar_tensor_tensor(
                                out=x1_sb[:, nt, hsl],
                                in0=s2, scalar=O_DS,
                                in1=srar[:, nt, hsl],
                                op0=OP.mult, op1=OP.add,
                            )
                            # half-stats immediately: shortens the LN chain
                            nc.vector.bn_stats(
                                out=st1[nt][:, dh, :], in_=x1_sb[:, nt, hsl]
                            )
                        mv = lnp.tile([128, 2], F32, tag="lnmv", name="lnmv")
                        nc.vector.bn_aggr(out=mv, in_=st1[nt])
                        nc.scalar.activation(
                            out=mv[:, 1:2], in_=mv[:, 1:2], func=AF.Sqrt,
                            bias=eps_sb, scale=1.0,
                        )
                        nc.vector.reciprocal(out=mv[:, 1:2], in_=mv[:, 1:2])
                        xq = xqp.tile([128, D], BF16, tag="xq", name="xq")
                        xqs[nt] = xq
                        if g1bc is None:
                            # fold the LN apply into the quantize: xq =
                            # 16*(x-mu)*r straight from the raw x1 (Identity
                            # with per-partition scale/bias), so the DVE/Pool
                            # apply drops off the FFN1-gating chain; the
                            # in-place apply for the phase-5 residual runs
                            # after the quantize reads.
                            rs = lnp.tile([128, 2], F32, tag="lnrs", name="lnrs")
                            nc.vector.scalar_tensor_tensor(
                                out=rs[:, 1:2], in0=mv[:, 0:1],
                                scalar=-X_PRESCALE, in1=mv[:, 1:2],
                                op0=OP.mult, op1=OP.mult,
                            )
                            nc.vector.tensor_scalar_mul(
                                out=rs[:, 0:1], in0=mv[:, 1:2],
                                scalar1=X_PRESCALE,
                            )
                            for hh2 in range(2):
                                h2 = slice(hh2 * 512, hh2 * 512 + 512)
                                nc.scalar.activation(
                                    out=xq[:, h2], in_=x1_sb[:, nt, h2],
                                    func=AF.Identity,
                                    scale=rs[:, 0:1], bias=rs[:, 1:2],
                                )
                            # the in-place apply is only read by phase 5:
                            # defer it past the transposes so it never
                            # occupies the DVE/Pool queues on the chain
                            deferred_applies.append((nt, mv))
                        else:
                            for hh2, eng in ((0, nc.vector), (1, nc.gpsimd)):
                                h2 = slice(hh2 * 512, hh2 * 512 + 512)
                                eng.tensor_scalar(
                                    out=x1_sb[:, nt, h2], in0=x1_sb[:, nt, h2],
                                    scalar1=mv[:, 0:1], scalar2=mv[:, 1:2],
                                    op0=OP.subtract, op1=OP.mult,
                                )
                                eng.tensor_mul(
                                    out=x1_sb[:, nt, h2],
                                    in0=x1_sb[:, nt, h2], in1=g1bc[:, h2],
                                )
                                if be1bc is not None:
                                    eng.tensor_add(
                                        out=x1_sb[:, nt, h2],
                                        in0=x1_sb[:, nt, h2], in1=be1bc[:, h2],
                                    )
                                quantize_half(nt, h2, xq)
                        # emitted AFTER the LN/quantize block: the next nt's
                        # gating chain gets queue priority over these copies
                        if nt >= 1:
                            transposes(nt - 1, xqs.pop(nt - 1))
                    transposes(3, xqs.pop(3))
                    for nt_, mv_ in deferred_applies:
                        for hh2, eng in ((0, nc.vector), (1, nc.gpsimd)):
                            h2 = slice(hh2 * 512, hh2 * 512 + 512)
                            eng.tensor_scalar(
                                out=x1_sb[:, nt_, h2], in0=x1_sb[:, nt_, h2],
                                scalar1=mv_[:, 0:1], scalar2=mv_[:, 1:2],
                                op0=OP.subtract, op1=OP.mult,
                            )

                # --- phase 4: FFN1 (fp8 DoubleRow matmuls, gelu into fp8 h1T) ---
                h1T_hi = ffn.tile([128, 32, NT], F8, tag="h1Th")
                h1T_lo = (
                    ffn.tile([128, 32, NT], F8, tag="h1Tl", name="h1T_lo") if FFN_H_SPLIT else None
                )
                NPRE = 0
                with (
                    tc.tile_pool(name="h1gp", bufs=3) as h1gp,
                    tc.tile_pool(name="psH", bufs=3, space="PSUM") as psH,
                    tc.tile_pool(name="psHp", bufs=max(NPRE, 1), space="PSUM") as psHp,
                ):
                    # term list: (x plane, w plane); lo*lo is negligible
                    # lo-dependent terms last: the first 8 matmuls of each
                    # group only need the hi plane
                    x_terms = [(x1T_hi, 0), (x1T_hi, 1), (x1T_lo, 0)] \
                        if FFN_X_SPLIT else [(x1T_hi, 0), (x1T_hi, 1)]

                    def dma_w1(ft):
                        w1 = w1p.tile([128, 2, 8, 128], F8, tag="w1col", name="w1")
                        # ring-buffer DMAs block SP on their WAR semaphore;
                        # the 20-deep ring keeps the WAR anchor far behind
                        # consumption so SP's queue never stalls.
                        nc.sync.dma_start(
                            out=w1.rearrange("p w c n -> p w (c n)"),
                            in_=W1S[:, ft, :, :].rearrange("w p n -> p w n"),
                        )
                        return w1

                    def mm_ft_nt(hps, w1, ft, nt):
                        nsl = slice(nt * 128, nt * 128 + 128)
                        i = 0
                        for xh, wp in x_terms:
                            for dr in range(4):
                                nc.tensor.matmul(
                                    hps[:, nsl],
                                    w1[:, wp, 2 * dr : 2 * dr + 2, :],
                                    xh[:, 2 * dr : 2 * dr + 2, nsl],
                                    start=(i == 0), stop=(i == 11),
                                    perf_mode=DR,
                                )
                                i += 1

                    def h1_quant(hps, ft):
                        # PSUM holds (X*W1 prescales)*h; descale via gelu's
                        # input scale, rescale the fp8 planes by H_PRESCALE.
                        in_ds = 1.0 / (X_PRESCALE * W1_PRESCALE)
                        if FFN_H_SPLIT:
                            h1g = h1gp.tile([128, NT], BF16, tag="h1g", name="h1g")
                            nc.scalar.activation(
                                out=h1g, in_=hps, func=AF.Gelu,
                                bias=b1_sb[:, ft : ft + 1], scale=in_ds,
                            )
                            nc.vector.tensor_scalar_mul(
                                out=h1T_hi[:, ft, :], in0=h1g, scalar1=H_PRESCALE
                            )
                            nc.vector.scalar_tensor_tensor(
                                out=h1T_lo[:, ft, :], in0=h1g, scalar=H_PRESCALE,
                                in1=h1T_hi[:, ft, :], op0=OP.mult, op1=OP.subtract,
                            )
                        else:
                            nc.scalar.activation(
                                out=h1T_hi[:, ft, :], in_=hps, func=AF.Gelu,
                                bias=b1_sb[:, ft : ft + 1], scale=in_ds,
                            )

                    # The first NPRE fts run nt-sliced and nt-major: their
                    # (ft, nt) groups start as each x1T token tile lands,
                    # filling the PE during the phase-3 LN/quantize drain.
                    pre_w1 = [dma_w1(ft) for ft in range(NPRE)]
                    pre_h = [
                        psHp.tile([128, NT], F32, tag="h1p", name="hpsp")
                        for _ in range(NPRE)
                    ]
                    for nt in range(3):
                        for ft in range(NPRE):
                            mm_ft_nt(pre_h[ft], pre_w1[ft], ft, nt)
                    for ft in range(NPRE):
                        mm_ft_nt(pre_h[ft], pre_w1[ft], ft, 3)
                        h1_quant(pre_h[ft], ft)
                    for ft in range(NPRE, 32):
                        w1 = dma_w1(ft)
                        if NPRE <= ft < NPRE + 8:
                            dma_w2(0, ft - NPRE)
                            dma_w2(1, ft - NPRE)
                        hps = psH.tile([128, NT], F32, tag="h1", name="hps")
                        nmm = 4 * len(x_terms)
                        i = 0
                        for xh, wp in x_terms:
                            for dr in range(4):
                                nc.tensor.matmul(
                                    hps,
                                    w1[:, wp, 2 * dr : 2 * dr + 2, :],
                                    xh[:, 2 * dr : 2 * dr + 2, :],
                                    start=(i == 0), stop=(i == nmm - 1),
                                    perf_mode=DR,
                                )
                                i += 1
                        h1_quant(hps, ft)

                # --- phase 5: FFN2 (dh-major) + residual + LN2 + store ---
                # LN2 stats for the dh0 half are computed during the dh0
                # pass; after the dh1 STT only sg1 stats + apply + store
                # remain on the critical path.
                out_v = out[:, :].rearrange("(nt p) d -> p nt d", p=128)
                with tc.tile_pool(name="psY", bufs=3, space="PSUM") as psY:
                    h_terms = [(h1T_hi, 0), (h1T_hi, 1), (h1T_lo, 0)] \
                        if FFN_H_SPLIT else [(h1T_hi, 0), (h1T_hi, 1)]
                    y_ds = 1.0 / (
                        (H_PRESCALE if FFN_H_SPLIT else 1.0) * W2_PRESCALE
                    )
                    st2 = [
                        lnp.tile([128, 2, 6], F32, tag=f"ln2s{nt}", name=f"ln2s{nt}")
                        for nt in range(4)
                    ]
                    for dh in range(2):
                        for nt in range(4):
                            yps = psY.tile([128, 512], F32, tag="y", name="yps")
                            nmm = 16 * len(h_terms)
                            i = 0
                            for hh, wp in h_terms:
                                for dr in range(16):
                                    nc.tensor.matmul(
                                        yps,
                                        hh[:, 2 * dr : 2 * dr + 2, nt * 128 : nt * 128 + 128],
                                        W2h[:, dh, wp, 2 * dr : 2 * dr + 2, :],
                                        start=(i == 0), stop=(i == nmm - 1),
                                        perf_mode=DR,
                                    )
                                    i += 1
                            hsl = slice(dh * 512, dh * 512 + 512)
                            nc.vector.scalar_tensor_tensor(
                                out=x1_sb[:, nt, hsl],
                                in0=yps, scalar=y_ds,
                                in1=x1_sb[:, nt, hsl],
                                op0=OP.mult, op1=OP.add,
                            )
                            if b2bc is not None:
                                nc.vector.tensor_add(
                                    out=x1_sb[:, nt, hsl],
                                    in0=x1_sb[:, nt, hsl],
                                    in1=b2bc[:, hsl],
                                )
                            nc.vector.bn_stats(
                                out=st2[nt][:, dh, :], in_=x1_sb[:, nt, hsl]
                            )
                            if dh == 1:
                                mv = lnp.tile([128, 2], F32, tag="ln2mv", name="ln2mv")
                                nc.vector.bn_aggr(out=mv, in_=st2[nt])
                                nc.scalar.activation(
                                    out=mv[:, 1:2], in_=mv[:, 1:2], func=AF.Sqrt,
                                    bias=eps_sb, scale=1.0,
                                )
                                nc.vector.reciprocal(out=mv[:, 1:2], in_=mv[:, 1:2])
                                for hh2 in range(2):
                                    h2 = slice(hh2 * 512, hh2 * 512 + 512)
                                    nc.vector.tensor_scalar(
                                        out=x1_sb[:, nt, h2], in0=x1_sb[:, nt, h2],
                                        scalar1=mv[:, 0:1], scalar2=mv[:, 1:2],
                                        op0=OP.subtract, op1=OP.mult,
                                    )
                                    if g2bc is not None:
                                        nc.vector.tensor_mul(
                                            out=x1_sb[:, nt, h2],
                                            in0=x1_sb[:, nt, h2], in1=g2bc[:, h2],
                                        )
                                    if be2bc is not None:
                                        nc.vector.tensor_add(
                                            out=x1_sb[:, nt, h2],
                                            in0=x1_sb[:, nt, h2], in1=be2bc[:, h2],
                                        )
                                    nc.sync.dma_start(
                                        out=out_v[:, nt, h2], in_=x1_sb[:, nt, h2]
                                    )

    nc.finalize()
    return nc


def _hilo8(a):
    """Stack round-to-nearest fp8 hi and residual lo planes: [2, *a.shape]."""
    hi = np.asarray(a, np.float32).astype(F8NP)
    lo = (np.asarray(a, np.float32) - hi.astype(np.float32)).astype(F8NP)
    return np.ascontiguousarray(np.stack([hi, lo], axis=0))


def host_prep(inputs):
    """Build the 8 per-core input maps from the full problem inputs."""
    src = np.asarray(inputs["src"], np.float32)
    coords = np.asarray(inputs["coords"])
    Wq = np.asarray(inputs["Wq"], np.float32)
    Wk = np.asarray(inputs["Wk"], np.float32)
    Wv = np.asarray(inputs["Wv"], np.float32)
    Wo = np.asarray(inputs["Wo"], np.float32)
    W1 = np.asarray(inputs["W1"], np.float32)
    b1 = np.asarray(inputs["b1"], np.float32)
    W2 = np.asarray(inputs["W2"], np.float32)
    b2 = np.asarray(inputs["b2"], np.float32)
    g1 = np.asarray(inputs["g1"], np.float32)
    be1 = np.asarray(inputs["be1"], np.float32)
    g2 = np.asarray(inputs["g2"], np.float32)
    be2 = np.asarray(inputs["be2"], np.float32)

    def _blk8(wt, nblk, blk):
        # [dt, w, p, c*blk + j] from wt.T-like [c*128+p, dt*blk+j]
        x = (WP_PRESCALE * wt.T).reshape(8, 128, nblk, blk)
        x = x.transpose(2, 1, 0, 3).reshape(nblk, 128, 8 * blk)
        return np.ascontiguousarray(_hilo8(x).transpose(1, 0, 2, 3))

    # Projection weights as fp8 hi/lo planes; the per-head SCALE/slope_h for
    # q goes in as the PSUM->qaug copy descale on device.
    shared = {
        "WqS8": _blk8(Wq, 8, 128),
        "WkS8": _blk8(Wk, 8, 128),
        "WvS8": _blk8(Wv, 2, 512),
        "WoT8": _hilo8(WP_PRESCALE * Wo.T),
        # W1S[w, ft, p, dc*128+j] = hi/lo fp8 planes of W1.T[dc*128+p, ft*128+j]
        "W1S": _hilo8(
            (W1_PRESCALE * W1.T)
            .reshape(8, 128, 32, 128).transpose(2, 1, 0, 3).reshape(32, 128, D)
        ),
        # W2S[dh, w, dff, j] = hi/lo planes of W2.T[dff, dh*512+j]
        "W2S": np.ascontiguousarray(
            _hilo8(
                (W2_PRESCALE * W2.T).reshape(DFF, 2, 512).transpose(1, 0, 2)
            ).transpose(1, 0, 2, 3)
        ),
        "b1r": np.ascontiguousarray(b1.reshape(32, 128).T),
        "b2": b2.reshape(1, D),
        "g1": g1.reshape(1, D),
        "be1": be1.reshape(1, D),
        "g2": g2.reshape(1, D),
        "be2": be2.reshape(1, D),
    }

    in_maps = []
    for c in range(NCORES):
        b = c // 2
        half = c % 2
        rows = slice(half * NT, (half + 1) * NT)
        # key-axis permutation: own query rows first (Q proj reads the first
        # NT columns of srcT8); keys are a contraction axis everywhere, so
        # only kaug_x must be permuted consistently.
        perm = np.r_[half * NT : (half + 1) * NT, (1 - half) * NT : (2 - half) * NT]
        x = coords[b, :, 0].astype(np.float64)
        y = coords[b, :, 1].astype(np.float64)
        s = (x + y).astype(np.float32)
        thr = np.arange(1, GRID, dtype=np.float64)
        cx = (x[None, :] >= thr[:, None]).astype(np.float32)
        cy = (y[None, :] >= thr[:, None]).astype(np.float32)
        kaug = np.concatenate(
            [s.reshape(1, N), np.zeros((1, N), np.float32), cx, cy], axis=0
        ).astype(BF)
        qaug = np.empty((64, NT), np.float32)
        qaug[0, :] = 1.0
        qaug[1, :] = 0.0
        qaug[2:33, :] = -2.0 * cx[:, rows]
        qaug[33:64, :] = -2.0 * cy[:, rows]
        srcTb = np.ascontiguousarray(src[b].T[:, perm])
        m = dict(shared)
        m.update(
            {
                "srcT8": _hilo8(S_PRESCALE * srcTb),
                "src_rows": np.ascontiguousarray(src[b, rows, :]),
                "kaug_x": np.ascontiguousarray(kaug[:, perm]).reshape(64, 1, N),
                "qaug_x": qaug.astype(BF).reshape(64, 1, NT),
            }
        )
        in_maps.append(m)
    return in_maps


_NCS = {}
LAST_RUN_S = None


def get_nc(trivial_affine=True):
    if trivial_affine not in _NCS:
        _NCS[trivial_affine] = build_nc(trivial_affine)
    return _NCS[trivial_affine]


def _affine_trivial(inputs):
    return (
        np.all(np.asarray(inputs["g1"]) == 1.0)
        and np.all(np.asarray(inputs["g2"]) == 1.0)
        and not np.any(np.asarray(inputs["be1"]))
        and not np.any(np.asarray(inputs["be2"]))
        and not np.any(np.asarray(inputs["b2"]))
    )


def kernel(**inputs):
    global LAST_RUN_S
    from concourse.bass_utils import run_bass_kernel_spmd

    nc = get_nc(bool(_affine_trivial(inputs)))
    in_maps = host_prep(inputs)
    t0 = time.monotonic()
    res = run_bass_kernel_spmd(nc, in_maps, list(range(NCORES)))
    LAST_RUN_S = time.monotonic() - t0
    full = np.empty((B, N, D), np.float32)
    for c in range(NCORES):
        b = c // 2
        half = c % 2
        full[b, half * NT : (half + 1) * NT, :] = res.results[c]["out"]
    return full

